# revision 1
# baseline (speedup 1.0000x reference)
"""Multi-head causal attention (B=4,T=2048,E=1024,H=16,D=64) on 8 TRN2 cores.

Sharding: core c -> batch b=c//2, heads h0=(c%2)*8 .. h0+8 (tensor-parallel
over head halves within each batch). Each core computes its 8 heads'
attention and a partial output projection (row-split Wp); host sums the
two partials per batch (+bias).

Per-core kernel (all matmuls bf16 x bf16 -> fp32 PSUM):
  qT/kT = W.T @ x.T          [64*2, T] per head-pair (full PE efficiency)
  v     = x @ Wv             [T, 64] per head, +ones column for softmax sum
  sT    = kT.T @ qT          [tk, tq] blocks (K=64)
  expT  = exp(sT/8) (ACT), causal mask multiply on diagonal blocks (DVE)
  oT,Z  = [v|1].T @ expT     accumulated over tk chunks; row 64 = sum(exp)
  oT   /= Z  (DVE recip + gpsimd partition_broadcast + DVE mul)
  part  = oT.T @ WpT         [T, E] partial projection, fp32 out
"""
import sys
import numpy as np

sys.path.insert(0, "/opt/trn_rl_repo")

import ml_dtypes
import concourse.bass as bass
import concourse.bacc as bacc
import concourse.mybir as mybir
from concourse import tile
from concourse.bass_utils import run_bass_kernel_spmd

B, T, E, H, D = 4, 2048, 1024, 16, 64
HL = H // 2      # 8 local heads per core
NP = HL // 2     # 4 head pairs
NJ = T // 512    # 4 tq tiles
NCK = T // 128   # 16 t chunks
NE = E // 128    # 8 e chunks
BF16 = mybir.dt.bfloat16
F32 = mybir.dt.float32
bfnp = ml_dtypes.bfloat16

_CACHE = {}


def _build():
    nc = bacc.Bacc("TRN2", target_bir_lowering=False)
    xT = nc.declare_dram_parameter("xT", [E, T], BF16, isOutput=False)
    wq = nc.declare_dram_parameter("wq", [E, HL * D], BF16, isOutput=False)
    wk = nc.declare_dram_parameter("wk", [E, HL * D], BF16, isOutput=False)
    wv = nc.declare_dram_parameter("wv", [E, HL * D], BF16, isOutput=False)
    wpT = nc.declare_dram_parameter("wpT", [HL * D, E], BF16, isOutput=False)
    maskb = nc.declare_dram_parameter("maskb", [128, 512], F32, isOutput=False)
    out = nc.declare_dram_parameter("out", [T, E], F32, isOutput=True)

    Exp = mybir.ActivationFunctionType.Exp

    with tile.TileContext(nc) as tc:
        with (
            tc.tile_pool(name="persist", bufs=1) as pp,
            tc.tile_pool(name="expp", bufs=12) as expp,
            tc.tile_pool(name="zpool", bufs=4) as zpool,
            tc.tile_pool(name="outp", bufs=4) as outp,
            tc.tile_pool(name="mm", bufs=2, space=bass.MemorySpace.PSUM) as ps_mm,
            tc.tile_pool(name="sT", bufs=3, space=bass.MemorySpace.PSUM) as ps_sT,
            tc.tile_pool(name="oT", bufs=3, space=bass.MemorySpace.PSUM) as ps_oT,
        ):
            xT_sb = [pp.tile([128, T], BF16, tag=f"xT{c}", name=f"xT{c}") for c in range(NE)]
            wv_sb = [pp.tile([128, HL * D], BF16, tag=f"wv{c}", name=f"wv{c}") for c in range(NE)]
            wq_sb = [pp.tile([128, HL * D], BF16, tag=f"wq{c}", name=f"wq{c}") for c in range(NE)]
            wk_sb = [pp.tile([128, HL * D], BF16, tag=f"wk{c}", name=f"wk{c}") for c in range(NE)]
            wpT_sb = [pp.tile([128, E], BF16, tag=f"wp{p}", name=f"wp{p}") for p in range(NP)]
            mask_sb = pp.tile([128, 512], F32, tag="mkb", name="mkb")
            v_sb = [pp.tile([128, HL * 65], BF16, tag=f"v{i}", name=f"v{i}") for i in range(NCK)]
            qT_sb = [[pp.tile([128, 512], BF16, tag=f"q{p}_{j}", name=f"q{p}_{j}") for j in range(NJ)]
                     for p in range(NP)]
            kT_sb = [[pp.tile([128, 512], BF16, tag=f"k{p}_{j}", name=f"k{p}_{j}") for j in range(NJ)]
                     for p in range(NP)]
            oT_sb = [[pp.tile([128, 512], BF16, tag=f"o{p}_{j}", name=f"o{p}_{j}") for j in range(NJ)]
                     for p in range(NP)]

            for c in range(NE):
                nc.sync.dma_start(wv_sb[c][:], wv[c * 128:(c + 1) * 128, :])
                nc.sync.dma_start(xT_sb[c][:, 0:512], xT[c * 128:(c + 1) * 128, 0:512])
            for q in range(1, 4):
                for c in range(NE):
                    nc.sync.dma_start(xT_sb[c][:, q * 512:(q + 1) * 512],
                                      xT[c * 128:(c + 1) * 128, q * 512:(q + 1) * 512])
            for c in range(NE):
                nc.sync.dma_start(wq_sb[c][:], wq[c * 128:(c + 1) * 128, :])
                nc.sync.dma_start(wk_sb[c][:], wk[c * 128:(c + 1) * 128, :])
            for p in range(NP):
                nc.sync.dma_start(wpT_sb[p][:], wpT[p * 128:(p + 1) * 128, :])
            nc.sync.dma_start(mask_sb[:], maskb[:])

            # V projection per t-chunk, interleaved [v_h | 1] layout
            for i in range(NCK):
                ps = ps_mm.tile([128, HL * D], F32, tag="mm", name="mmv")
                for c in range(NE):
                    nc.tensor.matmul(ps[:], xT_sb[c][:, i * 128:(i + 1) * 128],
                                     wv_sb[c][:], start=(c == 0), stop=(c == NE - 1))
                v3 = v_sb[i][:].rearrange("p (h d) -> p h d", d=65)
                nc.gpsimd.memset(v_sb[i][:], 1.0)
                nc.vector.tensor_copy(
                    v3[:, :, 0:64], ps[:].rearrange("p (h d) -> p h d", d=64))

            # Q/K projections: per head-pair p, tq tile j -> qT/kT [128, 512]
            for j in range(NJ):
                for p in range(NP):
                    ps = ps_mm.tile([128, 512], F32, tag="mm", name="mmq")
                    for c in range(NE):
                        nc.tensor.matmul(ps[:], wq_sb[c][:, p * 128:(p + 1) * 128],
                                         xT_sb[c][:, j * 512:(j + 1) * 512],
                                         start=(c == 0), stop=(c == NE - 1))
                    nc.vector.tensor_copy(qT_sb[p][j][:], ps[:])
                    ps2 = ps_mm.tile([128, 512], F32, tag="mm", name="mmq")
                    for c in range(NE):
                        nc.tensor.matmul(ps2[:], wk_sb[c][:, p * 128:(p + 1) * 128],
                                         xT_sb[c][:, j * 512:(j + 1) * 512],
                                         start=(c == 0), stop=(c == NE - 1))
                    nc.vector.tensor_copy(kT_sb[p][j][:], ps2[:])

            # attention per (tq tile j, head h); diagonal chunks narrowed to
            # the unmasked column range, mask multiply only on the 128-wide
            # triangle region after exp
            for j in range(NJ):
                for h in range(HL):
                    p, r0 = h // 2, (h % 2) * 64
                    oT_ps = ps_oT.tile([65, 512], F32, tag="oT", name="oTps")
                    nblk = 4 * (j + 1)
                    for c in range(nblk):
                        r = max(0, (c - 4 * j) * 128)
                        w = 512 - r
                        sT = ps_sT.tile([128, 512], F32, tag="sT", name="sTps")
                        nc.tensor.matmul(
                            sT[:, 0:w],
                            kT_sb[p][c // 4][r0:r0 + 64,
                                             (c % 4) * 128:(c % 4 + 1) * 128],
                            qT_sb[p][j][r0:r0 + 64, r:512],
                            start=True, stop=True)
                        et = expp.tile([128, 512], BF16, tag="expT", name="et")
                        nc.scalar.activation(et[:, 0:w], sT[:, 0:w], Exp, scale=0.125)
                        if c >= 4 * j:
                            mw = min(128, w)
                            nc.vector.tensor_mul(et[:, 0:mw], et[:, 0:mw],
                                                 mask_sb[:, 0:mw])
                        nc.tensor.matmul(oT_ps[:, r:512],
                                         v_sb[c][:, h * 65:(h + 1) * 65],
                                         et[:, 0:w],
                                         start=(c == 0), stop=(c == nblk - 1),
                                         skip_group_check=True)
                    zi = zpool.tile([1, 512], F32, tag="zi", name="zi")
                    nc.vector.reciprocal(zi[:], oT_ps[64:65, :])
                    zb = zpool.tile([64, 512], F32, tag="zb", name="zb")
                    nc.gpsimd.partition_broadcast(zb[:], zi[:])
                    nc.vector.tensor_mul(oT_sb[p][j][r0:r0 + 64, :],
                                         oT_ps[0:64, :], zb[:])

                for t in range(4 * j, 4 * j + 4):
                    ob = outp.tile([128, E], F32, tag="ob", name="ob")
                    for n in range(2):
                        ps = ps_mm.tile([128, 512], F32, tag="mm", name="mmq")
                        for p in range(NP):
                            nc.tensor.matmul(
                                ps[:],
                                oT_sb[p][j][:, (t % 4) * 128:(t % 4 + 1) * 128],
                                wpT_sb[p][:, n * 512:(n + 1) * 512],
                                start=(p == 0), stop=(p == NP - 1))
                        nc.vector.tensor_copy(ob[:, n * 512:(n + 1) * 512], ps[:])
                    nc.sync.dma_start(out[t * 128:(t + 1) * 128, :], ob[:])

    nc.compile()
    return nc


def _masks_np():
    f = np.arange(512)[None, :]
    p = np.arange(128)[:, None]
    return (f >= p).astype(np.float32)


def kernel(x, Wq, Wk, Wv, Wp, bp):
    x = np.asarray(x, dtype=np.float32)
    Wq = np.asarray(Wq, dtype=np.float32)
    Wk = np.asarray(Wk, dtype=np.float32)
    Wv = np.asarray(Wv, dtype=np.float32)
    Wp = np.asarray(Wp, dtype=np.float32)
    bp = np.asarray(bp, dtype=np.float32)

    if "nc" not in _CACHE:
        _CACHE["nc"] = _build()
    nc = _CACHE["nc"]

    masks = _masks_np()
    WpT = np.ascontiguousarray(Wp.T).astype(bfnp)  # [E(hd), E(n)]
    xTs = [np.ascontiguousarray(x[b].T).astype(bfnp) for b in range(B)]

    def wslice(W, h0):  # [H,E,D] -> [E, 8*64] col = local head*64+d
        return np.ascontiguousarray(
            W[h0:h0 + HL].transpose(1, 0, 2).reshape(E, HL * D)).astype(bfnp)

    in_maps = []
    for c in range(8):
        b, hh = c // 2, c % 2
        h0 = hh * HL
        in_maps.append({
            "xT": xTs[b],
            "wq": wslice(Wq, h0),
            "wk": wslice(Wk, h0),
            "wv": wslice(Wv, h0),
            "wpT": np.ascontiguousarray(WpT[h0 * D:(h0 + HL) * D, :]),
            "maskb": masks,
        })

    res = run_bass_kernel_spmd(nc, in_maps, list(range(8)))
    parts = [np.asarray(res.results[c]["out"], dtype=np.float32) for c in range(8)]
    out = np.stack([parts[2 * b] + parts[2 * b + 1] for b in range(B)], axis=0)
    return (out + bp[None, None, :]).astype(np.float32)



# revision 89
# speedup vs baseline: 1.1436x; 1.1436x over previous
"""Multi-head causal attention (B=4,T=2048,E=1024,H=16,D=64) on 8 TRN2 cores.

Sharding: core c -> batch b=c//2, heads h0=(c%2)*8 .. h0+8. Each core computes
its 8 heads' attention and a partial output projection (row-split Wp); host
sums the two partials per batch (+bias).

Per-core kernel:
  - Q/K/V projections in fp8e4 DoubleRow (2 k-tiles of 128 per instr, 0.5
    cyc/col), M=64 outputs at PSUM base 0. Host pre-scales Wq,Wk by 64 and
    Wv by 16 (powers of 2, folded back via exp scale / Wp scale).
  - scores: bf16 kT.T @ qT per 128-row tk chunk, chunk PAIRS share one
    [128,1024] PSUM tile (2 banks) so exp runs as one ACT op per pair.
  - exp -> fp8e4 "et" tiles [128, 2, N]; causal diag handled by gpsimd
    memset (dead block) + gpsimd triangle mask multiply.
  - attnV: o-layout fp8 DoubleRow: o[tq64, d64] += et.T @ v over chunk
    pairs; Z via DR ones-column matmuls into a shared Z bank (per-stream
    8-column slots). Normalize is per-partition: DVE reciprocal [64,8] +
    one broadcast multiply.
  - o -> oT via PE transpose (identity matmul) packed 2 heads/bank, then
    bf16 output projection, DVE copy, DMA out.
"""
import sys
import numpy as np

sys.path.insert(0, "/opt/trn_rl_repo")

import ml_dtypes
import concourse.bass as bass
import concourse.bacc as bacc
import concourse.mybir as mybir
from concourse import tile
from concourse.bass_utils import run_bass_kernel_spmd

B, T, E, H, D = 4, 2048, 1024, 16, 64
HL = H // 2          # 8 local heads per core
NJ = T // 512        # 4 tq tiles
NPE = E // 256       # 4 E-chunk-pairs
NPAIR = T // 256     # 8 tk chunk pairs
BF16 = mybir.dt.bfloat16
F32 = mybir.dt.float32
FP8 = mybir.dt.float8e4
DR = mybir.MatmulPerfMode.DoubleRow
Exp = mybir.ActivationFunctionType.Exp
f8np = ml_dtypes.float8_e4m3
bfnp = ml_dtypes.bfloat16

SW = 64.0   # Wq/Wk host prescale (exp scale folds 1/SW^2)
SV = 16.0   # Wv host prescale (Wp folds 1/SV)
EXP_SCALE = 0.125 / (SW * SW)  # 2^-15 exactly

_CACHE = {}


def _build():
    nc = bacc.Bacc("TRN2", target_bir_lowering=False)
    xT8 = nc.declare_dram_parameter("xT8", [E, T], FP8, isOutput=False)
    wq8 = nc.declare_dram_parameter("wq8", [E, HL * D], FP8, isOutput=False)
    wk8 = nc.declare_dram_parameter("wk8", [E, HL * D], FP8, isOutput=False)
    wv8 = nc.declare_dram_parameter("wv8", [E, HL * D], FP8, isOutput=False)
    wpT = nc.declare_dram_parameter("wpT", [HL * D, E], BF16, isOutput=False)
    tri2 = nc.declare_dram_parameter("tri2", [128, 256], FP8, isOutput=False)
    tri2b = nc.declare_dram_parameter("tri2b", [128, 256], BF16, isOutput=False)
    xbT = nc.declare_dram_parameter("xbT", [E, 512], BF16, isOutput=False)
    wqb = nc.declare_dram_parameter("wqb", [E, HL * D], BF16, isOutput=False)
    wkb = nc.declare_dram_parameter("wkb", [E, HL * D], BF16, isOutput=False)
    wvb = nc.declare_dram_parameter("wvb", [E, HL * D], BF16, isOutput=False)
    ident = nc.declare_dram_parameter("ident", [64, 64], BF16, isOutput=False)
    out = nc.declare_dram_parameter("out", [T, E], F32, isOutput=True)

    with tile.TileContext(nc) as tc:
        with (
            tc.tile_pool(name="pp", bufs=1) as pp,
            tc.tile_pool(name="etp", bufs=8) as etp,
            tc.tile_pool(name="osb", bufs=8) as osb,
            tc.tile_pool(name="rzp", bufs=6) as rzp,
            tc.tile_pool(name="otp", bufs=2) as otp,
            tc.tile_pool(name="obp", bufs=3) as obp,
            tc.tile_pool(name="sT", bufs=2, space=bass.MemorySpace.PSUM) as ps_sT,
            tc.tile_pool(name="po", bufs=2, space=bass.MemorySpace.PSUM) as ps_po,
            tc.tile_pool(name="zz", bufs=1, space=bass.MemorySpace.PSUM) as ps_z,
            tc.tile_pool(name="mm", bufs=1, space=bass.MemorySpace.PSUM) as ps_mm,
        ):
            # ---- persistent SBUF tiles
            x8t = [pp.tile([128, 2, T], FP8, tag=f"x{p}", name=f"x{p}")
                   for p in range(NPE)]
            wq8t = [pp.tile([128, 2, 512], FP8, tag=f"wq{p}", name=f"wq{p}")
                    for p in range(NPE)]
            wk8t = [pp.tile([128, 2, 512], FP8, tag=f"wk{p}", name=f"wk{p}")
                    for p in range(NPE)]
            wv8t = [pp.tile([128, 2, 512], FP8, tag=f"wv{p}", name=f"wv{p}")
                    for p in range(NPE)]
            wpt = [pp.tile([128, E], BF16, tag=f"wp{p}", name=f"wp{p}")
                   for p in range(4)]
            trit = pp.tile([128, 2, 128], FP8, tag="tri", name="trit")
            tritb = pp.tile([128, 2, 128], BF16, tag="trib", name="tritb")
            idt = pp.tile([64, 64], BF16, tag="id", name="idt")
            qTt = [[pp.tile([128, 512], BF16, tag=f"q{p}_{j}", name=f"q{p}_{j}")
                    for j in range(NJ)] for p in range(4)]
            kTt = [[pp.tile([128, 512], BF16, tag=f"k{p}_{j}", name=f"k{p}_{j}")
                    for j in range(NJ)] for p in range(4)]
            vp = [pp.tile([128, 2, HL, 65], FP8, tag=f"v{q}", name=f"v{q}")
                  for q in range(NPAIR)]
            # bf16 v for chunks 0..3: j=0 attention runs in bf16 (short-support
            # softmax rows can't average away fp8 quantization noise)
            vb = [pp.tile([128, 2, HL, 65], BF16, tag=f"vb{q}", name=f"vb{q}")
                  for q in range(2)]
            # bf16 x/W and q/k for the first 128 tokens (block-0 scores)
            xbt = [pp.tile([128, 2, 512], BF16, tag=f"xb{p}", name=f"xb{p}")
                   for p in range(NPE)]
            wqbt = [pp.tile([128, 2, 512], BF16, tag=f"wqb{p}", name=f"wqb{p}")
                    for p in range(NPE)]
            wkbt = [pp.tile([128, 2, 512], BF16, tag=f"wkb{p}", name=f"wkb{p}")
                    for p in range(NPE)]
            wvbt = [pp.tile([128, 2, 512], BF16, tag=f"wvb{p}", name=f"wvb{p}")
                    for p in range(NPE)]
            qbt = [pp.tile([128, 128], BF16, tag=f"qb{p}", name=f"qb{p}")
                   for p in range(4)]
            kbt = [pp.tile([128, 128], BF16, tag=f"kb{p}", name=f"kb{p}")
                   for p in range(4)]

            # ---- DMA in. j tiles are processed in order [1,2,3,0], so the
            # critical path is x cols [0:1024] + wv (SP) and wq/wk (ACT).
            # The bf16 sidecar tiles (j=0 accuracy path) ride at the SP tail.
            def tdma(queue, dst, src, cols=None):
                # both t-halves in one DMA: src rows (t p) -> dst [p, t, n]
                s = src if cols is None else src[:, cols[0]:cols[1]]
                queue.dma_start(dst, s.rearrange("(t p) n -> p t n", t=2))
            for p in range(NPE):
                tdma(nc.sync, x8t[p][:, :, 0:512], xT8[256 * p:256 * p + 256],
                     (0, 512))
                tdma(nc.sync, wv8t[p][:], wv8[256 * p:256 * p + 256])
            for p in range(NPE):
                tdma(nc.scalar, wq8t[p][:], wq8[256 * p:256 * p + 256])
                tdma(nc.scalar, wk8t[p][:], wk8[256 * p:256 * p + 256])
            for p in range(NPE):
                tdma(nc.sync, x8t[p][:, :, 512:1024],
                     xT8[256 * p:256 * p + 256], (512, 1024))
            nc.sync.dma_start(trit[:], tri2[:].rearrange("p (a n) -> p a n", a=2))
            nc.sync.dma_start(idt[:], ident[:])
            for jj in range(2, NJ):
                for p in range(NPE):
                    tdma(nc.sync, x8t[p][:, :, 512 * jj:512 * jj + 512],
                         xT8[256 * p:256 * p + 256], (512 * jj, 512 * jj + 512))
            for p in range(4):
                nc.sync.dma_start(wpt[p][:], wpT[128 * p:128 * p + 128, :])
            nc.sync.dma_start(tritb[:],
                              tri2b[:].rearrange("p (a n) -> p a n", a=2))
            for p in range(NPE):
                tdma(nc.sync, xbt[p][:], xbT[256 * p:256 * p + 256])
                tdma(nc.sync, wvbt[p][:], wvb[256 * p:256 * p + 256])
                tdma(nc.sync, wqbt[p][:], wqb[256 * p:256 * p + 256])
                tdma(nc.sync, wkbt[p][:], wkb[256 * p:256 * p + 256])
            for q in range(NPAIR):
                nc.gpsimd.memset(vp[q][:, :, :, 64:65], 1.0)
            for q in range(2):
                nc.gpsimd.memset(vb[q][:, :, :, 64:65], 1.0)

            ZT = ps_z.tile([64, 256], F32, tag="z", name="ZT")

            # ---- background task machinery (qkv groups, proj tiles)
            def qk_group(kind, h, j, pool=None):
                def emit():
                    wt = wq8t if kind == "q" else wk8t
                    dst = qTt if kind == "q" else kTt
                    if pool is None:
                        m = ps_mm.tile([128, 512], F32, tag="mm", name="mmq")
                    else:
                        m = pool.tile([128, 1024], F32, tag="sT",
                                      name="mmq")[:, 0:512]
                    for p in range(NPE):
                        nc.tensor.matmul(
                            m[0:64, :], wt[p][:, :, 64 * h:64 * h + 64],
                            x8t[p][:, :, 512 * j:512 * j + 512],
                            start=(p == 0), stop=(p == NPE - 1), perf_mode=DR)
                    nc.vector.tensor_copy(
                        dst[h // 2][j][64 * (h % 2):64 * (h % 2) + 64, :],
                        m[0:64, :])
                return emit

            def qkb_group(pe_hp):  # bf16 q/k for tokens 0..127, head pair
                def emit():
                    for wt, dst in ((wqbt, qbt), (wkbt, kbt)):
                        m = ps_mm.tile([128, 512], F32, tag="mm", name="mmb")
                        for p in range(NPE):
                            for t in range(2):
                                nc.tensor.matmul(
                                    m[0:128, 0:128],
                                    wt[p][:, t, 128 * pe_hp:128 * pe_hp + 128],
                                    xbt[p][:, t, 0:128],
                                    start=(p == 0 and t == 0),
                                    stop=(p == NPE - 1 and t == 1))
                        nc.vector.tensor_copy(dst[pe_hp][:], m[0:128, 0:128])
                return emit

            def v_group(c, g, pool=None):  # chunk c, t-64 sub g
                def emit():
                    if pool is None:
                        m = ps_mm.tile([128, 512], F32, tag="mm", name="mmv")
                    else:
                        m = pool.tile([128, 1024], F32, tag="sT",
                                      name="mmv")[:, 0:512]
                    t0 = 128 * c + 64 * g
                    for p in range(NPE):
                        nc.tensor.matmul(
                            m[0:64, :], x8t[p][:, :, t0:t0 + 64], wv8t[p][:],
                            start=(p == 0), stop=(p == NPE - 1), perf_mode=DR)
                    nc.vector.tensor_copy(
                        vp[c // 2][64 * g:64 * g + 64, c % 2, :, 0:64],
                        m[0:64, :].rearrange("p (h d) -> p h d", d=64))
                return emit

            def vb_group(c, g):  # true bf16 v for j=0 (bf16 x and Wv)
                def emit():
                    m = ps_mm.tile([128, 512], F32, tag="mm", name="mvb")
                    t0 = 128 * c + 64 * g
                    for p in range(NPE):
                        for t in range(2):
                            nc.tensor.matmul(
                                m[0:64, :],
                                xbt[p][:, t, t0:t0 + 64],
                                wvbt[p][:, t, :],
                                start=(p == 0 and t == 0),
                                stop=(p == NPE - 1 and t == 1))
                    nc.vector.tensor_copy(
                        vb[c // 2][64 * g:64 * g + 64, c % 2, :, 0:64],
                        m[0:64, :].rearrange("p (h d) -> p h d", d=64))
                return emit

            def proj_tile(j, t, nh, ot_tile):
                def emit():
                    m = ps_mm.tile([128, 512], F32, tag="mm", name="mmp")
                    for p in range(4):
                        nc.tensor.matmul(
                            m[:], ot_tile[:, p, 128 * t:128 * t + 128],
                            wpt[p][:, 512 * nh:512 * nh + 512],
                            start=(p == 0), stop=(p == 3))
                    ob = obp.tile([128, 512], F32, tag="ob", name="ob")
                    nc.vector.tensor_copy(ob[:], m[:])
                    nc.sync.dma_start(
                        out[512 * j + 128 * t:512 * j + 128 * t + 128,
                            512 * nh:512 * nh + 512], ob[:])
                return emit

            bg = []  # queue of (tag, emitter); tags order forced at stream starts

            def drain_bg(n=1):
                for _ in range(min(n, len(bg))):
                    bg.pop(0)[1]()

            def force_bg(pred):
                """Emit from the front until no queued task satisfies pred."""
                while any(pred(t) for t, _ in bg):
                    bg.pop(0)[1]()

            # j tiles processed [1, 2, 3, 0]: the bf16 j=0 accuracy path runs
            # last so its DMAs/projections never gate the critical path.
            JORDER = (1, 2, 3, 0)
            # prologue: v pairs 0..1 (chunks 0..3); q tile 1, k tiles 0..1
            # for h=0. k projections are scheduled by tk-tile index: stream
            # (j, h) consumes k tiles 0..j.
            # interleave so the first stream's scores are unblocked ASAP
            qk_group("k", 0, 0)()
            v_group(0, 0, ps_sT)()
            v_group(0, 1)()
            qk_group("k", 0, 1, ps_sT)()
            v_group(1, 0)()
            v_group(1, 1, ps_sT)()
            qk_group("q", 0, 1)()
            for c in (2, 3):
                for g in range(2):
                    bg.append((("vq", 1), v_group(c, g)))
            for q in range(2, 4):
                for g in range(2):
                    bg.append((("vq", q), v_group(2 * q, g)))
                    bg.append((("vq", q), v_group(2 * q + 1, g)))
            for h in range(1, HL):
                bg.append((("q", 1, h), qk_group("q", h, 1)))
                bg.append((("k", 0, h), qk_group("k", h, 0)))
                bg.append((("k", 1, h), qk_group("k", h, 1)))
            # bf16 sidecar tasks wait on tail-end DMAs; keep them out of the
            # main drain queue until their inputs have surely landed
            bg_late = []
            for hp in range(4):
                bg_late.append((("qkb", 2 * hp), qkb_group(hp)))
            for c in range(4):
                for g in range(2):
                    bg_late.append((("vb", c // 2), vb_group(c, g)))

            ot_tiles = {}
            pend = []       # one-pair-lagged PE emissions (attnV/Z, normalize)
            o_sb_pair = {}  # normalized o for the in-flight head pair
            z_started = [False]  # ZT bank gets exactly one start=True ever

            pending_proj = []
            for si, j in enumerate(JORDER):
                if si == 1:  # release the bf16 sidecar work mid-flight
                    bg.extend(bg_late)
                    bg_late = []
                # enqueue the next sequence step's inputs
                if si + 1 < NJ:
                    nj = JORDER[si + 1]
                    if nj != 0:
                        for q in range(2 * nj, 2 * nj + 2):
                            for g in range(2):
                                bg.append((("vq", q), v_group(2 * q, g)))
                                bg.append((("vq", q), v_group(2 * q + 1, g)))
                    for h in range(HL):
                        bg.append((("q", nj, h), qk_group("q", h, nj)))
                        if nj != 0:  # k tile 0 was produced in the prologue
                            bg.append((("k", nj, h), qk_group("k", h, nj)))
                # weave last step's proj tiles into the fresh queue (avoids a
                # PE-only burst that starves ACT at the step boundary); all
                # transposes must go first (proj reads their output)
                if pending_proj:
                    force_bg(lambda t: t[0] == "tr")
                for i, task in enumerate(pending_proj):
                    bg.insert(min(5 * i + 2, len(bg)), task)
                pending_proj = []
                # proj from two sequence steps back must be out before its
                # ot slot is reused
                force_bg(lambda t: t[0] == "proj" and t[1] <= si - 2)
                ot_tile = otp.tile([128, 4, 512], BF16, tag="ot", name=f"ot{j}")
                ot_tiles[j] = ot_tile

                for h in range(HL):
                    hp, r0 = h // 2, 64 * (h % 2)
                    zc = 8 * (8 * j + h)
                    # everything this stream reads must already be emitted
                    force_bg(lambda t: (t[0] == "q" and t[1] == j and t[2] <= h) or
                             (t[0] == "k" and t[1] <= j and t[2] <= h) or
                             (j == 0 and t[0] == "qkb" and t[1] <= h))
                    po_t = ps_po.tile([128, 512], F32, tag="po", name="po")
                    po = po_t[0:64, :]
                    nq = 2 * j + 2
                    for q in range(nq):
                        m0, m2 = (q == 2 * j), (q == 2 * j + 1)
                        force_bg(lambda t: (t[0] == "vq" and t[1] <= q) or
                                 (j == 0 and t[0] == "vb" and t[1] <= q))
                        sT = ps_sT.tile([128, 1024], F32, tag="sT", name="sT")
                        if j == 0:
                            et = etp.tile([128, 1024], BF16, tag="etb",
                                          name="etb", bufs=4)
                            trm = tritb
                        else:
                            et = etp.tile([128, 1024], FP8, tag="et", name="et")
                            trm = trit
                        if not (m0 or m2):
                            # off-diag pair: both chunks full [0:512]
                            for t in range(2):
                                c = 2 * q + t
                                nc.tensor.matmul(
                                    sT[:, 512 * t:512 * t + 512],
                                    kTt[hp][c // 4][r0:r0 + 64,
                                                    128 * (c % 4):128 * (c % 4) + 128],
                                    qTt[hp][j][r0:r0 + 64, :],
                                    start=True, stop=True, skip_group_check=True)
                            nc.scalar.activation(et[:], sT[:], Exp, scale=EXP_SCALE)
                            stride, width = 512, 512
                        elif m0:
                            # chunks 4j (full), 4j+1 (cols 128:512)
                            c = 4 * j
                            if j == 0:
                                # block-0 scores from bf16-accurate q/k
                                nc.tensor.matmul(
                                    sT[:, 0:128],
                                    kbt[hp][r0:r0 + 64, :],
                                    qbt[hp][r0:r0 + 64, :],
                                    start=True, stop=False,
                                    skip_group_check=True)
                                nc.tensor.matmul(
                                    sT[:, 128:512],
                                    kTt[hp][j][r0:r0 + 64, 0:128],
                                    qTt[hp][j][r0:r0 + 64, 128:512],
                                    start=False, stop=True,
                                    skip_group_check=True)
                            else:
                                nc.tensor.matmul(
                                    sT[:, 0:512],
                                    kTt[hp][j][r0:r0 + 64, 0:128],
                                    qTt[hp][j][r0:r0 + 64, :],
                                    start=True, stop=True,
                                    skip_group_check=True)
                            # cover [512:640] too so exp never reads stale
                            # bytes (those weights get memset to 0 after)
                            nc.tensor.matmul(
                                sT[:, 512:1024],
                                kTt[hp][j][r0:r0 + 64, 128:256],
                                qTt[hp][j][r0:r0 + 64, :],
                                start=True, stop=True, skip_group_check=True)
                            nc.scalar.activation(et[:], sT[:], Exp, scale=EXP_SCALE)
                            et3 = et[:].rearrange("p (a n) -> p a n", n=128)
                            nc.vector.tensor_mul(et3[:, 0:6:5, :], et3[:, 0:6:5, :],
                                                 trm[:])
                            stride, width = 512, 512
                        else:
                            # m2: chunks 4j+2 (cols 256:512 -> [0:256]),
                            #     4j+3 (cols 384:512 -> [384:512])
                            nc.tensor.matmul(
                                sT[:, 0:256],
                                kTt[hp][j][r0:r0 + 64, 256:384],
                                qTt[hp][j][r0:r0 + 64, 256:512],
                                start=True, stop=True, skip_group_check=True)
                            nc.tensor.matmul(
                                sT[:, 256:512],
                                kTt[hp][j][r0:r0 + 64, 384:512],
                                qTt[hp][j][r0:r0 + 64, 256:512],
                                start=False, stop=True, skip_group_check=True)
                            nc.scalar.activation(et[:, 0:512], sT[:, 0:512],
                                                 Exp, scale=EXP_SCALE)
                            et3 = et[:].rearrange("p (a n) -> p a n", n=128)
                            nc.vector.tensor_mul(et3[:, 0:4:3, :], et3[:, 0:4:3, :],
                                                 trm[:])
                            stride, width = 256, 256
                        etv = et[:, 0:2 * stride].rearrange(
                            "p (a n) -> p a n", a=2)
                        s_lo = 4 if m2 else 0
                        base = 256 if m2 else 0

                        def attn_emit(et=et, etv=etv, q=q, h=h, po=po, zc=zc,
                                      s_lo=s_lo, base=base, m0=m0, m2=m2, j=j):
                            # exactly ONE start=True per bank-use: start=True
                            # flags the whole 2KB bank pending-zero; every
                            # other group's first touch consumes its flag.
                            if j == 0:
                                # bf16 non-DR path (no fp8 noise on the short-
                                # support rows). (tile, subtiles, flat offset fn)
                                if m0:
                                    work = [(0, range(0, 8), lambda s: 64 * s),
                                            (1, range(2, 8),
                                             lambda s: 512 + 64 * s)]
                                else:
                                    work = [(0, range(4, 8),
                                             lambda s: 64 * s - 256),
                                            (1, range(6, 8), lambda s: 64 * s)]
                                stop_at = {0: (0, 0), 1: (0, 0), 2: (0, 1),
                                           3: (0, 1), 4: (1, 0), 5: (1, 0),
                                           6: (1, 1), 7: (1, 1)}
                                for tt, srange, off in work:
                                    for s in srange:
                                        fo = off(s)
                                        stop = stop_at[s] == (q, tt)
                                        st = (q == 0 and tt == 0 and s == 0)
                                        nc.tensor.matmul(
                                            po[:, 64 * s:64 * s + 64],
                                            et[:, fo:fo + 64],
                                            vb[q][:, tt, h, 0:64],
                                            start=st, stop=stop,
                                            skip_group_check=True)
                                        nc.tensor.matmul(
                                            ZT[:, zc + s:zc + s + 1],
                                            et[:, fo:fo + 64],
                                            vb[q][:, tt, h, 64:65],
                                            start=(not z_started[0]), stop=stop,
                                            skip_group_check=True)
                                        z_started[0] = True
                                return
                            for s in range(s_lo, 8):
                                cc = 64 * s - base
                                stop = (s < 4 and m0) or (s >= 4 and m2)
                                # tile1's dead block is never read: subtiles
                                # under the diagonal use a single-tile matmul
                                single = (m0 and s < 2) or (m2 and s < 6)
                                if single:
                                    lhs_o = etv[:, 0, cc:cc + 64]
                                    rhs_o = vp[q][:, 0, h, 0:64]
                                    rhs_z = vp[q][:, 0, h, 64:65]
                                    pm = None
                                else:
                                    lhs_o = etv[:, :, cc:cc + 64]
                                    rhs_o = vp[q][:, :, h, 0:64]
                                    rhs_z = vp[q][:, :, h, 64:65]
                                    pm = DR
                                nc.tensor.matmul(
                                    po[:, 64 * s:64 * s + 64],
                                    lhs_o, rhs_o,
                                    start=(q == 0 and s == 0), stop=stop,
                                    perf_mode=pm, skip_group_check=True)
                                nc.tensor.matmul(
                                    ZT[:, zc + s:zc + s + 1],
                                    lhs_o, rhs_z,
                                    start=(not z_started[0]),
                                    perf_mode=pm, stop=stop,
                                    skip_group_check=True)
                                z_started[0] = True
                        # one-pair software pipeline: previous pair's attnV/Z
                        # runs while this pair's exp/masks are in flight
                        while pend:
                            pend.pop(0)()
                        pend.append(attn_emit)
                        drain_bg(2 if len(bg) > 14 else 1)

                    def norm_emit(po=po, zc=zc, h=h, hp=hp, ot_tile=ot_tile):
                        rz = rzp.tile([64, 8], F32, tag="rz", name="rz")
                        nc.vector.reciprocal(rz[:], ZT[:, zc:zc + 8])
                        o_sb = osb.tile([64, 512], BF16, tag="os", name="os")
                        nc.vector.tensor_tensor(
                            o_sb[:].rearrange("p (e s) -> p e s", e=8),
                            po[:].rearrange("p (e s) -> p e s", e=8),
                            rz[:].unsqueeze(2).broadcast_to([64, 8, 64]),
                            mybir.AluOpType.mult)
                        o_sb_pair[h % 2] = o_sb
                        if h % 2 == 1:
                            def transpose_emit(hp=hp, ot_tile=ot_tile,
                                               pair=dict(o_sb_pair)):
                                pt_t = ps_po.tile([128, 512], F32, tag="po",
                                                  name="pt")
                                pt = pt_t[:].bitcast(BF16)[:, 0:512]
                                for hh in range(2):
                                    for s in range(8):
                                        nc.tensor.matmul(
                                            pt[64 * hh:64 * hh + 64,
                                               64 * s:64 * s + 64],
                                            pair[hh][:, 64 * s:64 * s + 64],
                                            idt[:], is_transpose=True,
                                            start=(s == 0),
                                            stop=(hh == 1 and s == 7),
                                            skip_group_check=True)
                                nc.vector.tensor_copy(ot_tile[:, hp, :], pt[:])
                            bg.insert(min(1, len(bg)),
                                      (("tr", None), transpose_emit))
                    pend.append(norm_emit)

                # flush the pipeline at the j boundary so the last head pair's
                # normalize + transpose are queued before proj tasks
                while pend:
                    pend.pop(0)()
                for t in range(4):
                    for nh in range(2):
                        pending_proj.append((("proj", si),
                                             proj_tile(j, t, nh, ot_tile)))

            while pend:
                pend.pop(0)()
            drain_bg(len(bg))
            for _, task in pending_proj:
                task()

    nc.compile()
    return nc


def _host_prep(x, Wq, Wk, Wv, Wp):
    """Per-core input maps."""
    tri = (np.arange(128)[None, :] >= np.arange(128)[:, None]).astype(np.float32)
    tri2 = np.concatenate([tri, tri], axis=1).astype(f8np)
    ident = np.eye(64, dtype=np.float32).astype(bfnp)
    WpT = np.ascontiguousarray(Wp.T) * (1.0 / SV)  # [E(hd), E]

    def wslice(W, h0, scale, dt=f8np):  # [H,E,D] -> [E, 8*64]
        w = W[h0:h0 + HL].transpose(1, 0, 2).reshape(E, HL * D) * scale
        return np.ascontiguousarray(w).astype(dt)

    in_maps = []
    for c in range(8):
        b, hh = c // 2, c % 2
        h0 = hh * HL
        in_maps.append({
            "xT8": np.ascontiguousarray(x[b].T).astype(f8np),
            "wq8": wslice(Wq, h0, SW),
            "wk8": wslice(Wk, h0, SW),
            "wv8": wslice(Wv, h0, SV),
            "wpT": np.ascontiguousarray(
                WpT[h0 * D:(h0 + HL) * D, :]).astype(bfnp),
            "tri2": tri2,
            "tri2b": tri2.astype(np.float32).astype(bfnp),
            "ident": ident,
            "xbT": np.ascontiguousarray(x[b].T[:, 0:512]).astype(bfnp),
            "wqb": wslice(Wq, h0, SW, bfnp),
            "wkb": wslice(Wk, h0, SW, bfnp),
            "wvb": wslice(Wv, h0, SV, bfnp),
        })
    return in_maps


def kernel(x, Wq, Wk, Wv, Wp, bp):
    x = np.asarray(x, dtype=np.float32)
    Wq = np.asarray(Wq, dtype=np.float32)
    Wk = np.asarray(Wk, dtype=np.float32)
    Wv = np.asarray(Wv, dtype=np.float32)
    Wp = np.asarray(Wp, dtype=np.float32)
    bp = np.asarray(bp, dtype=np.float32)

    if "nc" not in _CACHE:
        _CACHE["nc"] = _build()
    nc = _CACHE["nc"]

    in_maps = _host_prep(x, Wq, Wk, Wv, Wp)
    res = run_bass_kernel_spmd(nc, in_maps, list(range(8)))
    parts = [np.asarray(res.results[c]["out"], dtype=np.float32) for c in range(8)]
    out = np.stack([parts[2 * b] + parts[2 * b + 1] for b in range(B)], axis=0)
    return (out + bp[None, None, :]).astype(np.float32)


# revision 94
# speedup vs baseline: 1.2694x; 1.1100x over previous
"""Multi-head causal attention (B=4,T=2048,E=1024,H=16,D=64) on 8 TRN2 cores.

Sharding: core c -> batch b=c//2, heads h0=(c%2)*8 .. h0+8. Each core computes
its 8 heads' attention and a partial output projection (row-split Wp); host
sums the two partials per batch (+bias).

Per-core kernel:
  - Q/K/V projections in fp8e4 DoubleRow (2 k-tiles of 128 per instr, 0.5
    cyc/col), M=64 outputs at PSUM base 0. Host pre-scales Wq,Wk by 64 and
    Wv by 16 (powers of 2, folded back via exp scale / Wp scale).
  - scores: bf16 kT.T @ qT per 128-row tk chunk, chunk PAIRS share one
    [128,1024] PSUM tile (2 banks) so exp runs as one ACT op per pair.
  - exp -> fp8e4 "et" tiles [128, 2, N]; causal diag handled by gpsimd
    memset (dead block) + gpsimd triangle mask multiply.
  - attnV: o-layout fp8 DoubleRow: o[tq64, d64] += et.T @ v over chunk
    pairs; Z via DR ones-column matmuls into a shared Z bank (per-stream
    8-column slots). Normalize is per-partition: DVE reciprocal [64,8] +
    one broadcast multiply.
  - o -> oT via PE transpose (identity matmul) packed 2 heads/bank, then
    bf16 output projection, DVE copy, DMA out.
"""
import sys
import numpy as np

sys.path.insert(0, "/opt/trn_rl_repo")

import ml_dtypes
import concourse.bass as bass
import concourse.bacc as bacc
import concourse.mybir as mybir
from concourse import tile
from concourse.bass_utils import run_bass_kernel_spmd

B, T, E, H, D = 4, 2048, 1024, 16, 64
HL = H // 2          # 8 local heads per core
NJ = T // 512        # 4 tq tiles
NPE = E // 256       # 4 E-chunk-pairs
NPAIR = T // 256     # 8 tk chunk pairs
BF16 = mybir.dt.bfloat16
F32 = mybir.dt.float32
FP8 = mybir.dt.float8e4
DR = mybir.MatmulPerfMode.DoubleRow
Exp = mybir.ActivationFunctionType.Exp
f8np = ml_dtypes.float8_e4m3
bfnp = ml_dtypes.bfloat16

SW = 64.0   # Wq/Wk host prescale (exp scale folds 1/SW^2)
SV = 16.0   # Wv host prescale (Wp folds 1/SV)
EXP_SCALE = 0.125 / (SW * SW)  # 2^-15 exactly

_CACHE = {}


def _build():
    nc = bacc.Bacc("TRN2", target_bir_lowering=False)
    xT8 = nc.declare_dram_parameter("xT8", [E, T], FP8, isOutput=False)
    wq8 = nc.declare_dram_parameter("wq8", [E, HL * D], FP8, isOutput=False)
    wk8 = nc.declare_dram_parameter("wk8", [E, HL * D], FP8, isOutput=False)
    wv8 = nc.declare_dram_parameter("wv8", [E, HL * D], FP8, isOutput=False)
    wpT = nc.declare_dram_parameter("wpT", [HL * D, E], BF16, isOutput=False)
    tri2 = nc.declare_dram_parameter("tri2", [128, 256], FP8, isOutput=False)
    tri2b = nc.declare_dram_parameter("tri2b", [128, 256], BF16, isOutput=False)
    xbT = nc.declare_dram_parameter("xbT", [E, 512], BF16, isOutput=False)
    wqb = nc.declare_dram_parameter("wqb", [E, HL * D], BF16, isOutput=False)
    wkb = nc.declare_dram_parameter("wkb", [E, HL * D], BF16, isOutput=False)
    wvb = nc.declare_dram_parameter("wvb", [E, HL * D], BF16, isOutput=False)
    ident = nc.declare_dram_parameter("ident", [64, 64], BF16, isOutput=False)
    out = nc.declare_dram_parameter("out", [T, E], F32, isOutput=True)

    with tile.TileContext(nc) as tc:
        with (
            tc.tile_pool(name="pp", bufs=1) as pp,
            tc.tile_pool(name="etp", bufs=8) as etp,
            tc.tile_pool(name="osb", bufs=8) as osb,
            tc.tile_pool(name="rzp", bufs=6) as rzp,
            tc.tile_pool(name="otp", bufs=2) as otp,
            tc.tile_pool(name="obp", bufs=3) as obp,
            tc.tile_pool(name="sT", bufs=2, space=bass.MemorySpace.PSUM) as ps_sT,
            tc.tile_pool(name="po", bufs=2, space=bass.MemorySpace.PSUM) as ps_po,
            tc.tile_pool(name="mm", bufs=2, space=bass.MemorySpace.PSUM) as ps_mm,
        ):
            # ---- persistent SBUF tiles
            x8t = [pp.tile([128, 2, T], FP8, tag=f"x{p}", name=f"x{p}")
                   for p in range(NPE)]
            wq8t = [pp.tile([128, 2, 512], FP8, tag=f"wq{p}", name=f"wq{p}")
                    for p in range(NPE)]
            wk8t = [pp.tile([128, 2, 512], FP8, tag=f"wk{p}", name=f"wk{p}")
                    for p in range(NPE)]
            wv8t = [pp.tile([128, 2, 512], FP8, tag=f"wv{p}", name=f"wv{p}")
                    for p in range(NPE)]
            wpt = [pp.tile([128, E], BF16, tag=f"wp{p}", name=f"wp{p}")
                   for p in range(4)]
            trit = pp.tile([128, 2, 128], FP8, tag="tri", name="trit")
            tritb = pp.tile([128, 2, 128], BF16, tag="trib", name="tritb")
            idt = pp.tile([64, 64], BF16, tag="id", name="idt")
            qTt = [[pp.tile([128, 512], BF16, tag=f"q{p}_{j}", name=f"q{p}_{j}")
                    for j in range(NJ)] for p in range(4)]
            kTt = [[pp.tile([128, 512], BF16, tag=f"k{p}_{j}", name=f"k{p}_{j}")
                    for j in range(NJ)] for p in range(4)]
            vp = [pp.tile([128, 2, HL, 65], FP8, tag=f"v{q}", name=f"v{q}")
                  for q in range(NPAIR)]
            # bf16 v for chunks 0..3: j=0 attention runs in bf16 (short-support
            # softmax rows can't average away fp8 quantization noise)
            vb = [pp.tile([128, 2, HL, 65], BF16, tag=f"vb{q}", name=f"vb{q}")
                  for q in range(2)]
            # bf16 x/W and q/k for the first 128 tokens (block-0 scores)
            xbt = [pp.tile([128, 2, 512], BF16, tag=f"xb{p}", name=f"xb{p}")
                   for p in range(NPE)]
            wqbt = [pp.tile([128, 2, 512], BF16, tag=f"wqb{p}", name=f"wqb{p}")
                    for p in range(NPE)]
            wkbt = [pp.tile([128, 2, 512], BF16, tag=f"wkb{p}", name=f"wkb{p}")
                    for p in range(NPE)]
            wvbt = [pp.tile([128, 2, 512], BF16, tag=f"wvb{p}", name=f"wvb{p}")
                    for p in range(NPE)]
            qbt = [pp.tile([128, 128], BF16, tag=f"qb{p}", name=f"qb{p}")
                   for p in range(4)]
            kbt = [pp.tile([128, 128], BF16, tag=f"kb{p}", name=f"kb{p}")
                   for p in range(4)]

            # ---- DMA in. j tiles are processed in order [1,2,3,0], so the
            # critical path is x cols [0:1024] + wv (SP) and wq/wk (ACT).
            # The bf16 sidecar tiles (j=0 accuracy path) ride at the SP tail.
            def tdma(queue, dst, src, cols=None):
                # both t-halves in one DMA: src rows (t p) -> dst [p, t, n]
                s = src if cols is None else src[:, cols[0]:cols[1]]
                queue.dma_start(dst, s.rearrange("(t p) n -> p t n", t=2))
            for p in range(NPE):
                tdma(nc.sync, x8t[p][:, :, 0:512], xT8[256 * p:256 * p + 256],
                     (0, 512))
                tdma(nc.sync, wv8t[p][:], wv8[256 * p:256 * p + 256])
            for p in range(NPE):
                tdma(nc.scalar, wq8t[p][:], wq8[256 * p:256 * p + 256])
                tdma(nc.scalar, wk8t[p][:], wk8[256 * p:256 * p + 256])
            for p in range(NPE):
                tdma(nc.sync, x8t[p][:, :, 512:1024],
                     xT8[256 * p:256 * p + 256], (512, 1024))
            nc.sync.dma_start(trit[:], tri2[:].rearrange("p (a n) -> p a n", a=2))
            nc.sync.dma_start(idt[:], ident[:])
            for jj in range(2, NJ):
                for p in range(NPE):
                    tdma(nc.sync, x8t[p][:, :, 512 * jj:512 * jj + 512],
                         xT8[256 * p:256 * p + 256], (512 * jj, 512 * jj + 512))
            for p in range(4):
                nc.sync.dma_start(wpt[p][:], wpT[128 * p:128 * p + 128, :])
            nc.sync.dma_start(tritb[:],
                              tri2b[:].rearrange("p (a n) -> p a n", a=2))
            for p in range(NPE):
                tdma(nc.sync, xbt[p][:], xbT[256 * p:256 * p + 256])
                tdma(nc.sync, wvbt[p][:], wvb[256 * p:256 * p + 256])
                tdma(nc.sync, wqbt[p][:], wqb[256 * p:256 * p + 256])
                tdma(nc.sync, wkbt[p][:], wkb[256 * p:256 * p + 256])
            for q in range(NPAIR):
                nc.gpsimd.memset(vp[q][:, :, :, 64:65], 1.0)
            for q in range(2):
                nc.gpsimd.memset(vb[q][:, :, :, 64:65], 1.0)

            # ---- background task machinery (qkv groups, proj tiles)
            def qk_group(kind, h, j, pool=None):
                def emit():
                    wt = wq8t if kind == "q" else wk8t
                    dst = qTt if kind == "q" else kTt
                    if pool is None:
                        m = ps_mm.tile([128, 512], F32, tag="mm", name="mmq")
                    else:
                        m = pool.tile([128, 1024], F32, tag="sT",
                                      name="mmq")[:, 0:512]
                    for p in range(NPE):
                        nc.tensor.matmul(
                            m[0:64, :], wt[p][:, :, 64 * h:64 * h + 64],
                            x8t[p][:, :, 512 * j:512 * j + 512],
                            start=(p == 0), stop=(p == NPE - 1), perf_mode=DR)
                    nc.vector.tensor_copy(
                        dst[h // 2][j][64 * (h % 2):64 * (h % 2) + 64, :],
                        m[0:64, :])
                return emit

            def qkb_group(pe_hp):  # bf16 q/k for tokens 0..127, head pair
                def emit():
                    for wt, dst in ((wqbt, qbt), (wkbt, kbt)):
                        m = ps_mm.tile([128, 512], F32, tag="mm", name="mmb")
                        for p in range(NPE):
                            for t in range(2):
                                nc.tensor.matmul(
                                    m[0:128, 0:128],
                                    wt[p][:, t, 128 * pe_hp:128 * pe_hp + 128],
                                    xbt[p][:, t, 0:128],
                                    start=(p == 0 and t == 0),
                                    stop=(p == NPE - 1 and t == 1))
                        nc.vector.tensor_copy(dst[pe_hp][:], m[0:128, 0:128])
                return emit

            def v_group(c, g, pool=None):  # chunk c, t-64 sub g
                def emit():
                    if pool is None:
                        m = ps_mm.tile([128, 512], F32, tag="mm", name="mmv")
                    else:
                        m = pool.tile([128, 1024], F32, tag="sT",
                                      name="mmv")[:, 0:512]
                    t0 = 128 * c + 64 * g
                    for p in range(NPE):
                        nc.tensor.matmul(
                            m[0:64, :], x8t[p][:, :, t0:t0 + 64], wv8t[p][:],
                            start=(p == 0), stop=(p == NPE - 1), perf_mode=DR)
                    nc.vector.tensor_copy(
                        vp[c // 2][64 * g:64 * g + 64, c % 2, :, 0:64],
                        m[0:64, :].rearrange("p (h d) -> p h d", d=64))
                return emit

            def vb_group(c, g):  # true bf16 v for j=0 (bf16 x and Wv)
                def emit():
                    m = ps_mm.tile([128, 512], F32, tag="mm", name="mvb")
                    t0 = 128 * c + 64 * g
                    for p in range(NPE):
                        for t in range(2):
                            nc.tensor.matmul(
                                m[0:64, :],
                                xbt[p][:, t, t0:t0 + 64],
                                wvbt[p][:, t, :],
                                start=(p == 0 and t == 0),
                                stop=(p == NPE - 1 and t == 1))
                    nc.vector.tensor_copy(
                        vb[c // 2][64 * g:64 * g + 64, c % 2, :, 0:64],
                        m[0:64, :].rearrange("p (h d) -> p h d", d=64))
                return emit

            def proj_tile(j, t, nh, ot_tile):
                def emit():
                    m = ps_mm.tile([128, 512], F32, tag="mm", name="mmp")
                    for p in range(4):
                        nc.tensor.matmul(
                            m[:], ot_tile[:, p, 128 * t:128 * t + 128],
                            wpt[p][:, 512 * nh:512 * nh + 512],
                            start=(p == 0), stop=(p == 3))
                    ob = obp.tile([128, 512], F32, tag="ob", name="ob")
                    nc.vector.tensor_copy(ob[:], m[:])
                    nc.sync.dma_start(
                        out[512 * j + 128 * t:512 * j + 128 * t + 128,
                            512 * nh:512 * nh + 512], ob[:])
                return emit

            bg = []  # queue of (tag, emitter); tags order forced at stream starts

            def drain_bg(n=1):
                for _ in range(min(n, len(bg))):
                    bg.pop(0)[1]()

            def force_bg(pred):
                """Emit from the front until no queued task satisfies pred."""
                while any(pred(t) for t, _ in bg):
                    bg.pop(0)[1]()

            # j tiles processed [1, 2, 3, 0]: the bf16 j=0 accuracy path runs
            # last so its DMAs/projections never gate the critical path.
            JORDER = (1, 2, 3, 0)
            # prologue: v pairs 0..1 (chunks 0..3); q tile 1, k tiles 0..1
            # for h=0. k projections are scheduled by tk-tile index: stream
            # (j, h) consumes k tiles 0..j.
            # interleave so the first stream's scores are unblocked ASAP
            qk_group("k", 0, 0)()
            v_group(0, 0, ps_sT)()
            v_group(0, 1)()
            qk_group("k", 0, 1, ps_sT)()
            v_group(1, 0)()
            v_group(1, 1, ps_sT)()
            qk_group("q", 0, 1)()
            for c in (2, 3):
                for g in range(2):
                    bg.append((("vq", 1), v_group(c, g)))
            for q in range(2, 4):
                for g in range(2):
                    bg.append((("vq", q), v_group(2 * q, g)))
                    bg.append((("vq", q), v_group(2 * q + 1, g)))
            for h in range(1, HL):
                bg.append((("q", 1, h), qk_group("q", h, 1)))
                bg.append((("k", 0, h), qk_group("k", h, 0)))
                bg.append((("k", 1, h), qk_group("k", h, 1)))
            # bf16 sidecar tasks wait on tail-end DMAs; keep them out of the
            # main drain queue until their inputs have surely landed
            bg_late = []
            for hp in range(4):
                bg_late.append((("qkb", 2 * hp), qkb_group(hp)))
            for c in range(4):
                for g in range(2):
                    bg_late.append((("vb", c // 2), vb_group(c, g)))

            ot_tiles = {}
            pend = []       # one-pair-lagged PE emissions (attnV/Z, normalize)
            o_sb_pair = {}  # normalized o for the in-flight head pair

            pending_proj = []
            for si, j in enumerate(JORDER):
                if si == 1:  # release the bf16 sidecar work mid-flight
                    bg.extend(bg_late)
                    bg_late = []
                # enqueue the next sequence step's inputs
                if si + 1 < NJ:
                    nj = JORDER[si + 1]
                    if nj != 0:
                        for q in range(2 * nj, 2 * nj + 2):
                            for g in range(2):
                                bg.append((("vq", q), v_group(2 * q, g)))
                                bg.append((("vq", q), v_group(2 * q + 1, g)))
                    for h in range(HL):
                        bg.append((("q", nj, h), qk_group("q", h, nj)))
                        if nj != 0:  # k tile 0 was produced in the prologue
                            bg.append((("k", nj, h), qk_group("k", h, nj)))
                # weave last step's proj tiles into the fresh queue (avoids a
                # PE-only burst that starves ACT at the step boundary); all
                # transposes must go first (proj reads their output)
                if pending_proj:
                    force_bg(lambda t: t[0] == "tr")
                for i, task in enumerate(pending_proj):
                    bg.insert(min(5 * i + 2, len(bg)), task)
                pending_proj = []
                # proj from two sequence steps back must be out before its
                # ot slot is reused
                force_bg(lambda t: t[0] == "proj" and t[1] <= si - 2)
                ot_tile = otp.tile([128, 4, 512], BF16, tag="ot", name=f"ot{j}")
                ot_tiles[j] = ot_tile

                for h in range(HL):
                    hp, r0 = h // 2, 64 * (h % 2)
                    zc = 8 * (8 * j + h)
                    # everything this stream reads must already be emitted
                    force_bg(lambda t: (t[0] == "q" and t[1] == j and t[2] <= h) or
                             (t[0] == "k" and t[1] <= j and t[2] <= h) or
                             (j == 0 and t[0] == "qkb" and t[1] <= h))
                    po_t = ps_po.tile([128, 512], F32, tag="po", name="po")
                    po = po_t[0:64, :]
                    nq = 2 * j + 2
                    for q in range(nq):
                        m0, m2 = (q == 2 * j), (q == 2 * j + 1)
                        force_bg(lambda t: (t[0] == "vq" and t[1] < q) or
                                 (j == 0 and t[0] == "vb" and t[1] < q))
                        sT = ps_sT.tile([128, 1024], F32, tag="sT", name="sT")
                        if j == 0:
                            et = etp.tile([128, 1024], BF16, tag="etb",
                                          name="etb", bufs=4)
                            trm = tritb
                        else:
                            et = etp.tile([128, 1024], FP8, tag="et", name="et")
                            trm = trit
                        if not (m0 or m2):
                            # off-diag pair: both chunks full [0:512]
                            for t in range(2):
                                c = 2 * q + t
                                nc.tensor.matmul(
                                    sT[:, 512 * t:512 * t + 512],
                                    kTt[hp][c // 4][r0:r0 + 64,
                                                    128 * (c % 4):128 * (c % 4) + 128],
                                    qTt[hp][j][r0:r0 + 64, :],
                                    start=True, stop=True, skip_group_check=True)
                            nc.scalar.activation(et[:], sT[:], Exp, scale=EXP_SCALE)
                            stride, width = 512, 512
                        elif m0:
                            # chunks 4j (full), 4j+1 (cols 128:512)
                            c = 4 * j
                            if j == 0:
                                # block-0 scores from bf16-accurate q/k
                                nc.tensor.matmul(
                                    sT[:, 0:128],
                                    kbt[hp][r0:r0 + 64, :],
                                    qbt[hp][r0:r0 + 64, :],
                                    start=True, stop=False,
                                    skip_group_check=True)
                                nc.tensor.matmul(
                                    sT[:, 128:512],
                                    kTt[hp][j][r0:r0 + 64, 0:128],
                                    qTt[hp][j][r0:r0 + 64, 128:512],
                                    start=False, stop=True,
                                    skip_group_check=True)
                            else:
                                nc.tensor.matmul(
                                    sT[:, 0:512],
                                    kTt[hp][j][r0:r0 + 64, 0:128],
                                    qTt[hp][j][r0:r0 + 64, :],
                                    start=True, stop=True,
                                    skip_group_check=True)
                            # cover [512:640] too so exp never reads stale
                            # bytes (those weights get memset to 0 after)
                            nc.tensor.matmul(
                                sT[:, 512:1024],
                                kTt[hp][j][r0:r0 + 64, 128:256],
                                qTt[hp][j][r0:r0 + 64, :],
                                start=True, stop=True, skip_group_check=True)
                            nc.scalar.activation(et[:], sT[:], Exp, scale=EXP_SCALE)
                            et3 = et[:].rearrange("p (a n) -> p a n", n=128)
                            nc.vector.tensor_mul(et3[:, 0:6:5, :], et3[:, 0:6:5, :],
                                                 trm[:])
                            stride, width = 512, 512
                        else:
                            # m2: chunks 4j+2 (cols 256:512 -> [0:256]),
                            #     4j+3 (cols 384:512 -> [384:512])
                            nc.tensor.matmul(
                                sT[:, 0:256],
                                kTt[hp][j][r0:r0 + 64, 256:384],
                                qTt[hp][j][r0:r0 + 64, 256:512],
                                start=True, stop=True, skip_group_check=True)
                            nc.tensor.matmul(
                                sT[:, 256:512],
                                kTt[hp][j][r0:r0 + 64, 384:512],
                                qTt[hp][j][r0:r0 + 64, 256:512],
                                start=False, stop=True, skip_group_check=True)
                            nc.scalar.activation(et[:, 0:512], sT[:, 0:512],
                                                 Exp, scale=EXP_SCALE)
                            et3 = et[:].rearrange("p (a n) -> p a n", n=128)
                            nc.vector.tensor_mul(et3[:, 0:4:3, :], et3[:, 0:4:3, :],
                                                 trm[:])
                            stride, width = 256, 256
                        etv = et[:, 0:2 * stride].rearrange(
                            "p (a n) -> p a n", a=2)
                        s_lo = 4 if m2 else 0
                        base = 256 if m2 else 0

                        def attn_emit(et=et, etv=etv, q=q, h=h, po=po,
                                      po_t=po_t, zc=zc,
                                      s_lo=s_lo, base=base, m0=m0, m2=m2, j=j):
                            # exactly ONE start=True per bank-use: start=True
                            # flags the whole 2KB bank pending-zero; every
                            # other group's first touch consumes its flag.
                            if j == 0:
                                # bf16 non-DR path (no fp8 noise on the short-
                                # support rows). (tile, subtiles, flat offset fn)
                                if m0:
                                    work = [(0, range(0, 8), lambda s: 64 * s),
                                            (1, range(2, 8),
                                             lambda s: 512 + 64 * s)]
                                else:
                                    work = [(0, range(4, 8),
                                             lambda s: 64 * s - 256),
                                            (1, range(6, 8), lambda s: 64 * s)]
                                stop_at = {0: (0, 0), 1: (0, 0), 2: (0, 1),
                                           3: (0, 1), 4: (1, 0), 5: (1, 0),
                                           6: (1, 1), 7: (1, 1)}
                                for tt, srange, off in work:
                                    for s in srange:
                                        fo = off(s)
                                        stop = stop_at[s] == (q, tt)
                                        st = (q == 0 and tt == 0 and s == 0)
                                        nc.tensor.matmul(
                                            po[:, 64 * s:64 * s + 64],
                                            et[:, fo:fo + 64],
                                            vb[q][:, tt, h, 0:64],
                                            start=st, stop=stop,
                                            skip_group_check=True)
                                        nc.tensor.matmul(
                                            po_t[64:128, s:s + 1],
                                            et[:, fo:fo + 64],
                                            vb[q][:, tt, h, 64:65],
                                            start=st, stop=stop,
                                            skip_group_check=True)
                                return
                            for s in range(s_lo, 8):
                                cc = 64 * s - base
                                stop = (s < 4 and m0) or (s >= 4 and m2)
                                # tile1's dead block is never read: subtiles
                                # under the diagonal use a single-tile matmul
                                single = (m0 and s < 2) or (m2 and s < 6)
                                if single:
                                    lhs_o = etv[:, 0, cc:cc + 64]
                                    rhs_o = vp[q][:, 0, h, 0:64]
                                    pm = None
                                else:
                                    lhs_o = etv[:, :, cc:cc + 64]
                                    rhs_o = vp[q][:, :, h, 0:64]
                                    pm = DR
                                nc.tensor.matmul(
                                    po[:, 64 * s:64 * s + 64],
                                    lhs_o, rhs_o,
                                    start=(q == 0 and s == 0), stop=stop,
                                    perf_mode=pm, skip_group_check=True)
                                # Z columns live at partitions 64:128 of the
                                # same po bank (single-tile, non-DR: base-64)
                                nc.tensor.matmul(
                                    po_t[64:128, s:s + 1],
                                    etv[:, 0, cc:cc + 64],
                                    vp[q][:, 0, h, 64:65],
                                    start=(q == 0 and s == 0), stop=(stop and single),
                                    skip_group_check=True)
                                if not single:
                                    nc.tensor.matmul(
                                        po_t[64:128, s:s + 1],
                                        etv[:, 1, cc:cc + 64],
                                        vp[q][:, 1, h, 64:65],
                                        start=False, stop=stop,
                                        skip_group_check=True)
                        # one-pair software pipeline: previous pair's attnV/Z
                        # runs while this pair's exp/masks are in flight
                        while pend:
                            pend.pop(0)()
                        pend.append(attn_emit)
                        drain_bg(3 if h == HL - 1 else (2 if len(bg) > 14 else 1))

                    force_bg(lambda t: (t[0] == "vq" and t[1] <= 2 * j + 1) or
                             (j == 0 and t[0] == "vb"))

                    def norm_emit(po=po, po_t=po_t, h=h, hp=hp,
                                  ot_tile=ot_tile):
                        rz = rzp.tile([64, 8], F32, tag="rz", name="rz")
                        nc.vector.reciprocal(rz[:], po_t[64:128, 0:8])
                        o_sb = osb.tile([64, 512], BF16, tag="os", name="os")
                        nc.vector.tensor_tensor(
                            o_sb[:].rearrange("p (e s) -> p e s", e=8),
                            po[:].rearrange("p (e s) -> p e s", e=8),
                            rz[:].unsqueeze(2).broadcast_to([64, 8, 64]),
                            mybir.AluOpType.mult)
                        o_sb_pair[h % 2] = o_sb
                        if h % 2 == 1:
                            def transpose_emit(hp=hp, ot_tile=ot_tile,
                                               pair=dict(o_sb_pair)):
                                pt_t = ps_po.tile([128, 512], F32, tag="po",
                                                  name="pt")
                                pt = pt_t[:].bitcast(BF16)[:, 0:512]
                                for hh in range(2):
                                    for s in range(8):
                                        nc.tensor.matmul(
                                            pt[64 * hh:64 * hh + 64,
                                               64 * s:64 * s + 64],
                                            pair[hh][:, 64 * s:64 * s + 64],
                                            idt[:], is_transpose=True,
                                            start=(s == 0),
                                            stop=(hh == 1 and s == 7),
                                            skip_group_check=True)
                                nc.vector.tensor_copy(ot_tile[:, hp, :], pt[:])
                            bg.insert(min(1, len(bg)),
                                      (("tr", None), transpose_emit))
                    pend.append(norm_emit)

                # flush the pipeline at the j boundary so the last head pair's
                # normalize + transpose are queued before proj tasks
                while pend:
                    pend.pop(0)()
                for t in range(4):
                    for nh in range(2):
                        pending_proj.append((("proj", si),
                                             proj_tile(j, t, nh, ot_tile)))

            while pend:
                pend.pop(0)()
            drain_bg(len(bg))
            for _, task in pending_proj:
                task()

    nc.compile()
    return nc


def _host_prep(x, Wq, Wk, Wv, Wp):
    """Per-core input maps."""
    tri = (np.arange(128)[None, :] >= np.arange(128)[:, None]).astype(np.float32)
    tri2 = np.concatenate([tri, tri], axis=1).astype(f8np)
    ident = np.eye(64, dtype=np.float32).astype(bfnp)
    WpT = np.ascontiguousarray(Wp.T) * (1.0 / SV)  # [E(hd), E]

    def wslice(W, h0, scale, dt=f8np):  # [H,E,D] -> [E, 8*64]
        w = W[h0:h0 + HL].transpose(1, 0, 2).reshape(E, HL * D) * scale
        return np.ascontiguousarray(w).astype(dt)

    in_maps = []
    for c in range(8):
        b, hh = c // 2, c % 2
        h0 = hh * HL
        in_maps.append({
            "xT8": np.ascontiguousarray(x[b].T).astype(f8np),
            "wq8": wslice(Wq, h0, SW),
            "wk8": wslice(Wk, h0, SW),
            "wv8": wslice(Wv, h0, SV),
            "wpT": np.ascontiguousarray(
                WpT[h0 * D:(h0 + HL) * D, :]).astype(bfnp),
            "tri2": tri2,
            "tri2b": tri2.astype(np.float32).astype(bfnp),
            "ident": ident,
            "xbT": np.ascontiguousarray(x[b].T[:, 0:512]).astype(bfnp),
            "wqb": wslice(Wq, h0, SW, bfnp),
            "wkb": wslice(Wk, h0, SW, bfnp),
            "wvb": wslice(Wv, h0, SV, bfnp),
        })
    return in_maps


def kernel(x, Wq, Wk, Wv, Wp, bp):
    x = np.asarray(x, dtype=np.float32)
    Wq = np.asarray(Wq, dtype=np.float32)
    Wk = np.asarray(Wk, dtype=np.float32)
    Wv = np.asarray(Wv, dtype=np.float32)
    Wp = np.asarray(Wp, dtype=np.float32)
    bp = np.asarray(bp, dtype=np.float32)

    if "nc" not in _CACHE:
        _CACHE["nc"] = _build()
    nc = _CACHE["nc"]

    in_maps = _host_prep(x, Wq, Wk, Wv, Wp)
    res = run_bass_kernel_spmd(nc, in_maps, list(range(8)))
    parts = [np.asarray(res.results[c]["out"], dtype=np.float32) for c in range(8)]
    out = np.stack([parts[2 * b] + parts[2 * b + 1] for b in range(B)], axis=0)
    return (out + bp[None, None, :]).astype(np.float32)


# revision 100
# speedup vs baseline: 1.3019x; 1.0256x over previous
"""Multi-head causal attention (B=4,T=2048,E=1024,H=16,D=64) on 8 TRN2 cores.

Sharding: core c -> batch b=c//2, heads h0=(c%2)*8 .. h0+8. Each core computes
its 8 heads' attention and a partial output projection (row-split Wp); host
sums the two partials per batch (+bias).

Per-core kernel:
  - Q/K/V projections in fp8e4 DoubleRow (2 k-tiles of 128 per instr, 0.5
    cyc/col), M=64 outputs at PSUM base 0. Host pre-scales Wq,Wk by 64 and
    Wv by 16 (powers of 2, folded back via exp scale / Wp scale).
  - scores: bf16 kT.T @ qT per 128-row tk chunk, chunk PAIRS share one
    [128,1024] PSUM tile (2 banks) so exp runs as one ACT op per pair.
  - exp -> fp8e4 "et" tiles [128, 2, N]; causal diag handled by gpsimd
    memset (dead block) + gpsimd triangle mask multiply.
  - attnV: o-layout fp8 DoubleRow: o[tq64, d64] += et.T @ v over chunk
    pairs; Z via DR ones-column matmuls into a shared Z bank (per-stream
    8-column slots). Normalize is per-partition: DVE reciprocal [64,8] +
    one broadcast multiply.
  - o -> oT via PE transpose (identity matmul) packed 2 heads/bank, then
    bf16 output projection, DVE copy, DMA out.
"""
import sys
import numpy as np

sys.path.insert(0, "/opt/trn_rl_repo")

import ml_dtypes
import concourse.bass as bass
import concourse.bacc as bacc
import concourse.mybir as mybir
from concourse import tile
from concourse.bass_utils import run_bass_kernel_spmd

B, T, E, H, D = 4, 2048, 1024, 16, 64
HL = H // 2          # 8 local heads per core
NJ = T // 512        # 4 tq tiles
NPE = E // 256       # 4 E-chunk-pairs
NPAIR = T // 256     # 8 tk chunk pairs
BF16 = mybir.dt.bfloat16
F32 = mybir.dt.float32
FP8 = mybir.dt.float8e4
DR = mybir.MatmulPerfMode.DoubleRow
Exp = mybir.ActivationFunctionType.Exp
f8np = ml_dtypes.float8_e4m3
bfnp = ml_dtypes.bfloat16

SW = 64.0   # Wq/Wk host prescale (exp scale folds 1/SW^2)
SV = 16.0   # Wv host prescale (Wp folds 1/SV)
EXP_SCALE = 0.125 / (SW * SW)  # 2^-15 exactly

_CACHE = {}


def _build():
    nc = bacc.Bacc("TRN2", target_bir_lowering=False)
    xT8 = nc.declare_dram_parameter("xT8", [E, T], FP8, isOutput=False)
    wq8 = nc.declare_dram_parameter("wq8", [E, HL * D], FP8, isOutput=False)
    wk8 = nc.declare_dram_parameter("wk8", [E, HL * D], FP8, isOutput=False)
    wv8 = nc.declare_dram_parameter("wv8", [E, HL * D], FP8, isOutput=False)
    wpT = nc.declare_dram_parameter("wpT", [HL * D, E], BF16, isOutput=False)
    tri2 = nc.declare_dram_parameter("tri2", [128, 256], FP8, isOutput=False)
    tri2b = nc.declare_dram_parameter("tri2b", [128, 256], BF16, isOutput=False)
    xbT = nc.declare_dram_parameter("xbT", [E, 512], BF16, isOutput=False)
    wqb = nc.declare_dram_parameter("wqb", [E, HL * D], BF16, isOutput=False)
    wkb = nc.declare_dram_parameter("wkb", [E, HL * D], BF16, isOutput=False)
    wvb = nc.declare_dram_parameter("wvb", [E, HL * D], BF16, isOutput=False)
    ident = nc.declare_dram_parameter("ident", [64, 64], BF16, isOutput=False)
    out = nc.declare_dram_parameter("out", [T, E], F32, isOutput=True)

    with tile.TileContext(nc) as tc:
        with (
            tc.tile_pool(name="pp", bufs=1) as pp,
            tc.tile_pool(name="etp", bufs=8) as etp,
            tc.tile_pool(name="osb", bufs=8) as osb,
            tc.tile_pool(name="rzp", bufs=6) as rzp,
            tc.tile_pool(name="otp", bufs=2) as otp,
            tc.tile_pool(name="obp", bufs=3) as obp,
            tc.tile_pool(name="sT", bufs=2, space=bass.MemorySpace.PSUM) as ps_sT,
            tc.tile_pool(name="po", bufs=2, space=bass.MemorySpace.PSUM) as ps_po,
            tc.tile_pool(name="mm", bufs=2, space=bass.MemorySpace.PSUM) as ps_mm,
        ):
            # ---- persistent SBUF tiles
            x8t = [pp.tile([128, 2, T], FP8, tag=f"x{p}", name=f"x{p}")
                   for p in range(NPE)]
            wq8t = [pp.tile([128, 2, 512], FP8, tag=f"wq{p}", name=f"wq{p}")
                    for p in range(NPE)]
            wk8t = [pp.tile([128, 2, 512], FP8, tag=f"wk{p}", name=f"wk{p}")
                    for p in range(NPE)]
            wv8t = [pp.tile([128, 2, 512], FP8, tag=f"wv{p}", name=f"wv{p}")
                    for p in range(NPE)]
            wpt = [pp.tile([128, E], BF16, tag=f"wp{p}", name=f"wp{p}")
                   for p in range(4)]
            trit = pp.tile([128, 2, 128], FP8, tag="tri", name="trit")
            tritb = pp.tile([128, 2, 128], BF16, tag="trib", name="tritb")
            idt = pp.tile([64, 64], BF16, tag="id", name="idt")
            qTt = [[pp.tile([128, 512], BF16, tag=f"q{p}_{j}", name=f"q{p}_{j}")
                    for j in range(NJ)] for p in range(4)]
            kTt = [[pp.tile([128, 512], BF16, tag=f"k{p}_{j}", name=f"k{p}_{j}")
                    for j in range(NJ)] for p in range(4)]
            vp = [pp.tile([128, 2, HL, 65], FP8, tag=f"v{q}", name=f"v{q}")
                  for q in range(NPAIR)]
            # bf16 v for chunks 0..3: j=0 attention runs in bf16 (short-support
            # softmax rows can't average away fp8 quantization noise)
            vb = [pp.tile([128, 2, HL, 65], BF16, tag=f"vb{q}", name=f"vb{q}")
                  for q in range(2)]
            # bf16 x/W and q/k for the first 128 tokens (block-0 scores)
            xbt = [pp.tile([128, 2, 512], BF16, tag=f"xb{p}", name=f"xb{p}")
                   for p in range(NPE)]
            wqbt = [pp.tile([128, 2, 512], BF16, tag=f"wqb{p}", name=f"wqb{p}")
                    for p in range(NPE)]
            wkbt = [pp.tile([128, 2, 512], BF16, tag=f"wkb{p}", name=f"wkb{p}")
                    for p in range(NPE)]
            wvbt = [pp.tile([128, 2, 512], BF16, tag=f"wvb{p}", name=f"wvb{p}")
                    for p in range(NPE)]
            qbt = [pp.tile([128, 128], BF16, tag=f"qb{p}", name=f"qb{p}")
                   for p in range(4)]
            kbt = [pp.tile([128, 128], BF16, tag=f"kb{p}", name=f"kb{p}")
                   for p in range(4)]

            # ---- DMA in. j tiles are processed in order [1,2,3,0], so the
            # critical path is x cols [0:1024] + wv (SP) and wq/wk (ACT).
            # The bf16 sidecar tiles (j=0 accuracy path) ride at the SP tail.
            def tdma(queue, dst, src, cols=None):
                # both t-halves in one DMA: src rows (t p) -> dst [p, t, n]
                s = src if cols is None else src[:, cols[0]:cols[1]]
                queue.dma_start(dst, s.rearrange("(t p) n -> p t n", t=2))
            for p in range(NPE):
                tdma(nc.sync, x8t[p][:, :, 0:512], xT8[256 * p:256 * p + 256],
                     (0, 512))
                tdma(nc.sync, wv8t[p][:], wv8[256 * p:256 * p + 256])
            for p in range(NPE):
                tdma(nc.scalar, wq8t[p][:], wq8[256 * p:256 * p + 256])
                tdma(nc.scalar, wk8t[p][:], wk8[256 * p:256 * p + 256])
            for p in range(NPE):
                tdma(nc.sync, x8t[p][:, :, 512:1024],
                     xT8[256 * p:256 * p + 256], (512, 1024))
            nc.sync.dma_start(trit[:], tri2[:].rearrange("p (a n) -> p a n", a=2))
            nc.sync.dma_start(idt[:], ident[:])
            for jj in range(2, NJ):
                for p in range(NPE):
                    tdma(nc.sync, x8t[p][:, :, 512 * jj:512 * jj + 512],
                         xT8[256 * p:256 * p + 256], (512 * jj, 512 * jj + 512))
            for p in range(4):
                nc.sync.dma_start(wpt[p][:], wpT[128 * p:128 * p + 128, :])
            nc.sync.dma_start(tritb[:],
                              tri2b[:].rearrange("p (a n) -> p a n", a=2))
            for p in range(NPE):
                tdma(nc.sync, xbt[p][:], xbT[256 * p:256 * p + 256])
                tdma(nc.sync, wvbt[p][:], wvb[256 * p:256 * p + 256])
                tdma(nc.sync, wqbt[p][:], wqb[256 * p:256 * p + 256])
                tdma(nc.sync, wkbt[p][:], wkb[256 * p:256 * p + 256])
            for q in range(NPAIR):
                nc.gpsimd.memset(vp[q][:, :, :, 64:65], 1.0)
            for q in range(2):
                nc.gpsimd.memset(vb[q][:, :, :, 64:65], 1.0)

            # ---- background task machinery (qkv groups, proj tiles)
            def qk_group(kind, h, j, pool=None):
                def emit():
                    wt = wq8t if kind == "q" else wk8t
                    dst = qTt if kind == "q" else kTt
                    if pool is None:
                        m = ps_mm.tile([128, 512], F32, tag="mm", name="mmq")
                    else:
                        m = pool.tile([128, 1024], F32, tag="sT",
                                      name="mmq")[:, 0:512]
                    for p in range(NPE):
                        nc.tensor.matmul(
                            m[0:64, :], wt[p][:, :, 64 * h:64 * h + 64],
                            x8t[p][:, :, 512 * j:512 * j + 512],
                            start=(p == 0), stop=(p == NPE - 1), perf_mode=DR)
                    nc.vector.tensor_copy(
                        dst[h // 2][j][64 * (h % 2):64 * (h % 2) + 64, :],
                        m[0:64, :])
                return emit

            def qkb_group(pe_hp):  # bf16 q/k for tokens 0..127, head pair
                def emit():
                    for wt, dst in ((wqbt, qbt), (wkbt, kbt)):
                        m = ps_mm.tile([128, 512], F32, tag="mm", name="mmb")
                        for p in range(NPE):
                            for t in range(2):
                                nc.tensor.matmul(
                                    m[0:128, 0:128],
                                    wt[p][:, t, 128 * pe_hp:128 * pe_hp + 128],
                                    xbt[p][:, t, 0:128],
                                    start=(p == 0 and t == 0),
                                    stop=(p == NPE - 1 and t == 1))
                        nc.vector.tensor_copy(dst[pe_hp][:], m[0:128, 0:128])
                return emit

            def v_group(c, g, pool=None):  # chunk c, t-64 sub g
                def emit():
                    if pool is None:
                        m = ps_mm.tile([128, 512], F32, tag="mm", name="mmv")
                    else:
                        m = pool.tile([128, 1024], F32, tag="sT",
                                      name="mmv")[:, 0:512]
                    t0 = 128 * c + 64 * g
                    for p in range(NPE):
                        nc.tensor.matmul(
                            m[0:64, :], x8t[p][:, :, t0:t0 + 64], wv8t[p][:],
                            start=(p == 0), stop=(p == NPE - 1), perf_mode=DR)
                    nc.vector.tensor_copy(
                        vp[c // 2][64 * g:64 * g + 64, c % 2, :, 0:64],
                        m[0:64, :].rearrange("p (h d) -> p h d", d=64))
                return emit

            def vb_group(c, g):  # true bf16 v for j=0 (bf16 x and Wv)
                def emit():
                    m = ps_mm.tile([128, 512], F32, tag="mm", name="mvb")
                    t0 = 128 * c + 64 * g
                    for p in range(NPE):
                        for t in range(2):
                            nc.tensor.matmul(
                                m[0:64, :],
                                xbt[p][:, t, t0:t0 + 64],
                                wvbt[p][:, t, :],
                                start=(p == 0 and t == 0),
                                stop=(p == NPE - 1 and t == 1))
                    nc.vector.tensor_copy(
                        vb[c // 2][64 * g:64 * g + 64, c % 2, :, 0:64],
                        m[0:64, :].rearrange("p (h d) -> p h d", d=64))
                return emit

            def proj_tile(j, t, nh, ot_tile):
                def emit():
                    m = ps_mm.tile([128, 512], F32, tag="mm", name="mmp")
                    for p in range(4):
                        nc.tensor.matmul(
                            m[:], ot_tile[:, p, 128 * t:128 * t + 128],
                            wpt[p][:, 512 * nh:512 * nh + 512],
                            start=(p == 0), stop=(p == 3))
                    ob = obp.tile([128, 512], F32, tag="ob", name="ob")
                    nc.vector.tensor_copy(ob[:], m[:])
                    nc.sync.dma_start(
                        out[512 * j + 128 * t:512 * j + 128 * t + 128,
                            512 * nh:512 * nh + 512], ob[:])
                return emit

            bg = []  # queue of (tag, emitter); tags order forced at stream starts

            def drain_bg(n=1):
                for _ in range(min(n, len(bg))):
                    bg.pop(0)[1]()

            def force_bg(pred):
                """Emit from the front until no queued task satisfies pred."""
                while any(pred(t) for t, _ in bg):
                    bg.pop(0)[1]()

            # j tiles processed [1, 2, 3, 0]: the bf16 j=0 accuracy path runs
            # last so its DMAs/projections never gate the critical path.
            JORDER = (1, 2, 3, 0)
            # prologue: v pairs 0..1 (chunks 0..3); q tile 1, k tiles 0..1
            # for h=0. k projections are scheduled by tk-tile index: stream
            # (j, h) consumes k tiles 0..j.
            # interleave so the first stream's scores are unblocked ASAP
            qk_group("k", 0, 0)()
            v_group(0, 0, ps_sT)()
            v_group(0, 1)()
            qk_group("k", 0, 1, ps_sT)()
            v_group(1, 0)()
            v_group(1, 1, ps_sT)()
            qk_group("q", 0, 1)()
            for c in (2, 3):
                for g in range(2):
                    bg.append((("vq", 1), v_group(c, g)))
            for q in range(2, 4):
                for g in range(2):
                    bg.append((("vq", q), v_group(2 * q, g)))
                    bg.append((("vq", q), v_group(2 * q + 1, g)))
            for h in range(1, HL):
                bg.append((("q", 1, h), qk_group("q", h, 1)))
                bg.append((("k", 0, h), qk_group("k", h, 0)))
                bg.append((("k", 1, h), qk_group("k", h, 1)))
            # bf16 sidecar tasks wait on tail-end DMAs; keep them out of the
            # main drain queue until their inputs have surely landed
            bg_late = []
            for hp in range(4):
                bg_late.append((("qkb", 2 * hp), qkb_group(hp)))
            for c in range(4):
                for g in range(2):
                    bg_late.append((("vb", c // 2), vb_group(c, g)))

            ot_tiles = {}
            pend = []       # one-pair-lagged PE emissions (attnV/Z, normalize)
            o_sb_pair = {}  # normalized o for the in-flight head pair

            pending_proj = []
            for si, j in enumerate(JORDER):
                if si == 1:  # release the bf16 sidecar work mid-flight
                    bg.extend(bg_late)
                    bg_late = []
                # enqueue the next sequence step's inputs
                if si + 1 < NJ:
                    nj = JORDER[si + 1]
                    if nj != 0:
                        for q in range(2 * nj, 2 * nj + 2):
                            for g in range(2):
                                bg.append((("vq", q), v_group(2 * q, g)))
                                bg.append((("vq", q), v_group(2 * q + 1, g)))
                    for h in range(HL):
                        bg.append((("q", nj, h), qk_group("q", h, nj)))
                        if nj != 0:  # k tile 0 was produced in the prologue
                            bg.append((("k", nj, h), qk_group("k", h, nj)))
                # weave last step's proj tiles into the fresh queue (avoids a
                # PE-only burst that starves ACT at the step boundary); all
                # transposes must go first (proj reads their output)
                if pending_proj:
                    force_bg(lambda t: t[0] == "tr")
                for i, task in enumerate(pending_proj):
                    bg.insert(min(5 * i + 2, len(bg)), task)
                pending_proj = []
                # proj from two sequence steps back must be out before its
                # ot slot is reused
                force_bg(lambda t: t[0] == "proj" and t[1] <= si - 2)
                ot_tile = otp.tile([128, 4, 512], BF16, tag="ot", name=f"ot{j}")
                ot_tiles[j] = ot_tile

                for h in range(HL):
                    hp, r0 = h // 2, 64 * (h % 2)
                    zc = 8 * (8 * j + h)
                    # everything this stream reads must already be emitted
                    force_bg(lambda t: (t[0] == "q" and t[1] == j and t[2] <= h) or
                             (t[0] == "k" and t[1] <= j and t[2] <= h) or
                             (j == 0 and t[0] == "qkb" and t[1] <= h))
                    po_t = ps_po.tile([128, 512], F32, tag="po", name="po")
                    po = po_t[0:64, :]
                    nq = 2 * j + 2
                    for q in range(nq):
                        m0, m2 = (q == 2 * j), (q == 2 * j + 1)
                        force_bg(lambda t: (t[0] == "vq" and t[1] < q) or
                                 (j == 0 and t[0] == "vb" and t[1] < q))
                        sT = ps_sT.tile([128, 1024], F32, tag="sT", name="sT")
                        if j == 0:
                            et = etp.tile([128, 1024], BF16, tag="etb",
                                          name="etb", bufs=4)
                            trm = tritb
                        else:
                            et = etp.tile([128, 1024], FP8, tag="et", name="et")
                            trm = trit
                        if not (m0 or m2):
                            # off-diag pair: both chunks full [0:512]
                            for t in range(2):
                                c = 2 * q + t
                                nc.tensor.matmul(
                                    sT[:, 512 * t:512 * t + 512],
                                    kTt[hp][c // 4][r0:r0 + 64,
                                                    128 * (c % 4):128 * (c % 4) + 128],
                                    qTt[hp][j][r0:r0 + 64, :],
                                    start=True, stop=True, skip_group_check=True)
                            nc.scalar.activation(et[:], sT[:], Exp, scale=EXP_SCALE)
                            stride, width = 512, 512
                        elif m0:
                            # chunks 4j (full), 4j+1 (cols 128:512)
                            c = 4 * j
                            if j == 0:
                                # block-0 scores from bf16-accurate q/k
                                nc.tensor.matmul(
                                    sT[:, 0:128],
                                    kbt[hp][r0:r0 + 64, :],
                                    qbt[hp][r0:r0 + 64, :],
                                    start=True, stop=False,
                                    skip_group_check=True)
                                nc.tensor.matmul(
                                    sT[:, 128:512],
                                    kTt[hp][j][r0:r0 + 64, 0:128],
                                    qTt[hp][j][r0:r0 + 64, 128:512],
                                    start=False, stop=True,
                                    skip_group_check=True)
                            else:
                                nc.tensor.matmul(
                                    sT[:, 0:512],
                                    kTt[hp][j][r0:r0 + 64, 0:128],
                                    qTt[hp][j][r0:r0 + 64, :],
                                    start=True, stop=True,
                                    skip_group_check=True)
                            # cover [512:640] too so exp never reads stale
                            # bytes (those weights get memset to 0 after)
                            nc.tensor.matmul(
                                sT[:, 512:1024],
                                kTt[hp][j][r0:r0 + 64, 128:256],
                                qTt[hp][j][r0:r0 + 64, :],
                                start=True, stop=True, skip_group_check=True)
                            nc.scalar.activation(et[:], sT[:], Exp, scale=EXP_SCALE)
                            et3 = et[:].rearrange("p (a n) -> p a n", n=128)
                            nc.vector.tensor_mul(et3[:, 0:6:5, :], et3[:, 0:6:5, :],
                                                 trm[:])
                            stride, width = 512, 512
                        else:
                            # m2: chunks 4j+2 (cols 256:512 -> [0:256]),
                            #     4j+3 (cols 384:512 -> [384:512])
                            nc.tensor.matmul(
                                sT[:, 0:256],
                                kTt[hp][j][r0:r0 + 64, 256:384],
                                qTt[hp][j][r0:r0 + 64, 256:512],
                                start=True, stop=True, skip_group_check=True)
                            nc.tensor.matmul(
                                sT[:, 256:512],
                                kTt[hp][j][r0:r0 + 64, 384:512],
                                qTt[hp][j][r0:r0 + 64, 256:512],
                                start=False, stop=True, skip_group_check=True)
                            nc.scalar.activation(et[:, 0:512], sT[:, 0:512],
                                                 Exp, scale=EXP_SCALE)
                            et3 = et[:].rearrange("p (a n) -> p a n", n=128)
                            nc.vector.tensor_mul(et3[:, 0:4:3, :], et3[:, 0:4:3, :],
                                                 trm[:])
                            stride, width = 256, 256
                        etv = et[:, 0:2 * stride].rearrange(
                            "p (a n) -> p a n", a=2)
                        s_lo = 4 if m2 else 0
                        base = 256 if m2 else 0

                        def attn_emit(et=et, etv=etv, q=q, h=h, po=po,
                                      po_t=po_t, zc=zc,
                                      s_lo=s_lo, base=base, m0=m0, m2=m2, j=j):
                            # exactly ONE start=True per bank-use: start=True
                            # flags the whole 2KB bank pending-zero; every
                            # other group's first touch consumes its flag.
                            if j == 0:
                                # bf16 non-DR path (no fp8 noise on the short-
                                # support rows). (tile, subtiles, flat offset fn)
                                if m0:
                                    work = [(0, range(0, 8), lambda s: 64 * s),
                                            (1, range(2, 8),
                                             lambda s: 512 + 64 * s)]
                                else:
                                    work = [(0, range(4, 8),
                                             lambda s: 64 * s - 256),
                                            (1, range(6, 8), lambda s: 64 * s)]
                                stop_at = {0: (0, 0), 1: (0, 0), 2: (0, 1),
                                           3: (0, 1), 4: (1, 0), 5: (1, 0),
                                           6: (1, 1), 7: (1, 1)}
                                for tt, srange, off in work:
                                    for s in srange:
                                        fo = off(s)
                                        stop = stop_at[s] == (q, tt)
                                        st = (q == 0 and tt == 0 and s == 0)
                                        nc.tensor.matmul(
                                            po[:, 64 * s:64 * s + 64],
                                            et[:, fo:fo + 64],
                                            vb[q][:, tt, h, 0:64],
                                            start=st, stop=stop,
                                            skip_group_check=True)
                                        nc.tensor.matmul(
                                            po_t[64:128, s:s + 1],
                                            et[:, fo:fo + 64],
                                            vb[q][:, tt, h, 64:65],
                                            start=st, stop=stop,
                                            skip_group_check=True)
                                return
                            for s in range(s_lo, 8):
                                cc = 64 * s - base
                                stop = (s < 4 and m0) or (s >= 4 and m2)
                                # tile1's dead block is never read: subtiles
                                # under the diagonal use a single-tile matmul
                                single = (m0 and s < 2) or (m2 and s < 6)
                                if single:
                                    lhs_o = etv[:, 0, cc:cc + 64]
                                    rhs_o = vp[q][:, 0, h, 0:64]
                                    pm = None
                                else:
                                    lhs_o = etv[:, :, cc:cc + 64]
                                    rhs_o = vp[q][:, :, h, 0:64]
                                    pm = DR
                                nc.tensor.matmul(
                                    po[:, 64 * s:64 * s + 64],
                                    lhs_o, rhs_o,
                                    start=(q == 0 and s == 0), stop=stop,
                                    perf_mode=pm, skip_group_check=True)
                                # Z columns live at partitions 64:128 of the
                                # same po bank (single-tile, non-DR: base-64)
                                nc.tensor.matmul(
                                    po_t[64:128, s:s + 1],
                                    etv[:, 0, cc:cc + 64],
                                    vp[q][:, 0, h, 64:65],
                                    start=(q == 0 and s == 0), stop=(stop and single),
                                    skip_group_check=True)
                                if not single:
                                    nc.tensor.matmul(
                                        po_t[64:128, s:s + 1],
                                        etv[:, 1, cc:cc + 64],
                                        vp[q][:, 1, h, 64:65],
                                        start=False, stop=stop,
                                        skip_group_check=True)
                        # one-pair software pipeline: previous pair's attnV/Z
                        # runs while this pair's exp/masks are in flight
                        while pend:
                            pend.pop(0)()
                        pend.append(attn_emit)
                        drain_bg(3 if h == HL - 1 else (2 if len(bg) > 40 else 1))

                    force_bg(lambda t: (t[0] == "vq" and t[1] <= 2 * j + 1) or
                             (j == 0 and t[0] == "vb"))

                    def norm_emit(po=po, po_t=po_t, h=h, hp=hp,
                                  ot_tile=ot_tile):
                        rz = rzp.tile([64, 8], F32, tag="rz", name="rz")
                        nc.vector.reciprocal(rz[:], po_t[64:128, 0:8])
                        o_sb = osb.tile([64, 512], BF16, tag="os", name="os")
                        nc.vector.tensor_tensor(
                            o_sb[:].rearrange("p (e s) -> p e s", e=8),
                            po[:].rearrange("p (e s) -> p e s", e=8),
                            rz[:].unsqueeze(2).broadcast_to([64, 8, 64]),
                            mybir.AluOpType.mult)
                        o_sb_pair[h % 2] = o_sb
                        if h % 2 == 1:
                            def transpose_emit(hp=hp, ot_tile=ot_tile,
                                               pair=dict(o_sb_pair)):
                                pt_t = ps_po.tile([128, 512], F32, tag="po",
                                                  name="pt")
                                pt = pt_t[:].bitcast(BF16)[:, 0:512]
                                for hh in range(2):
                                    for s in range(8):
                                        nc.tensor.matmul(
                                            pt[64 * hh:64 * hh + 64,
                                               64 * s:64 * s + 64],
                                            pair[hh][:, 64 * s:64 * s + 64],
                                            idt[:], is_transpose=True,
                                            start=(s == 0),
                                            stop=(hh == 1 and s == 7),
                                            skip_group_check=True)
                                nc.vector.tensor_copy(ot_tile[:, hp, :], pt[:])
                            bg.insert(min(1, len(bg)),
                                      (("tr", None), transpose_emit))
                    pend.append(norm_emit)

                # flush the pipeline at the j boundary so the last head pair's
                # normalize + transpose are queued before proj tasks
                while pend:
                    pend.pop(0)()
                for t in range(4):
                    for nh in range(2):
                        pending_proj.append((("proj", si),
                                             proj_tile(j, t, nh, ot_tile)))

            while pend:
                pend.pop(0)()
            drain_bg(len(bg))
            for _, task in pending_proj:
                task()

    nc.compile()
    return nc


def _host_prep(x, Wq, Wk, Wv, Wp):
    """Per-core input maps."""
    tri = (np.arange(128)[None, :] >= np.arange(128)[:, None]).astype(np.float32)
    tri2 = np.concatenate([tri, tri], axis=1).astype(f8np)
    ident = np.eye(64, dtype=np.float32).astype(bfnp)
    WpT = np.ascontiguousarray(Wp.T) * (1.0 / SV)  # [E(hd), E]

    def wslice(W, h0, scale, dt=f8np):  # [H,E,D] -> [E, 8*64]
        w = W[h0:h0 + HL].transpose(1, 0, 2).reshape(E, HL * D) * scale
        return np.ascontiguousarray(w).astype(dt)

    in_maps = []
    for c in range(8):
        b, hh = c // 2, c % 2
        h0 = hh * HL
        in_maps.append({
            "xT8": np.ascontiguousarray(x[b].T).astype(f8np),
            "wq8": wslice(Wq, h0, SW),
            "wk8": wslice(Wk, h0, SW),
            "wv8": wslice(Wv, h0, SV),
            "wpT": np.ascontiguousarray(
                WpT[h0 * D:(h0 + HL) * D, :]).astype(bfnp),
            "tri2": tri2,
            "tri2b": tri2.astype(np.float32).astype(bfnp),
            "ident": ident,
            "xbT": np.ascontiguousarray(x[b].T[:, 0:512]).astype(bfnp),
            "wqb": wslice(Wq, h0, SW, bfnp),
            "wkb": wslice(Wk, h0, SW, bfnp),
            "wvb": wslice(Wv, h0, SV, bfnp),
        })
    return in_maps


def kernel(x, Wq, Wk, Wv, Wp, bp):
    x = np.asarray(x, dtype=np.float32)
    Wq = np.asarray(Wq, dtype=np.float32)
    Wk = np.asarray(Wk, dtype=np.float32)
    Wv = np.asarray(Wv, dtype=np.float32)
    Wp = np.asarray(Wp, dtype=np.float32)
    bp = np.asarray(bp, dtype=np.float32)

    if "nc" not in _CACHE:
        _CACHE["nc"] = _build()
    nc = _CACHE["nc"]

    in_maps = _host_prep(x, Wq, Wk, Wv, Wp)
    res = run_bass_kernel_spmd(nc, in_maps, list(range(8)))
    parts = [np.asarray(res.results[c]["out"], dtype=np.float32) for c in range(8)]
    out = np.stack([parts[2 * b] + parts[2 * b + 1] for b in range(B)], axis=0)
    return (out + bp[None, None, :]).astype(np.float32)


# revision 104
# speedup vs baseline: 1.3182x; 1.0125x over previous
"""Multi-head causal attention (B=4,T=2048,E=1024,H=16,D=64) on 8 TRN2 cores.

Sharding: core c -> batch b=c//2, heads h0=(c%2)*8 .. h0+8. Each core computes
its 8 heads' attention and a partial output projection (row-split Wp); host
sums the two partials per batch (+bias).

Per-core kernel:
  - Q/K/V projections in fp8e4 DoubleRow (2 k-tiles of 128 per instr, 0.5
    cyc/col), M=64 outputs at PSUM base 0. Host pre-scales Wq,Wk by 64 and
    Wv by 16 (powers of 2, folded back via exp scale / Wp scale).
  - scores: bf16 kT.T @ qT per 128-row tk chunk, chunk PAIRS share one
    [128,1024] PSUM tile (2 banks) so exp runs as one ACT op per pair.
  - exp -> fp8e4 "et" tiles [128, 2, N]; causal diag handled by gpsimd
    memset (dead block) + gpsimd triangle mask multiply.
  - attnV: o-layout fp8 DoubleRow: o[tq64, d64] += et.T @ v over chunk
    pairs; Z via DR ones-column matmuls into a shared Z bank (per-stream
    8-column slots). Normalize is per-partition: DVE reciprocal [64,8] +
    one broadcast multiply.
  - o -> oT via PE transpose (identity matmul) packed 2 heads/bank, then
    bf16 output projection, DVE copy, DMA out.
"""
import sys
import numpy as np

sys.path.insert(0, "/opt/trn_rl_repo")

import ml_dtypes
import concourse.bass as bass
import concourse.bacc as bacc
import concourse.mybir as mybir
from concourse import tile
from concourse.bass_utils import run_bass_kernel_spmd

B, T, E, H, D = 4, 2048, 1024, 16, 64
HL = H // 2          # 8 local heads per core
NJ = T // 512        # 4 tq tiles
NPE = E // 256       # 4 E-chunk-pairs
NPAIR = T // 256     # 8 tk chunk pairs
BF16 = mybir.dt.bfloat16
F32 = mybir.dt.float32
FP8 = mybir.dt.float8e4
DR = mybir.MatmulPerfMode.DoubleRow
Exp = mybir.ActivationFunctionType.Exp
f8np = ml_dtypes.float8_e4m3
bfnp = ml_dtypes.bfloat16

SW = 64.0   # Wq/Wk host prescale (exp scale folds 1/SW^2)
SV = 16.0   # Wv host prescale (Wp folds 1/SV)
EXP_SCALE = 0.125 / (SW * SW)  # 2^-15 exactly

_CACHE = {}


def _build():
    nc = bacc.Bacc("TRN2", target_bir_lowering=False)
    xT8 = nc.declare_dram_parameter("xT8", [E, T], FP8, isOutput=False)
    wq8 = nc.declare_dram_parameter("wq8", [E, HL * D], FP8, isOutput=False)
    wk8 = nc.declare_dram_parameter("wk8", [E, HL * D], FP8, isOutput=False)
    wv8 = nc.declare_dram_parameter("wv8", [E, HL * D], FP8, isOutput=False)
    wpT = nc.declare_dram_parameter("wpT", [HL * D, E], BF16, isOutput=False)
    tri2 = nc.declare_dram_parameter("tri2", [128, 256], FP8, isOutput=False)
    tri2b = nc.declare_dram_parameter("tri2b", [128, 256], BF16, isOutput=False)
    xbT = nc.declare_dram_parameter("xbT", [E, 512], BF16, isOutput=False)
    wqb = nc.declare_dram_parameter("wqb", [E, HL * D], BF16, isOutput=False)
    wkb = nc.declare_dram_parameter("wkb", [E, HL * D], BF16, isOutput=False)
    wvb = nc.declare_dram_parameter("wvb", [E, HL * D], BF16, isOutput=False)
    ident = nc.declare_dram_parameter("ident", [64, 64], BF16, isOutput=False)
    out = nc.declare_dram_parameter("out", [T, E], F32, isOutput=True)

    with tile.TileContext(nc) as tc:
        with (
            tc.tile_pool(name="pp", bufs=1) as pp,
            tc.tile_pool(name="etp", bufs=14) as etp,
            tc.tile_pool(name="osb", bufs=10) as osb,
            tc.tile_pool(name="rzp", bufs=8) as rzp,
            tc.tile_pool(name="otp", bufs=3) as otp,
            tc.tile_pool(name="obp", bufs=6) as obp,
            tc.tile_pool(name="sT", bufs=2, space=bass.MemorySpace.PSUM) as ps_sT,
            tc.tile_pool(name="po", bufs=2, space=bass.MemorySpace.PSUM) as ps_po,
            tc.tile_pool(name="mm", bufs=2, space=bass.MemorySpace.PSUM) as ps_mm,
        ):
            # ---- persistent SBUF tiles
            x8t = [pp.tile([128, 2, T], FP8, tag=f"x{p}", name=f"x{p}")
                   for p in range(NPE)]
            wq8t = [pp.tile([128, 2, 512], FP8, tag=f"wq{p}", name=f"wq{p}")
                    for p in range(NPE)]
            wk8t = [pp.tile([128, 2, 512], FP8, tag=f"wk{p}", name=f"wk{p}")
                    for p in range(NPE)]
            wv8t = [pp.tile([128, 2, 512], FP8, tag=f"wv{p}", name=f"wv{p}")
                    for p in range(NPE)]
            wpt = [pp.tile([128, E], BF16, tag=f"wp{p}", name=f"wp{p}")
                   for p in range(4)]
            trit = pp.tile([128, 2, 128], FP8, tag="tri", name="trit")
            tritb = pp.tile([128, 2, 128], BF16, tag="trib", name="tritb")
            idt = pp.tile([64, 64], BF16, tag="id", name="idt")
            qTt = [[pp.tile([128, 512], BF16, tag=f"q{p}_{j}", name=f"q{p}_{j}")
                    for j in range(NJ)] for p in range(4)]
            kTt = [[pp.tile([128, 512], BF16, tag=f"k{p}_{j}", name=f"k{p}_{j}")
                    for j in range(NJ)] for p in range(4)]
            vp = [pp.tile([128, 2, HL, 65], FP8, tag=f"v{q}", name=f"v{q}")
                  for q in range(NPAIR)]
            # bf16 v for chunks 0..3: j=0 attention runs in bf16 (short-support
            # softmax rows can't average away fp8 quantization noise)
            vb = [pp.tile([128, 2, HL, 65], BF16, tag=f"vb{q}", name=f"vb{q}")
                  for q in range(2)]
            # bf16 x/W and q/k for the first 128 tokens (block-0 scores)
            xbt = [pp.tile([128, 2, 512], BF16, tag=f"xb{p}", name=f"xb{p}")
                   for p in range(NPE)]
            wqbt = [pp.tile([128, 2, 512], BF16, tag=f"wqb{p}", name=f"wqb{p}")
                    for p in range(NPE)]
            wkbt = [pp.tile([128, 2, 512], BF16, tag=f"wkb{p}", name=f"wkb{p}")
                    for p in range(NPE)]
            wvbt = [pp.tile([128, 2, 512], BF16, tag=f"wvb{p}", name=f"wvb{p}")
                    for p in range(NPE)]
            qbt = [pp.tile([128, 128], BF16, tag=f"qb{p}", name=f"qb{p}")
                   for p in range(4)]
            kbt = [pp.tile([128, 128], BF16, tag=f"kb{p}", name=f"kb{p}")
                   for p in range(4)]

            # ---- DMA in. j tiles are processed in order [1,2,3,0], so the
            # critical path is x cols [0:1024] + wv (SP) and wq/wk (ACT).
            # The bf16 sidecar tiles (j=0 accuracy path) ride at the SP tail.
            def tdma(queue, dst, src, cols=None):
                # both t-halves in one DMA: src rows (t p) -> dst [p, t, n]
                s = src if cols is None else src[:, cols[0]:cols[1]]
                queue.dma_start(dst, s.rearrange("(t p) n -> p t n", t=2))
            for p in range(NPE):
                tdma(nc.sync, x8t[p][:, :, 0:512], xT8[256 * p:256 * p + 256],
                     (0, 512))
                tdma(nc.sync, wv8t[p][:], wv8[256 * p:256 * p + 256])
            for p in range(NPE):
                tdma(nc.scalar, wq8t[p][:], wq8[256 * p:256 * p + 256])
                tdma(nc.scalar, wk8t[p][:], wk8[256 * p:256 * p + 256])
            for p in range(NPE):
                tdma(nc.sync, x8t[p][:, :, 512:1024],
                     xT8[256 * p:256 * p + 256], (512, 1024))
            nc.sync.dma_start(trit[:], tri2[:].rearrange("p (a n) -> p a n", a=2))
            nc.sync.dma_start(idt[:], ident[:])
            for jj in range(2, NJ):
                for p in range(NPE):
                    tdma(nc.sync, x8t[p][:, :, 512 * jj:512 * jj + 512],
                         xT8[256 * p:256 * p + 256], (512 * jj, 512 * jj + 512))
            for p in range(4):
                nc.sync.dma_start(wpt[p][:], wpT[128 * p:128 * p + 128, :])
            nc.sync.dma_start(tritb[:],
                              tri2b[:].rearrange("p (a n) -> p a n", a=2))
            for p in range(NPE):
                tdma(nc.sync, xbt[p][:], xbT[256 * p:256 * p + 256])
                tdma(nc.sync, wvbt[p][:], wvb[256 * p:256 * p + 256])
                tdma(nc.sync, wqbt[p][:], wqb[256 * p:256 * p + 256])
                tdma(nc.sync, wkbt[p][:], wkb[256 * p:256 * p + 256])
            for q in range(NPAIR):
                nc.gpsimd.memset(vp[q][:, :, :, 64:65], 1.0)
            for q in range(2):
                nc.gpsimd.memset(vb[q][:, :, :, 64:65], 1.0)

            # ---- background task machinery (qkv groups, proj tiles)
            def qk_group(kind, h, j, pool=None):
                def emit():
                    wt = wq8t if kind == "q" else wk8t
                    dst = qTt if kind == "q" else kTt
                    if pool is None:
                        m = ps_mm.tile([128, 512], F32, tag="mm", name="mmq")
                    else:
                        m = pool.tile([128, 1024], F32, tag="sT",
                                      name="mmq")[:, 0:512]
                    for p in range(NPE):
                        nc.tensor.matmul(
                            m[0:64, :], wt[p][:, :, 64 * h:64 * h + 64],
                            x8t[p][:, :, 512 * j:512 * j + 512],
                            start=(p == 0), stop=(p == NPE - 1), perf_mode=DR)
                    nc.vector.tensor_copy(
                        dst[h // 2][j][64 * (h % 2):64 * (h % 2) + 64, :],
                        m[0:64, :])
                return emit

            def qkb_group(pe_hp):  # bf16 q/k for tokens 0..127, head pair
                def emit():
                    for wt, dst in ((wqbt, qbt), (wkbt, kbt)):
                        m = ps_mm.tile([128, 512], F32, tag="mm", name="mmb")
                        for p in range(NPE):
                            for t in range(2):
                                nc.tensor.matmul(
                                    m[0:128, 0:128],
                                    wt[p][:, t, 128 * pe_hp:128 * pe_hp + 128],
                                    xbt[p][:, t, 0:128],
                                    start=(p == 0 and t == 0),
                                    stop=(p == NPE - 1 and t == 1))
                        nc.vector.tensor_copy(dst[pe_hp][:], m[0:128, 0:128])
                return emit

            def v_group(c, g, pool=None):  # chunk c, t-64 sub g
                def emit():
                    if pool is None:
                        m = ps_mm.tile([128, 512], F32, tag="mm", name="mmv")
                    else:
                        m = pool.tile([128, 1024], F32, tag="sT",
                                      name="mmv")[:, 0:512]
                    t0 = 128 * c + 64 * g
                    for p in range(NPE):
                        nc.tensor.matmul(
                            m[0:64, :], x8t[p][:, :, t0:t0 + 64], wv8t[p][:],
                            start=(p == 0), stop=(p == NPE - 1), perf_mode=DR)
                    nc.vector.tensor_copy(
                        vp[c // 2][64 * g:64 * g + 64, c % 2, :, 0:64],
                        m[0:64, :].rearrange("p (h d) -> p h d", d=64))
                return emit

            def vb_group(c, g):  # true bf16 v for j=0 (bf16 x and Wv)
                def emit():
                    m = ps_mm.tile([128, 512], F32, tag="mm", name="mvb")
                    t0 = 128 * c + 64 * g
                    for p in range(NPE):
                        for t in range(2):
                            nc.tensor.matmul(
                                m[0:64, :],
                                xbt[p][:, t, t0:t0 + 64],
                                wvbt[p][:, t, :],
                                start=(p == 0 and t == 0),
                                stop=(p == NPE - 1 and t == 1))
                    nc.vector.tensor_copy(
                        vb[c // 2][64 * g:64 * g + 64, c % 2, :, 0:64],
                        m[0:64, :].rearrange("p (h d) -> p h d", d=64))
                return emit

            def proj_tile(j, t, nh, ot_tile):
                def emit():
                    m = ps_mm.tile([128, 512], F32, tag="mm", name="mmp")
                    for p in range(4):
                        nc.tensor.matmul(
                            m[:], ot_tile[:, p, 128 * t:128 * t + 128],
                            wpt[p][:, 512 * nh:512 * nh + 512],
                            start=(p == 0), stop=(p == 3))
                    ob = obp.tile([128, 512], F32, tag="ob", name="ob")
                    nc.vector.tensor_copy(ob[:], m[:])
                    nc.sync.dma_start(
                        out[512 * j + 128 * t:512 * j + 128 * t + 128,
                            512 * nh:512 * nh + 512], ob[:])
                return emit

            bg = []  # queue of (tag, emitter); tags order forced at stream starts

            def drain_bg(n=1):
                for _ in range(min(n, len(bg))):
                    bg.pop(0)[1]()

            def force_bg(pred):
                """Emit from the front until no queued task satisfies pred."""
                while any(pred(t) for t, _ in bg):
                    bg.pop(0)[1]()

            # j tiles processed [1, 2, 3, 0]: the bf16 j=0 accuracy path runs
            # last so its DMAs/projections never gate the critical path.
            JORDER = (1, 2, 3, 0)
            # prologue: v pairs 0..1 (chunks 0..3); q tile 1, k tiles 0..1
            # for h=0. k projections are scheduled by tk-tile index: stream
            # (j, h) consumes k tiles 0..j.
            # interleave so the first stream's scores are unblocked ASAP
            qk_group("k", 0, 0)()
            v_group(0, 0, ps_sT)()
            v_group(0, 1)()
            qk_group("k", 0, 1, ps_sT)()
            v_group(1, 0)()
            v_group(1, 1, ps_sT)()
            qk_group("q", 0, 1)()
            for c in (2, 3):
                for g in range(2):
                    bg.append((("vq", 1), v_group(c, g)))
            for q in range(2, 4):
                for g in range(2):
                    bg.append((("vq", q), v_group(2 * q, g)))
                    bg.append((("vq", q), v_group(2 * q + 1, g)))
            for h in range(1, HL):
                bg.append((("q", 1, h), qk_group("q", h, 1)))
                bg.append((("k", 0, h), qk_group("k", h, 0)))
                bg.append((("k", 1, h), qk_group("k", h, 1)))
            # bf16 sidecar tasks wait on tail-end DMAs; keep them out of the
            # main drain queue until their inputs have surely landed
            bg_late = []
            for hp in range(4):
                bg_late.append((("qkb", 2 * hp), qkb_group(hp)))
            for c in range(4):
                for g in range(2):
                    bg_late.append((("vb", c // 2), vb_group(c, g)))

            ot_tiles = {}
            pend = []       # one-pair-lagged PE emissions (attnV/Z, normalize)
            o_sb_pair = {}  # normalized o for the in-flight head pair

            pending_proj = []
            for si, j in enumerate(JORDER):
                if si == 1:  # release the bf16 sidecar work mid-flight
                    bg.extend(bg_late)
                    bg_late = []
                # enqueue the next sequence step's inputs
                if si + 1 < NJ:
                    nj = JORDER[si + 1]
                    if nj != 0:
                        for q in range(2 * nj, 2 * nj + 2):
                            for g in range(2):
                                bg.append((("vq", q), v_group(2 * q, g)))
                                bg.append((("vq", q), v_group(2 * q + 1, g)))
                    for h in range(HL):
                        bg.append((("q", nj, h), qk_group("q", h, nj)))
                        if nj != 0:  # k tile 0 was produced in the prologue
                            bg.append((("k", nj, h), qk_group("k", h, nj)))
                # weave last step's proj tiles into the fresh queue (avoids a
                # PE-only burst that starves ACT at the step boundary); all
                # transposes must go first (proj reads their output)
                if pending_proj:
                    force_bg(lambda t: t[0] == "tr")
                for i, task in enumerate(pending_proj):
                    bg.insert(min(5 * i + 2, len(bg)), task)
                pending_proj = []
                # proj from two sequence steps back must be out before its
                # ot slot is reused
                force_bg(lambda t: t[0] == "proj" and t[1] <= si - 2)
                ot_tile = otp.tile([128, 4, 512], BF16, tag="ot", name=f"ot{j}")
                ot_tiles[j] = ot_tile

                for h in range(HL):
                    hp, r0 = h // 2, 64 * (h % 2)
                    zc = 8 * (8 * j + h)
                    # everything this stream reads must already be emitted
                    force_bg(lambda t: (t[0] == "q" and t[1] == j and t[2] <= h) or
                             (t[0] == "k" and t[1] <= j and t[2] <= h) or
                             (j == 0 and t[0] == "qkb" and t[1] <= h))
                    po_t = ps_po.tile([128, 512], F32, tag="po", name="po")
                    po = po_t[0:64, :]
                    nq = 2 * j + 2
                    for q in range(nq):
                        m0, m2 = (q == 2 * j), (q == 2 * j + 1)
                        force_bg(lambda t: (t[0] == "vq" and t[1] < q) or
                                 (j == 0 and t[0] == "vb" and t[1] < q))
                        sT = ps_sT.tile([128, 1024], F32, tag="sT", name="sT")
                        if j == 0:
                            et = etp.tile([128, 1024], BF16, tag="etb",
                                          name="etb", bufs=4)
                            trm = tritb
                        else:
                            et = etp.tile([128, 1024], FP8, tag="et", name="et")
                            trm = trit
                        if not (m0 or m2):
                            # off-diag pair: both chunks full [0:512]
                            for t in range(2):
                                c = 2 * q + t
                                nc.tensor.matmul(
                                    sT[:, 512 * t:512 * t + 512],
                                    kTt[hp][c // 4][r0:r0 + 64,
                                                    128 * (c % 4):128 * (c % 4) + 128],
                                    qTt[hp][j][r0:r0 + 64, :],
                                    start=True, stop=True, skip_group_check=True)
                            nc.scalar.activation(et[:], sT[:], Exp, scale=EXP_SCALE)
                            stride, width = 512, 512
                        elif m0:
                            # chunks 4j (full), 4j+1 (cols 128:512)
                            c = 4 * j
                            if j == 0:
                                # block-0 scores from bf16-accurate q/k
                                nc.tensor.matmul(
                                    sT[:, 0:128],
                                    kbt[hp][r0:r0 + 64, :],
                                    qbt[hp][r0:r0 + 64, :],
                                    start=True, stop=False,
                                    skip_group_check=True)
                                nc.tensor.matmul(
                                    sT[:, 128:512],
                                    kTt[hp][j][r0:r0 + 64, 0:128],
                                    qTt[hp][j][r0:r0 + 64, 128:512],
                                    start=False, stop=True,
                                    skip_group_check=True)
                            else:
                                nc.tensor.matmul(
                                    sT[:, 0:512],
                                    kTt[hp][j][r0:r0 + 64, 0:128],
                                    qTt[hp][j][r0:r0 + 64, :],
                                    start=True, stop=True,
                                    skip_group_check=True)
                            # cover [512:640] too so exp never reads stale
                            # bytes (those weights get memset to 0 after)
                            nc.tensor.matmul(
                                sT[:, 512:1024],
                                kTt[hp][j][r0:r0 + 64, 128:256],
                                qTt[hp][j][r0:r0 + 64, :],
                                start=True, stop=True, skip_group_check=True)
                            nc.scalar.activation(et[:], sT[:], Exp, scale=EXP_SCALE)
                            et3 = et[:].rearrange("p (a n) -> p a n", n=128)
                            nc.vector.tensor_mul(et3[:, 0:6:5, :], et3[:, 0:6:5, :],
                                                 trm[:])
                            stride, width = 512, 512
                        else:
                            # m2: chunks 4j+2 (cols 256:512 -> [0:256]),
                            #     4j+3 (cols 384:512 -> [384:512])
                            nc.tensor.matmul(
                                sT[:, 0:256],
                                kTt[hp][j][r0:r0 + 64, 256:384],
                                qTt[hp][j][r0:r0 + 64, 256:512],
                                start=True, stop=True, skip_group_check=True)
                            nc.tensor.matmul(
                                sT[:, 256:512],
                                kTt[hp][j][r0:r0 + 64, 384:512],
                                qTt[hp][j][r0:r0 + 64, 256:512],
                                start=False, stop=True, skip_group_check=True)
                            nc.scalar.activation(et[:, 0:512], sT[:, 0:512],
                                                 Exp, scale=EXP_SCALE)
                            et3 = et[:].rearrange("p (a n) -> p a n", n=128)
                            nc.vector.tensor_mul(et3[:, 0:4:3, :], et3[:, 0:4:3, :],
                                                 trm[:])
                            stride, width = 256, 256
                        etv = et[:, 0:2 * stride].rearrange(
                            "p (a n) -> p a n", a=2)
                        s_lo = 4 if m2 else 0
                        base = 256 if m2 else 0

                        def attn_emit(et=et, etv=etv, q=q, h=h, po=po,
                                      po_t=po_t, zc=zc,
                                      s_lo=s_lo, base=base, m0=m0, m2=m2, j=j):
                            # exactly ONE start=True per bank-use: start=True
                            # flags the whole 2KB bank pending-zero; every
                            # other group's first touch consumes its flag.
                            if j == 0:
                                # bf16 non-DR path (no fp8 noise on the short-
                                # support rows). (tile, subtiles, flat offset fn)
                                if m0:
                                    work = [(0, range(0, 8), lambda s: 64 * s),
                                            (1, range(2, 8),
                                             lambda s: 512 + 64 * s)]
                                else:
                                    work = [(0, range(4, 8),
                                             lambda s: 64 * s - 256),
                                            (1, range(6, 8), lambda s: 64 * s)]
                                stop_at = {0: (0, 0), 1: (0, 0), 2: (0, 1),
                                           3: (0, 1), 4: (1, 0), 5: (1, 0),
                                           6: (1, 1), 7: (1, 1)}
                                for tt, srange, off in work:
                                    for s in srange:
                                        fo = off(s)
                                        stop = stop_at[s] == (q, tt)
                                        st = (q == 0 and tt == 0 and s == 0)
                                        nc.tensor.matmul(
                                            po[:, 64 * s:64 * s + 64],
                                            et[:, fo:fo + 64],
                                            vb[q][:, tt, h, 0:64],
                                            start=st, stop=stop,
                                            skip_group_check=True)
                                        nc.tensor.matmul(
                                            po_t[64:128, s:s + 1],
                                            et[:, fo:fo + 64],
                                            vb[q][:, tt, h, 64:65],
                                            start=st, stop=stop,
                                            skip_group_check=True)
                                return
                            for s in range(s_lo, 8):
                                cc = 64 * s - base
                                stop = (s < 4 and m0) or (s >= 4 and m2)
                                # tile1's dead block is never read: subtiles
                                # under the diagonal use a single-tile matmul
                                single = (m0 and s < 2) or (m2 and s < 6)
                                if single:
                                    lhs_o = etv[:, 0, cc:cc + 64]
                                    rhs_o = vp[q][:, 0, h, 0:64]
                                    pm = None
                                else:
                                    lhs_o = etv[:, :, cc:cc + 64]
                                    rhs_o = vp[q][:, :, h, 0:64]
                                    pm = DR
                                nc.tensor.matmul(
                                    po[:, 64 * s:64 * s + 64],
                                    lhs_o, rhs_o,
                                    start=(q == 0 and s == 0), stop=stop,
                                    perf_mode=pm, skip_group_check=True)
                                # Z columns live at partitions 64:128 of the
                                # same po bank (single-tile, non-DR: base-64)
                                nc.tensor.matmul(
                                    po_t[64:128, s:s + 1],
                                    etv[:, 0, cc:cc + 64],
                                    vp[q][:, 0, h, 64:65],
                                    start=(q == 0 and s == 0), stop=(stop and single),
                                    skip_group_check=True)
                                if not single:
                                    nc.tensor.matmul(
                                        po_t[64:128, s:s + 1],
                                        etv[:, 1, cc:cc + 64],
                                        vp[q][:, 1, h, 64:65],
                                        start=False, stop=stop,
                                        skip_group_check=True)
                        # one-pair software pipeline: previous pair's attnV/Z
                        # runs while this pair's exp/masks are in flight
                        while pend:
                            pend.pop(0)()
                        pend.append(attn_emit)
                        drain_bg(3 if h == HL - 1 else (2 if len(bg) > 40 else 1))

                    force_bg(lambda t: (t[0] == "vq" and t[1] <= 2 * j + 1) or
                             (j == 0 and t[0] == "vb"))

                    def norm_emit(po=po, po_t=po_t, h=h, hp=hp,
                                  ot_tile=ot_tile):
                        rz = rzp.tile([64, 8], F32, tag="rz", name="rz")
                        nc.vector.reciprocal(rz[:], po_t[64:128, 0:8])
                        o_sb = osb.tile([64, 512], BF16, tag="os", name="os")
                        nc.vector.tensor_tensor(
                            o_sb[:].rearrange("p (e s) -> p e s", e=8),
                            po[:].rearrange("p (e s) -> p e s", e=8),
                            rz[:].unsqueeze(2).broadcast_to([64, 8, 64]),
                            mybir.AluOpType.mult)
                        o_sb_pair[h % 2] = o_sb
                        if h % 2 == 1:
                            def transpose_emit(hp=hp, ot_tile=ot_tile,
                                               pair=dict(o_sb_pair)):
                                pt_t = ps_po.tile([128, 512], F32, tag="po",
                                                  name="pt")
                                pt = pt_t[:].bitcast(BF16)[:, 0:512]
                                for hh in range(2):
                                    for s in range(8):
                                        nc.tensor.matmul(
                                            pt[64 * hh:64 * hh + 64,
                                               64 * s:64 * s + 64],
                                            pair[hh][:, 64 * s:64 * s + 64],
                                            idt[:], is_transpose=True,
                                            start=(s == 0),
                                            stop=(hh == 1 and s == 7),
                                            skip_group_check=True)
                                nc.vector.tensor_copy(ot_tile[:, hp, :], pt[:])
                            bg.insert(min(1, len(bg)),
                                      (("tr", None), transpose_emit))
                    pend.append(norm_emit)

                # flush the pipeline at the j boundary so the last head pair's
                # normalize + transpose are queued before proj tasks
                while pend:
                    pend.pop(0)()
                for t in range(4):
                    for nh in range(2):
                        pending_proj.append((("proj", si),
                                             proj_tile(j, t, nh, ot_tile)))

            while pend:
                pend.pop(0)()
            drain_bg(len(bg))
            for _, task in pending_proj:
                task()

    nc.compile()
    return nc


def _host_prep(x, Wq, Wk, Wv, Wp):
    """Per-core input maps."""
    tri = (np.arange(128)[None, :] >= np.arange(128)[:, None]).astype(np.float32)
    tri2 = np.concatenate([tri, tri], axis=1).astype(f8np)
    ident = np.eye(64, dtype=np.float32).astype(bfnp)
    WpT = np.ascontiguousarray(Wp.T) * (1.0 / SV)  # [E(hd), E]

    def wslice(W, h0, scale, dt=f8np):  # [H,E,D] -> [E, 8*64]
        w = W[h0:h0 + HL].transpose(1, 0, 2).reshape(E, HL * D) * scale
        return np.ascontiguousarray(w).astype(dt)

    in_maps = []
    for c in range(8):
        b, hh = c // 2, c % 2
        h0 = hh * HL
        in_maps.append({
            "xT8": np.ascontiguousarray(x[b].T).astype(f8np),
            "wq8": wslice(Wq, h0, SW),
            "wk8": wslice(Wk, h0, SW),
            "wv8": wslice(Wv, h0, SV),
            "wpT": np.ascontiguousarray(
                WpT[h0 * D:(h0 + HL) * D, :]).astype(bfnp),
            "tri2": tri2,
            "tri2b": tri2.astype(np.float32).astype(bfnp),
            "ident": ident,
            "xbT": np.ascontiguousarray(x[b].T[:, 0:512]).astype(bfnp),
            "wqb": wslice(Wq, h0, SW, bfnp),
            "wkb": wslice(Wk, h0, SW, bfnp),
            "wvb": wslice(Wv, h0, SV, bfnp),
        })
    return in_maps


def kernel(x, Wq, Wk, Wv, Wp, bp):
    x = np.asarray(x, dtype=np.float32)
    Wq = np.asarray(Wq, dtype=np.float32)
    Wk = np.asarray(Wk, dtype=np.float32)
    Wv = np.asarray(Wv, dtype=np.float32)
    Wp = np.asarray(Wp, dtype=np.float32)
    bp = np.asarray(bp, dtype=np.float32)

    if "nc" not in _CACHE:
        _CACHE["nc"] = _build()
    nc = _CACHE["nc"]

    in_maps = _host_prep(x, Wq, Wk, Wv, Wp)
    res = run_bass_kernel_spmd(nc, in_maps, list(range(8)))
    parts = [np.asarray(res.results[c]["out"], dtype=np.float32) for c in range(8)]
    out = np.stack([parts[2 * b] + parts[2 * b + 1] for b in range(B)], axis=0)
    return (out + bp[None, None, :]).astype(np.float32)


# revision 108
# speedup vs baseline: 1.3203x; 1.0016x over previous
"""Multi-head causal attention (B=4,T=2048,E=1024,H=16,D=64) on 8 TRN2 cores.

Sharding: core c -> batch b=c//2, heads h0=(c%2)*8 .. h0+8. Each core computes
its 8 heads' attention and a partial output projection (row-split Wp); host
sums the two partials per batch (+bias).

Per-core kernel (tq tiles processed in order 1,2,3,0):
  - Q/K/V projections in fp8e4 DoubleRow (2 k-tiles of 128 per instr, 0.5
    cyc/col), M=64 outputs at PSUM base 0. Host pre-scales Wq,Wk by 64 and
    Wv by 16 (powers of 2, folded back via exp scale 2^-15 / Wp scale).
  - scores: bf16 kT.T @ qT per 128-row tk chunk; chunk PAIRS share one
    [128,1024] PSUM tile (2 banks) so exp runs as one ACT op per pair.
    Diagonal chunks compute their dead region too so exp never reads
    stale PSUM (race-free; CoreSim conflict-checker clean).
  - exp -> fp8e4 "et" tiles [128, 2, N]; causal triangle masked by DVE
    multiplies; fully-dead subtiles skip the second DR k-tile instead of
    being zeroed.
  - attnV: o-layout fp8 DoubleRow o[tq64, d64] += et.T @ v over chunk
    pairs; softmax Z accumulates as single-tile matmul columns in the
    SAME po bank at partitions 64:127 (base-64 is legal for non-DR).
    Normalize is per-partition: DVE reciprocal [64,8] + broadcast mult.
  - o -> oT via PE transposes (identity matmul) packed 2 heads/bank,
    then bf16 output projection, DVE copy, DMA out.
  - j=0 (first 512 tokens) runs attention in bf16 (v, q/k for tokens
    0:128 recomputed from bf16 x/W): short-support softmax rows cannot
    average away fp8 quantization noise. Processed last so its extra
    DMAs/projections never gate the critical path.
  - Emission is software-pipelined: attnV/Z trail scores by one pair and
    background work (projections, transposes, output tiles) drains from
    a tagged queue with just-in-time forced ordering.
"""
import sys
import numpy as np

sys.path.insert(0, "/opt/trn_rl_repo")

import ml_dtypes
import concourse.bass as bass
import concourse.bacc as bacc
import concourse.mybir as mybir
from concourse import tile
from concourse.bass_utils import run_bass_kernel_spmd

B, T, E, H, D = 4, 2048, 1024, 16, 64
HL = H // 2          # 8 local heads per core
NJ = T // 512        # 4 tq tiles
NPE = E // 256       # 4 E-chunk-pairs
NPAIR = T // 256     # 8 tk chunk pairs
BF16 = mybir.dt.bfloat16
F32 = mybir.dt.float32
FP8 = mybir.dt.float8e4
DR = mybir.MatmulPerfMode.DoubleRow
Exp = mybir.ActivationFunctionType.Exp
f8np = ml_dtypes.float8_e4m3
bfnp = ml_dtypes.bfloat16

SW = 64.0   # Wq/Wk host prescale (exp scale folds 1/SW^2)
SV = 16.0   # Wv host prescale (Wp folds 1/SV)
EXP_SCALE = 0.125 / (SW * SW)  # 2^-15 exactly

_CACHE = {}


def _build():
    nc = bacc.Bacc("TRN2", target_bir_lowering=False)
    xT8 = nc.declare_dram_parameter("xT8", [E, T], FP8, isOutput=False)
    wq8 = nc.declare_dram_parameter("wq8", [E, HL * D], FP8, isOutput=False)
    wk8 = nc.declare_dram_parameter("wk8", [E, HL * D], FP8, isOutput=False)
    wv8 = nc.declare_dram_parameter("wv8", [E, HL * D], FP8, isOutput=False)
    wpT = nc.declare_dram_parameter("wpT", [HL * D, E], BF16, isOutput=False)
    tri2 = nc.declare_dram_parameter("tri2", [128, 256], FP8, isOutput=False)
    tri2b = nc.declare_dram_parameter("tri2b", [128, 256], BF16, isOutput=False)
    xbT = nc.declare_dram_parameter("xbT", [E, 512], BF16, isOutput=False)
    wqb = nc.declare_dram_parameter("wqb", [E, HL * D], BF16, isOutput=False)
    wkb = nc.declare_dram_parameter("wkb", [E, HL * D], BF16, isOutput=False)
    wvb = nc.declare_dram_parameter("wvb", [E, HL * D], BF16, isOutput=False)
    ident = nc.declare_dram_parameter("ident", [64, 64], BF16, isOutput=False)
    out = nc.declare_dram_parameter("out", [T, E], F32, isOutput=True)

    with tile.TileContext(nc) as tc:
        with (
            tc.tile_pool(name="pp", bufs=1) as pp,
            tc.tile_pool(name="etp", bufs=14) as etp,
            tc.tile_pool(name="osb", bufs=10) as osb,
            tc.tile_pool(name="rzp", bufs=8) as rzp,
            tc.tile_pool(name="otp", bufs=3) as otp,
            tc.tile_pool(name="obp", bufs=6) as obp,
            tc.tile_pool(name="sT", bufs=2, space=bass.MemorySpace.PSUM) as ps_sT,
            tc.tile_pool(name="po", bufs=2, space=bass.MemorySpace.PSUM) as ps_po,
            tc.tile_pool(name="mm", bufs=2, space=bass.MemorySpace.PSUM) as ps_mm,
        ):
            # ---- persistent SBUF tiles
            x8t = [pp.tile([128, 2, T], FP8, tag=f"x{p}", name=f"x{p}")
                   for p in range(NPE)]
            wq8t = [pp.tile([128, 2, 512], FP8, tag=f"wq{p}", name=f"wq{p}")
                    for p in range(NPE)]
            wk8t = [pp.tile([128, 2, 512], FP8, tag=f"wk{p}", name=f"wk{p}")
                    for p in range(NPE)]
            wv8t = [pp.tile([128, 2, 512], FP8, tag=f"wv{p}", name=f"wv{p}")
                    for p in range(NPE)]
            wpt = [pp.tile([128, E], BF16, tag=f"wp{p}", name=f"wp{p}")
                   for p in range(4)]
            trit = pp.tile([128, 2, 128], FP8, tag="tri", name="trit")
            tritb = pp.tile([128, 2, 128], BF16, tag="trib", name="tritb")
            idt = pp.tile([64, 64], BF16, tag="id", name="idt")
            qTt = [[pp.tile([128, 512], BF16, tag=f"q{p}_{j}", name=f"q{p}_{j}")
                    for j in range(NJ)] for p in range(4)]
            kTt = [[pp.tile([128, 512], BF16, tag=f"k{p}_{j}", name=f"k{p}_{j}")
                    for j in range(NJ)] for p in range(4)]
            vp = [pp.tile([128, 2, HL, 65], FP8, tag=f"v{q}", name=f"v{q}")
                  for q in range(NPAIR)]
            # bf16 v for chunks 0..3: j=0 attention runs in bf16 (short-support
            # softmax rows can't average away fp8 quantization noise)
            vb = [pp.tile([128, 2, HL, 65], BF16, tag=f"vb{q}", name=f"vb{q}")
                  for q in range(2)]
            # bf16 x/W and q/k for the first 128 tokens (block-0 scores)
            xbt = [pp.tile([128, 2, 512], BF16, tag=f"xb{p}", name=f"xb{p}")
                   for p in range(NPE)]
            wqbt = [pp.tile([128, 2, 512], BF16, tag=f"wqb{p}", name=f"wqb{p}")
                    for p in range(NPE)]
            wkbt = [pp.tile([128, 2, 512], BF16, tag=f"wkb{p}", name=f"wkb{p}")
                    for p in range(NPE)]
            wvbt = [pp.tile([128, 2, 512], BF16, tag=f"wvb{p}", name=f"wvb{p}")
                    for p in range(NPE)]
            qbt = [pp.tile([128, 128], BF16, tag=f"qb{p}", name=f"qb{p}")
                   for p in range(4)]
            kbt = [pp.tile([128, 128], BF16, tag=f"kb{p}", name=f"kb{p}")
                   for p in range(4)]

            # ---- DMA in. j tiles are processed in order [1,2,3,0], so the
            # critical path is x cols [0:1024] + wv (SP) and wq/wk (ACT).
            # The bf16 sidecar tiles (j=0 accuracy path) ride at the SP tail.
            def tdma(queue, dst, src, cols=None):
                # both t-halves in one DMA: src rows (t p) -> dst [p, t, n]
                s = src if cols is None else src[:, cols[0]:cols[1]]
                queue.dma_start(dst, s.rearrange("(t p) n -> p t n", t=2))
            for p in range(NPE):
                tdma(nc.sync, x8t[p][:, :, 0:512], xT8[256 * p:256 * p + 256],
                     (0, 512))
                tdma(nc.sync, wv8t[p][:], wv8[256 * p:256 * p + 256])
            for p in range(NPE):
                tdma(nc.sync, wq8t[p][:], wq8[256 * p:256 * p + 256])
                tdma(nc.sync, wk8t[p][:], wk8[256 * p:256 * p + 256])
            for p in range(NPE):
                tdma(nc.sync, x8t[p][:, :, 512:1024],
                     xT8[256 * p:256 * p + 256], (512, 1024))
            nc.sync.dma_start(trit[:], tri2[:].rearrange("p (a n) -> p a n", a=2))
            nc.sync.dma_start(idt[:], ident[:])
            for jj in range(2, NJ):
                for p in range(NPE):
                    tdma(nc.sync, x8t[p][:, :, 512 * jj:512 * jj + 512],
                         xT8[256 * p:256 * p + 256], (512 * jj, 512 * jj + 512))
            for p in range(4):
                nc.sync.dma_start(wpt[p][:], wpT[128 * p:128 * p + 128, :])
            nc.sync.dma_start(tritb[:],
                              tri2b[:].rearrange("p (a n) -> p a n", a=2))
            for p in range(NPE):
                tdma(nc.sync, xbt[p][:], xbT[256 * p:256 * p + 256])
                tdma(nc.sync, wvbt[p][:], wvb[256 * p:256 * p + 256])
                tdma(nc.sync, wqbt[p][:], wqb[256 * p:256 * p + 256])
                tdma(nc.sync, wkbt[p][:], wkb[256 * p:256 * p + 256])
            for q in range(NPAIR):
                nc.gpsimd.memset(vp[q][:, :, :, 64:65], 1.0)
            for q in range(2):
                nc.gpsimd.memset(vb[q][:, :, :, 64:65], 1.0)

            # ---- background task machinery (qkv groups, proj tiles)
            def qk_group(kind, h, j, pool=None):
                def emit():
                    wt = wq8t if kind == "q" else wk8t
                    dst = qTt if kind == "q" else kTt
                    if pool is None:
                        m = ps_mm.tile([128, 512], F32, tag="mm", name="mmq")
                    else:
                        m = pool.tile([128, 1024], F32, tag="sT",
                                      name="mmq")[:, 0:512]
                    for p in range(NPE):
                        nc.tensor.matmul(
                            m[0:64, :], wt[p][:, :, 64 * h:64 * h + 64],
                            x8t[p][:, :, 512 * j:512 * j + 512],
                            start=(p == 0), stop=(p == NPE - 1), perf_mode=DR)
                    nc.vector.tensor_copy(
                        dst[h // 2][j][64 * (h % 2):64 * (h % 2) + 64, :],
                        m[0:64, :])
                return emit

            def qkb_group(pe_hp):  # bf16 q/k for tokens 0..127, head pair
                def emit():
                    for wt, dst in ((wqbt, qbt), (wkbt, kbt)):
                        m = ps_mm.tile([128, 512], F32, tag="mm", name="mmb")
                        for p in range(NPE):
                            for t in range(2):
                                nc.tensor.matmul(
                                    m[0:128, 0:128],
                                    wt[p][:, t, 128 * pe_hp:128 * pe_hp + 128],
                                    xbt[p][:, t, 0:128],
                                    start=(p == 0 and t == 0),
                                    stop=(p == NPE - 1 and t == 1))
                        nc.vector.tensor_copy(dst[pe_hp][:], m[0:128, 0:128])
                return emit

            def v_group(c, g, pool=None):  # chunk c, t-64 sub g
                def emit():
                    if pool is None:
                        m = ps_mm.tile([128, 512], F32, tag="mm", name="mmv")
                    else:
                        m = pool.tile([128, 1024], F32, tag="sT",
                                      name="mmv")[:, 0:512]
                    t0 = 128 * c + 64 * g
                    for p in range(NPE):
                        nc.tensor.matmul(
                            m[0:64, :], x8t[p][:, :, t0:t0 + 64], wv8t[p][:],
                            start=(p == 0), stop=(p == NPE - 1), perf_mode=DR)
                    nc.vector.tensor_copy(
                        vp[c // 2][64 * g:64 * g + 64, c % 2, :, 0:64],
                        m[0:64, :].rearrange("p (h d) -> p h d", d=64))
                return emit

            def vb_group(c, g):  # true bf16 v for j=0 (bf16 x and Wv)
                def emit():
                    m = ps_mm.tile([128, 512], F32, tag="mm", name="mvb")
                    t0 = 128 * c + 64 * g
                    for p in range(NPE):
                        for t in range(2):
                            nc.tensor.matmul(
                                m[0:64, :],
                                xbt[p][:, t, t0:t0 + 64],
                                wvbt[p][:, t, :],
                                start=(p == 0 and t == 0),
                                stop=(p == NPE - 1 and t == 1))
                    nc.vector.tensor_copy(
                        vb[c // 2][64 * g:64 * g + 64, c % 2, :, 0:64],
                        m[0:64, :].rearrange("p (h d) -> p h d", d=64))
                return emit

            def proj_tile(j, t, nh, ot_tile):
                def emit():
                    m = ps_mm.tile([128, 512], F32, tag="mm", name="mmp")
                    for p in range(4):
                        nc.tensor.matmul(
                            m[:], ot_tile[:, p, 128 * t:128 * t + 128],
                            wpt[p][:, 512 * nh:512 * nh + 512],
                            start=(p == 0), stop=(p == 3))
                    ob = obp.tile([128, 512], F32, tag="ob", name="ob")
                    nc.vector.tensor_copy(ob[:], m[:])
                    nc.sync.dma_start(
                        out[512 * j + 128 * t:512 * j + 128 * t + 128,
                            512 * nh:512 * nh + 512], ob[:])
                return emit

            bg = []  # queue of (tag, emitter); tags order forced at stream starts

            def drain_bg(n=1):
                for _ in range(min(n, len(bg))):
                    bg.pop(0)[1]()

            def force_bg(pred):
                """Emit from the front until no queued task satisfies pred."""
                while any(pred(t) for t, _ in bg):
                    bg.pop(0)[1]()

            # j tiles processed [1, 2, 3, 0]: the bf16 j=0 accuracy path runs
            # last so its DMAs/projections never gate the critical path.
            JORDER = (1, 2, 3, 0)
            # prologue: v pairs 0..1 (chunks 0..3); q tile 1, k tiles 0..1
            # for h=0. k projections are scheduled by tk-tile index: stream
            # (j, h) consumes k tiles 0..j.
            # interleave so the first stream's scores are unblocked ASAP
            qk_group("k", 0, 0)()
            v_group(0, 0, ps_sT)()
            v_group(0, 1)()
            qk_group("k", 0, 1, ps_sT)()
            v_group(1, 0)()
            v_group(1, 1, ps_sT)()
            qk_group("q", 0, 1)()
            for c in (2, 3):
                for g in range(2):
                    bg.append((("vq", 1), v_group(c, g)))
            for q in range(2, 4):
                for g in range(2):
                    bg.append((("vq", q), v_group(2 * q, g)))
                    bg.append((("vq", q), v_group(2 * q + 1, g)))
            for h in range(1, HL):
                bg.append((("q", 1, h), qk_group("q", h, 1)))
                bg.append((("k", 0, h), qk_group("k", h, 0)))
                bg.append((("k", 1, h), qk_group("k", h, 1)))
            # bf16 sidecar tasks wait on tail-end DMAs; keep them out of the
            # main drain queue until their inputs have surely landed
            bg_late = []
            for hp in range(4):
                bg_late.append((("qkb", 2 * hp), qkb_group(hp)))
            for c in range(4):
                for g in range(2):
                    bg_late.append((("vb", c // 2), vb_group(c, g)))

            ot_tiles = {}
            pend = []       # one-pair-lagged PE emissions (attnV/Z, normalize)
            o_sb_pair = {}  # normalized o for the in-flight head pair

            pending_proj = []
            for si, j in enumerate(JORDER):
                if si == 1:  # release the bf16 sidecar work mid-flight
                    bg.extend(bg_late)
                    bg_late = []
                # enqueue the next sequence step's inputs
                if si + 1 < NJ:
                    nj = JORDER[si + 1]
                    if nj != 0:
                        for q in range(2 * nj, 2 * nj + 2):
                            for g in range(2):
                                bg.append((("vq", q), v_group(2 * q, g)))
                                bg.append((("vq", q), v_group(2 * q + 1, g)))
                    for h in range(HL):
                        bg.append((("q", nj, h), qk_group("q", h, nj)))
                        if nj != 0:  # k tile 0 was produced in the prologue
                            bg.append((("k", nj, h), qk_group("k", h, nj)))
                # weave last step's proj tiles into the fresh queue (avoids a
                # PE-only burst that starves ACT at the step boundary); all
                # transposes must go first (proj reads their output)
                if pending_proj:
                    force_bg(lambda t: t[0] == "tr")
                for i, task in enumerate(pending_proj):
                    bg.insert(min(5 * i + 2, len(bg)), task)
                pending_proj = []
                # proj from two sequence steps back must be out before its
                # ot slot is reused
                force_bg(lambda t: t[0] == "proj" and t[1] <= si - 2)
                ot_tile = otp.tile([128, 4, 512], BF16, tag="ot", name=f"ot{j}")
                ot_tiles[j] = ot_tile

                for h in range(HL):
                    hp, r0 = h // 2, 64 * (h % 2)
                    zc = 8 * (8 * j + h)
                    # everything this stream reads must already be emitted
                    force_bg(lambda t: (t[0] == "q" and t[1] == j and t[2] <= h) or
                             (t[0] == "k" and t[1] <= j and t[2] <= h) or
                             (j == 0 and t[0] == "qkb" and t[1] <= h))
                    po_t = ps_po.tile([128, 512], F32, tag="po", name="po")
                    po = po_t[0:64, :]
                    nq = 2 * j + 2
                    for q in range(nq):
                        m0, m2 = (q == 2 * j), (q == 2 * j + 1)
                        force_bg(lambda t: (t[0] == "vq" and t[1] < q) or
                                 (j == 0 and t[0] == "vb" and t[1] < q))
                        sT = ps_sT.tile([128, 1024], F32, tag="sT", name="sT")
                        if j == 0:
                            et = etp.tile([128, 1024], BF16, tag="etb",
                                          name="etb", bufs=4)
                            trm = tritb
                        else:
                            et = etp.tile([128, 1024], FP8, tag="et", name="et")
                            trm = trit
                        if not (m0 or m2):
                            # off-diag pair: both chunks full [0:512]
                            for t in range(2):
                                c = 2 * q + t
                                nc.tensor.matmul(
                                    sT[:, 512 * t:512 * t + 512],
                                    kTt[hp][c // 4][r0:r0 + 64,
                                                    128 * (c % 4):128 * (c % 4) + 128],
                                    qTt[hp][j][r0:r0 + 64, :],
                                    start=True, stop=True, skip_group_check=True)
                            nc.scalar.activation(et[:], sT[:], Exp, scale=EXP_SCALE)
                            stride, width = 512, 512
                        elif m0:
                            # chunks 4j (full), 4j+1 (cols 128:512)
                            c = 4 * j
                            if j == 0:
                                # block-0 scores from bf16-accurate q/k
                                nc.tensor.matmul(
                                    sT[:, 0:128],
                                    kbt[hp][r0:r0 + 64, :],
                                    qbt[hp][r0:r0 + 64, :],
                                    start=True, stop=False,
                                    skip_group_check=True)
                                nc.tensor.matmul(
                                    sT[:, 128:512],
                                    kTt[hp][j][r0:r0 + 64, 0:128],
                                    qTt[hp][j][r0:r0 + 64, 128:512],
                                    start=False, stop=True,
                                    skip_group_check=True)
                            else:
                                nc.tensor.matmul(
                                    sT[:, 0:512],
                                    kTt[hp][j][r0:r0 + 64, 0:128],
                                    qTt[hp][j][r0:r0 + 64, :],
                                    start=True, stop=True,
                                    skip_group_check=True)
                            # cover [512:640] too so exp never reads stale
                            # bytes (those weights get memset to 0 after)
                            nc.tensor.matmul(
                                sT[:, 512:1024],
                                kTt[hp][j][r0:r0 + 64, 128:256],
                                qTt[hp][j][r0:r0 + 64, :],
                                start=True, stop=True, skip_group_check=True)
                            nc.scalar.activation(et[:], sT[:], Exp, scale=EXP_SCALE)
                            et3 = et[:].rearrange("p (a n) -> p a n", n=128)
                            nc.vector.tensor_mul(et3[:, 0:6:5, :], et3[:, 0:6:5, :],
                                                 trm[:])
                            stride, width = 512, 512
                        else:
                            # m2: chunks 4j+2 (cols 256:512 -> [0:256]),
                            #     4j+3 (cols 384:512 -> [384:512])
                            nc.tensor.matmul(
                                sT[:, 0:256],
                                kTt[hp][j][r0:r0 + 64, 256:384],
                                qTt[hp][j][r0:r0 + 64, 256:512],
                                start=True, stop=True, skip_group_check=True)
                            nc.tensor.matmul(
                                sT[:, 256:512],
                                kTt[hp][j][r0:r0 + 64, 384:512],
                                qTt[hp][j][r0:r0 + 64, 256:512],
                                start=False, stop=True, skip_group_check=True)
                            nc.scalar.activation(et[:, 0:512], sT[:, 0:512],
                                                 Exp, scale=EXP_SCALE)
                            et3 = et[:].rearrange("p (a n) -> p a n", n=128)
                            nc.vector.tensor_mul(et3[:, 0:4:3, :], et3[:, 0:4:3, :],
                                                 trm[:])
                            stride, width = 256, 256
                        etv = et[:, 0:2 * stride].rearrange(
                            "p (a n) -> p a n", a=2)
                        s_lo = 4 if m2 else 0
                        base = 256 if m2 else 0

                        def attn_emit(et=et, etv=etv, q=q, h=h, po=po,
                                      po_t=po_t, zc=zc,
                                      s_lo=s_lo, base=base, m0=m0, m2=m2, j=j):
                            # exactly ONE start=True per bank-use: start=True
                            # flags the whole 2KB bank pending-zero; every
                            # other group's first touch consumes its flag.
                            if j == 0:
                                # bf16 non-DR path (no fp8 noise on the short-
                                # support rows). (tile, subtiles, flat offset fn)
                                if m0:
                                    work = [(0, range(0, 8), lambda s: 64 * s),
                                            (1, range(2, 8),
                                             lambda s: 512 + 64 * s)]
                                else:
                                    work = [(0, range(4, 8),
                                             lambda s: 64 * s - 256),
                                            (1, range(6, 8), lambda s: 64 * s)]
                                stop_at = {0: (0, 0), 1: (0, 0), 2: (0, 1),
                                           3: (0, 1), 4: (1, 0), 5: (1, 0),
                                           6: (1, 1), 7: (1, 1)}
                                for tt, srange, off in work:
                                    for s in srange:
                                        fo = off(s)
                                        stop = stop_at[s] == (q, tt)
                                        st = (q == 0 and tt == 0 and s == 0)
                                        nc.tensor.matmul(
                                            po[:, 64 * s:64 * s + 64],
                                            et[:, fo:fo + 64],
                                            vb[q][:, tt, h, 0:64],
                                            start=st, stop=stop,
                                            skip_group_check=True)
                                        nc.tensor.matmul(
                                            po_t[64:128, s:s + 1],
                                            et[:, fo:fo + 64],
                                            vb[q][:, tt, h, 64:65],
                                            start=st, stop=stop,
                                            skip_group_check=True)
                                return
                            for s in range(s_lo, 8):
                                cc = 64 * s - base
                                stop = (s < 4 and m0) or (s >= 4 and m2)
                                # tile1's dead block is never read: subtiles
                                # under the diagonal use a single-tile matmul
                                single = (m0 and s < 2) or (m2 and s < 6)
                                if single:
                                    lhs_o = etv[:, 0, cc:cc + 64]
                                    rhs_o = vp[q][:, 0, h, 0:64]
                                    pm = None
                                else:
                                    lhs_o = etv[:, :, cc:cc + 64]
                                    rhs_o = vp[q][:, :, h, 0:64]
                                    pm = DR
                                nc.tensor.matmul(
                                    po[:, 64 * s:64 * s + 64],
                                    lhs_o, rhs_o,
                                    start=(q == 0 and s == 0), stop=stop,
                                    perf_mode=pm, skip_group_check=True)
                                # Z columns live at partitions 64:128 of the
                                # same po bank (single-tile, non-DR: base-64)
                                nc.tensor.matmul(
                                    po_t[64:128, s:s + 1],
                                    etv[:, 0, cc:cc + 64],
                                    vp[q][:, 0, h, 64:65],
                                    start=(q == 0 and s == 0), stop=(stop and single),
                                    skip_group_check=True)
                                if not single:
                                    nc.tensor.matmul(
                                        po_t[64:128, s:s + 1],
                                        etv[:, 1, cc:cc + 64],
                                        vp[q][:, 1, h, 64:65],
                                        start=False, stop=stop,
                                        skip_group_check=True)
                        # one-pair software pipeline: previous pair's attnV/Z
                        # runs while this pair's exp/masks are in flight
                        while pend:
                            pend.pop(0)()
                        pend.append(attn_emit)
                        drain_bg(3 if h == HL - 1 else (2 if len(bg) > 40 else 1))

                    force_bg(lambda t: (t[0] == "vq" and t[1] <= 2 * j + 1) or
                             (j == 0 and t[0] == "vb"))

                    def norm_emit(po=po, po_t=po_t, h=h, hp=hp,
                                  ot_tile=ot_tile):
                        rz = rzp.tile([64, 8], F32, tag="rz", name="rz")
                        nc.vector.reciprocal(rz[:], po_t[64:128, 0:8])
                        o_sb = osb.tile([64, 512], BF16, tag="os", name="os")
                        nc.vector.tensor_tensor(
                            o_sb[:].rearrange("p (e s) -> p e s", e=8),
                            po[:].rearrange("p (e s) -> p e s", e=8),
                            rz[:].unsqueeze(2).broadcast_to([64, 8, 64]),
                            mybir.AluOpType.mult)
                        o_sb_pair[h % 2] = o_sb
                        if h % 2 == 1:
                            def transpose_emit(hp=hp, ot_tile=ot_tile,
                                               pair=dict(o_sb_pair)):
                                pt_t = ps_po.tile([128, 512], F32, tag="po",
                                                  name="pt")
                                pt = pt_t[:].bitcast(BF16)[:, 0:512]
                                for hh in range(2):
                                    for s in range(8):
                                        nc.tensor.matmul(
                                            pt[64 * hh:64 * hh + 64,
                                               64 * s:64 * s + 64],
                                            pair[hh][:, 64 * s:64 * s + 64],
                                            idt[:], is_transpose=True,
                                            start=(s == 0),
                                            stop=(hh == 1 and s == 7),
                                            skip_group_check=True)
                                nc.vector.tensor_copy(ot_tile[:, hp, :], pt[:])
                            bg.insert(min(1, len(bg)),
                                      (("tr", None), transpose_emit))
                    pend.append(norm_emit)

                # flush the pipeline at the j boundary so the last head pair's
                # normalize + transpose are queued before proj tasks
                while pend:
                    pend.pop(0)()
                for t in range(4):
                    for nh in range(2):
                        pending_proj.append((("proj", si),
                                             proj_tile(j, t, nh, ot_tile)))

            while pend:
                pend.pop(0)()
            drain_bg(len(bg))
            for _, task in pending_proj:
                task()

    nc.compile()
    return nc


def _host_prep(x, Wq, Wk, Wv, Wp):
    """Per-core input maps."""
    tri = (np.arange(128)[None, :] >= np.arange(128)[:, None]).astype(np.float32)
    tri2 = np.concatenate([tri, tri], axis=1).astype(f8np)
    ident = np.eye(64, dtype=np.float32).astype(bfnp)
    WpT = np.ascontiguousarray(Wp.T) * (1.0 / SV)  # [E(hd), E]

    def wslice(W, h0, scale, dt=f8np):  # [H,E,D] -> [E, 8*64]
        w = W[h0:h0 + HL].transpose(1, 0, 2).reshape(E, HL * D) * scale
        return np.ascontiguousarray(w).astype(dt)

    in_maps = []
    for c in range(8):
        b, hh = c // 2, c % 2
        h0 = hh * HL
        in_maps.append({
            "xT8": np.ascontiguousarray(x[b].T).astype(f8np),
            "wq8": wslice(Wq, h0, SW),
            "wk8": wslice(Wk, h0, SW),
            "wv8": wslice(Wv, h0, SV),
            "wpT": np.ascontiguousarray(
                WpT[h0 * D:(h0 + HL) * D, :]).astype(bfnp),
            "tri2": tri2,
            "tri2b": tri2.astype(np.float32).astype(bfnp),
            "ident": ident,
            "xbT": np.ascontiguousarray(x[b].T[:, 0:512]).astype(bfnp),
            "wqb": wslice(Wq, h0, SW, bfnp),
            "wkb": wslice(Wk, h0, SW, bfnp),
            "wvb": wslice(Wv, h0, SV, bfnp),
        })
    return in_maps


def kernel(x, Wq, Wk, Wv, Wp, bp):
    x = np.asarray(x, dtype=np.float32)
    Wq = np.asarray(Wq, dtype=np.float32)
    Wk = np.asarray(Wk, dtype=np.float32)
    Wv = np.asarray(Wv, dtype=np.float32)
    Wp = np.asarray(Wp, dtype=np.float32)
    bp = np.asarray(bp, dtype=np.float32)

    if "nc" not in _CACHE:
        _CACHE["nc"] = _build()
    nc = _CACHE["nc"]

    in_maps = _host_prep(x, Wq, Wk, Wv, Wp)
    res = run_bass_kernel_spmd(nc, in_maps, list(range(8)))
    parts = [np.asarray(res.results[c]["out"], dtype=np.float32) for c in range(8)]
    out = np.stack([parts[2 * b] + parts[2 * b + 1] for b in range(B)], axis=0)
    return (out + bp[None, None, :]).astype(np.float32)


# revision 109
# speedup vs baseline: 1.3204x; 1.0000x over previous
"""Multi-head causal attention (B=4,T=2048,E=1024,H=16,D=64) on 8 TRN2 cores.

Sharding: core c -> batch b=c//2, heads h0=(c%2)*8 .. h0+8. Each core computes
its 8 heads' attention and a partial output projection (row-split Wp); host
sums the two partials per batch (+bias).

Per-core kernel (tq tiles processed in order 1,2,3,0):
  - Q/K/V projections in fp8e4 DoubleRow (2 k-tiles of 128 per instr, 0.5
    cyc/col), M=64 outputs at PSUM base 0. Host pre-scales Wq,Wk by 64 and
    Wv by 16 (powers of 2, folded back via exp scale 2^-15 / Wp scale).
  - scores: bf16 kT.T @ qT per 128-row tk chunk; chunk PAIRS share one
    [128,1024] PSUM tile (2 banks) so exp runs as one ACT op per pair.
    Diagonal chunks compute their dead region too so exp never reads
    stale PSUM (race-free; CoreSim conflict-checker clean).
  - exp -> fp8e4 "et" tiles [128, 2, N]; causal triangle masked by DVE
    multiplies; fully-dead subtiles skip the second DR k-tile instead of
    being zeroed.
  - attnV: o-layout fp8 DoubleRow o[tq64, d64] += et.T @ v over chunk
    pairs; softmax Z accumulates as single-tile matmul columns in the
    SAME po bank at partitions 64:127 (base-64 is legal for non-DR).
    Normalize is per-partition: DVE reciprocal [64,8] + broadcast mult.
  - o -> oT via PE transposes (identity matmul) packed 2 heads/bank,
    then bf16 output projection, DVE copy, DMA out.
  - j=0 (first 512 tokens) runs attention in bf16 (v, q/k for tokens
    0:128 recomputed from bf16 x/W): short-support softmax rows cannot
    average away fp8 quantization noise. Processed last so its extra
    DMAs/projections never gate the critical path.
  - Emission is software-pipelined: attnV/Z trail scores by one pair and
    background work (projections, transposes, output tiles) drains from
    a tagged queue with just-in-time forced ordering.
"""
import sys
import numpy as np

sys.path.insert(0, "/opt/trn_rl_repo")

import ml_dtypes
import concourse.bass as bass
import concourse.bacc as bacc
import concourse.mybir as mybir
from concourse import tile
from concourse.bass_utils import run_bass_kernel_spmd

B, T, E, H, D = 4, 2048, 1024, 16, 64
HL = H // 2          # 8 local heads per core
NJ = T // 512        # 4 tq tiles
NPE = E // 256       # 4 E-chunk-pairs
NPAIR = T // 256     # 8 tk chunk pairs
BF16 = mybir.dt.bfloat16
F32 = mybir.dt.float32
FP8 = mybir.dt.float8e4
DR = mybir.MatmulPerfMode.DoubleRow
Exp = mybir.ActivationFunctionType.Exp
f8np = ml_dtypes.float8_e4m3
bfnp = ml_dtypes.bfloat16

SW = 64.0   # Wq/Wk host prescale (exp scale folds 1/SW^2)
SV = 16.0   # Wv host prescale (Wp folds 1/SV)
EXP_SCALE = 0.125 / (SW * SW)  # 2^-15 exactly

_CACHE = {}


def _build():
    nc = bacc.Bacc("TRN2", target_bir_lowering=False)
    xT8 = nc.declare_dram_parameter("xT8", [E, T], FP8, isOutput=False)
    wq8 = nc.declare_dram_parameter("wq8", [E, HL * D], FP8, isOutput=False)
    wk8 = nc.declare_dram_parameter("wk8", [E, HL * D], FP8, isOutput=False)
    wv8 = nc.declare_dram_parameter("wv8", [E, HL * D], FP8, isOutput=False)
    wpT = nc.declare_dram_parameter("wpT", [HL * D, E], BF16, isOutput=False)
    tri2 = nc.declare_dram_parameter("tri2", [128, 256], FP8, isOutput=False)
    tri2b = nc.declare_dram_parameter("tri2b", [128, 256], BF16, isOutput=False)
    xbT = nc.declare_dram_parameter("xbT", [E, 512], BF16, isOutput=False)
    wqb = nc.declare_dram_parameter("wqb", [E, HL * D], BF16, isOutput=False)
    wkb = nc.declare_dram_parameter("wkb", [E, HL * D], BF16, isOutput=False)
    wvb = nc.declare_dram_parameter("wvb", [E, HL * D], BF16, isOutput=False)
    ident = nc.declare_dram_parameter("ident", [64, 64], BF16, isOutput=False)
    out = nc.declare_dram_parameter("out", [T, E], F32, isOutput=True)

    with tile.TileContext(nc) as tc:
        with (
            tc.tile_pool(name="pp", bufs=1) as pp,
            tc.tile_pool(name="etp", bufs=14) as etp,
            tc.tile_pool(name="osb", bufs=10) as osb,
            tc.tile_pool(name="rzp", bufs=8) as rzp,
            tc.tile_pool(name="otp", bufs=3) as otp,
            tc.tile_pool(name="obp", bufs=6) as obp,
            tc.tile_pool(name="sT", bufs=2, space=bass.MemorySpace.PSUM) as ps_sT,
            tc.tile_pool(name="po", bufs=2, space=bass.MemorySpace.PSUM) as ps_po,
            tc.tile_pool(name="mm", bufs=2, space=bass.MemorySpace.PSUM) as ps_mm,
        ):
            # ---- persistent SBUF tiles
            x8t = [pp.tile([128, 2, T], FP8, tag=f"x{p}", name=f"x{p}")
                   for p in range(NPE)]
            wq8t = [pp.tile([128, 2, 512], FP8, tag=f"wq{p}", name=f"wq{p}")
                    for p in range(NPE)]
            wk8t = [pp.tile([128, 2, 512], FP8, tag=f"wk{p}", name=f"wk{p}")
                    for p in range(NPE)]
            wv8t = [pp.tile([128, 2, 512], FP8, tag=f"wv{p}", name=f"wv{p}")
                    for p in range(NPE)]
            wpt = [pp.tile([128, E], BF16, tag=f"wp{p}", name=f"wp{p}")
                   for p in range(4)]
            trit = pp.tile([128, 2, 128], FP8, tag="tri", name="trit")
            tritb = pp.tile([128, 2, 128], BF16, tag="trib", name="tritb")
            idt = pp.tile([64, 64], BF16, tag="id", name="idt")
            qTt = [[pp.tile([128, 512], BF16, tag=f"q{p}_{j}", name=f"q{p}_{j}")
                    for j in range(NJ)] for p in range(4)]
            kTt = [[pp.tile([128, 512], BF16, tag=f"k{p}_{j}", name=f"k{p}_{j}")
                    for j in range(NJ)] for p in range(4)]
            vp = [pp.tile([128, 2, HL, 65], FP8, tag=f"v{q}", name=f"v{q}")
                  for q in range(NPAIR)]
            # bf16 v for chunks 0..3: j=0 attention runs in bf16 (short-support
            # softmax rows can't average away fp8 quantization noise)
            vb = [pp.tile([128, 2, HL, 65], BF16, tag=f"vb{q}", name=f"vb{q}")
                  for q in range(2)]
            # bf16 x/W and q/k for the first 128 tokens (block-0 scores)
            xbt = [pp.tile([128, 2, 512], BF16, tag=f"xb{p}", name=f"xb{p}")
                   for p in range(NPE)]
            wqbt = [pp.tile([128, 2, 512], BF16, tag=f"wqb{p}", name=f"wqb{p}")
                    for p in range(NPE)]
            wkbt = [pp.tile([128, 2, 512], BF16, tag=f"wkb{p}", name=f"wkb{p}")
                    for p in range(NPE)]
            wvbt = [pp.tile([128, 2, 512], BF16, tag=f"wvb{p}", name=f"wvb{p}")
                    for p in range(NPE)]
            qbt = [pp.tile([128, 128], BF16, tag=f"qb{p}", name=f"qb{p}")
                   for p in range(4)]
            kbt = [pp.tile([128, 128], BF16, tag=f"kb{p}", name=f"kb{p}")
                   for p in range(4)]

            # ---- DMA in. j tiles are processed in order [1,2,3,0], so the
            # critical path is x cols [0:1024] + wv (SP) and wq/wk (ACT).
            # The bf16 sidecar tiles (j=0 accuracy path) ride at the SP tail.
            def tdma(queue, dst, src, cols=None):
                # both t-halves in one DMA: src rows (t p) -> dst [p, t, n]
                s = src if cols is None else src[:, cols[0]:cols[1]]
                queue.dma_start(dst, s.rearrange("(t p) n -> p t n", t=2))
            for p in range(NPE):
                tdma(nc.sync, x8t[p][:, :, 0:512], xT8[256 * p:256 * p + 256],
                     (0, 512))
                tdma(nc.sync, wv8t[p][:], wv8[256 * p:256 * p + 256])
                tdma(nc.sync, wk8t[p][:], wk8[256 * p:256 * p + 256])
                tdma(nc.sync, wq8t[p][:], wq8[256 * p:256 * p + 256])
            for p in range(NPE):
                tdma(nc.sync, x8t[p][:, :, 512:1024],
                     xT8[256 * p:256 * p + 256], (512, 1024))
            nc.sync.dma_start(trit[:], tri2[:].rearrange("p (a n) -> p a n", a=2))
            nc.sync.dma_start(idt[:], ident[:])
            for jj in range(2, NJ):
                for p in range(NPE):
                    tdma(nc.sync, x8t[p][:, :, 512 * jj:512 * jj + 512],
                         xT8[256 * p:256 * p + 256], (512 * jj, 512 * jj + 512))
            for p in range(4):
                nc.sync.dma_start(wpt[p][:], wpT[128 * p:128 * p + 128, :])
            nc.sync.dma_start(tritb[:],
                              tri2b[:].rearrange("p (a n) -> p a n", a=2))
            for p in range(NPE):
                tdma(nc.sync, xbt[p][:], xbT[256 * p:256 * p + 256])
                tdma(nc.sync, wvbt[p][:], wvb[256 * p:256 * p + 256])
                tdma(nc.sync, wqbt[p][:], wqb[256 * p:256 * p + 256])
                tdma(nc.sync, wkbt[p][:], wkb[256 * p:256 * p + 256])
            for q in range(NPAIR):
                nc.gpsimd.memset(vp[q][:, :, :, 64:65], 1.0)
            for q in range(2):
                nc.gpsimd.memset(vb[q][:, :, :, 64:65], 1.0)

            # ---- background task machinery (qkv groups, proj tiles)
            def qk_group(kind, h, j, pool=None):
                def emit():
                    wt = wq8t if kind == "q" else wk8t
                    dst = qTt if kind == "q" else kTt
                    if pool is None:
                        m = ps_mm.tile([128, 512], F32, tag="mm", name="mmq")
                    else:
                        m = pool.tile([128, 1024], F32, tag="sT",
                                      name="mmq")[:, 0:512]
                    for p in range(NPE):
                        nc.tensor.matmul(
                            m[0:64, :], wt[p][:, :, 64 * h:64 * h + 64],
                            x8t[p][:, :, 512 * j:512 * j + 512],
                            start=(p == 0), stop=(p == NPE - 1), perf_mode=DR)
                    nc.vector.tensor_copy(
                        dst[h // 2][j][64 * (h % 2):64 * (h % 2) + 64, :],
                        m[0:64, :])
                return emit

            def qkb_group(pe_hp):  # bf16 q/k for tokens 0..127, head pair
                def emit():
                    for wt, dst in ((wqbt, qbt), (wkbt, kbt)):
                        m = ps_mm.tile([128, 512], F32, tag="mm", name="mmb")
                        for p in range(NPE):
                            for t in range(2):
                                nc.tensor.matmul(
                                    m[0:128, 0:128],
                                    wt[p][:, t, 128 * pe_hp:128 * pe_hp + 128],
                                    xbt[p][:, t, 0:128],
                                    start=(p == 0 and t == 0),
                                    stop=(p == NPE - 1 and t == 1))
                        nc.vector.tensor_copy(dst[pe_hp][:], m[0:128, 0:128])
                return emit

            def v_group(c, g, pool=None):  # chunk c, t-64 sub g
                def emit():
                    if pool is None:
                        m = ps_mm.tile([128, 512], F32, tag="mm", name="mmv")
                    else:
                        m = pool.tile([128, 1024], F32, tag="sT",
                                      name="mmv")[:, 0:512]
                    t0 = 128 * c + 64 * g
                    for p in range(NPE):
                        nc.tensor.matmul(
                            m[0:64, :], x8t[p][:, :, t0:t0 + 64], wv8t[p][:],
                            start=(p == 0), stop=(p == NPE - 1), perf_mode=DR)
                    nc.vector.tensor_copy(
                        vp[c // 2][64 * g:64 * g + 64, c % 2, :, 0:64],
                        m[0:64, :].rearrange("p (h d) -> p h d", d=64))
                return emit

            def vb_group(c, g):  # true bf16 v for j=0 (bf16 x and Wv)
                def emit():
                    m = ps_mm.tile([128, 512], F32, tag="mm", name="mvb")
                    t0 = 128 * c + 64 * g
                    for p in range(NPE):
                        for t in range(2):
                            nc.tensor.matmul(
                                m[0:64, :],
                                xbt[p][:, t, t0:t0 + 64],
                                wvbt[p][:, t, :],
                                start=(p == 0 and t == 0),
                                stop=(p == NPE - 1 and t == 1))
                    nc.vector.tensor_copy(
                        vb[c // 2][64 * g:64 * g + 64, c % 2, :, 0:64],
                        m[0:64, :].rearrange("p (h d) -> p h d", d=64))
                return emit

            def proj_tile(j, t, nh, ot_tile):
                def emit():
                    m = ps_mm.tile([128, 512], F32, tag="mm", name="mmp")
                    for p in range(4):
                        nc.tensor.matmul(
                            m[:], ot_tile[:, p, 128 * t:128 * t + 128],
                            wpt[p][:, 512 * nh:512 * nh + 512],
                            start=(p == 0), stop=(p == 3))
                    ob = obp.tile([128, 512], F32, tag="ob", name="ob")
                    nc.vector.tensor_copy(ob[:], m[:])
                    nc.sync.dma_start(
                        out[512 * j + 128 * t:512 * j + 128 * t + 128,
                            512 * nh:512 * nh + 512], ob[:])
                return emit

            bg = []  # queue of (tag, emitter); tags order forced at stream starts

            def drain_bg(n=1):
                for _ in range(min(n, len(bg))):
                    bg.pop(0)[1]()

            def force_bg(pred):
                """Emit from the front until no queued task satisfies pred."""
                while any(pred(t) for t, _ in bg):
                    bg.pop(0)[1]()

            # j tiles processed [1, 2, 3, 0]: the bf16 j=0 accuracy path runs
            # last so its DMAs/projections never gate the critical path.
            JORDER = (1, 2, 3, 0)
            # prologue: v pairs 0..1 (chunks 0..3); q tile 1, k tiles 0..1
            # for h=0. k projections are scheduled by tk-tile index: stream
            # (j, h) consumes k tiles 0..j.
            # interleave so the first stream's scores are unblocked ASAP
            qk_group("k", 0, 0)()
            v_group(0, 0, ps_sT)()
            v_group(0, 1)()
            qk_group("k", 0, 1, ps_sT)()
            v_group(1, 0)()
            v_group(1, 1, ps_sT)()
            qk_group("q", 0, 1)()
            for c in (2, 3):
                for g in range(2):
                    bg.append((("vq", 1), v_group(c, g)))
            for q in range(2, 4):
                for g in range(2):
                    bg.append((("vq", q), v_group(2 * q, g)))
                    bg.append((("vq", q), v_group(2 * q + 1, g)))
            for h in range(1, HL):
                bg.append((("q", 1, h), qk_group("q", h, 1)))
                bg.append((("k", 0, h), qk_group("k", h, 0)))
                bg.append((("k", 1, h), qk_group("k", h, 1)))
            # bf16 sidecar tasks wait on tail-end DMAs; keep them out of the
            # main drain queue until their inputs have surely landed
            bg_late = []
            for hp in range(4):
                bg_late.append((("qkb", 2 * hp), qkb_group(hp)))
            for c in range(4):
                for g in range(2):
                    bg_late.append((("vb", c // 2), vb_group(c, g)))

            ot_tiles = {}
            pend = []       # one-pair-lagged PE emissions (attnV/Z, normalize)
            o_sb_pair = {}  # normalized o for the in-flight head pair

            pending_proj = []
            for si, j in enumerate(JORDER):
                if si == 1:  # release the bf16 sidecar work mid-flight
                    bg.extend(bg_late)
                    bg_late = []
                # enqueue the next sequence step's inputs
                if si + 1 < NJ:
                    nj = JORDER[si + 1]
                    if nj != 0:
                        for q in range(2 * nj, 2 * nj + 2):
                            for g in range(2):
                                bg.append((("vq", q), v_group(2 * q, g)))
                                bg.append((("vq", q), v_group(2 * q + 1, g)))
                    for h in range(HL):
                        bg.append((("q", nj, h), qk_group("q", h, nj)))
                        if nj != 0:  # k tile 0 was produced in the prologue
                            bg.append((("k", nj, h), qk_group("k", h, nj)))
                # weave last step's proj tiles into the fresh queue (avoids a
                # PE-only burst that starves ACT at the step boundary); all
                # transposes must go first (proj reads their output)
                if pending_proj:
                    force_bg(lambda t: t[0] == "tr")
                for i, task in enumerate(pending_proj):
                    bg.insert(min(5 * i + 2, len(bg)), task)
                pending_proj = []
                # proj from two sequence steps back must be out before its
                # ot slot is reused
                force_bg(lambda t: t[0] == "proj" and t[1] <= si - 2)
                ot_tile = otp.tile([128, 4, 512], BF16, tag="ot", name=f"ot{j}")
                ot_tiles[j] = ot_tile

                for h in range(HL):
                    hp, r0 = h // 2, 64 * (h % 2)
                    zc = 8 * (8 * j + h)
                    # everything this stream reads must already be emitted
                    force_bg(lambda t: (t[0] == "q" and t[1] == j and t[2] <= h) or
                             (t[0] == "k" and t[1] <= j and t[2] <= h) or
                             (j == 0 and t[0] == "qkb" and t[1] <= h))
                    po_t = ps_po.tile([128, 512], F32, tag="po", name="po")
                    po = po_t[0:64, :]
                    nq = 2 * j + 2
                    for q in range(nq):
                        m0, m2 = (q == 2 * j), (q == 2 * j + 1)
                        force_bg(lambda t: (t[0] == "vq" and t[1] < q) or
                                 (j == 0 and t[0] == "vb" and t[1] < q))
                        sT = ps_sT.tile([128, 1024], F32, tag="sT", name="sT")
                        if j == 0:
                            et = etp.tile([128, 1024], BF16, tag="etb",
                                          name="etb", bufs=4)
                            trm = tritb
                        else:
                            et = etp.tile([128, 1024], FP8, tag="et", name="et")
                            trm = trit
                        if not (m0 or m2):
                            # off-diag pair: both chunks full [0:512]
                            for t in range(2):
                                c = 2 * q + t
                                nc.tensor.matmul(
                                    sT[:, 512 * t:512 * t + 512],
                                    kTt[hp][c // 4][r0:r0 + 64,
                                                    128 * (c % 4):128 * (c % 4) + 128],
                                    qTt[hp][j][r0:r0 + 64, :],
                                    start=True, stop=True, skip_group_check=True)
                            nc.scalar.activation(et[:], sT[:], Exp, scale=EXP_SCALE)
                            stride, width = 512, 512
                        elif m0:
                            # chunks 4j (full), 4j+1 (cols 128:512)
                            c = 4 * j
                            if j == 0:
                                # block-0 scores from bf16-accurate q/k
                                nc.tensor.matmul(
                                    sT[:, 0:128],
                                    kbt[hp][r0:r0 + 64, :],
                                    qbt[hp][r0:r0 + 64, :],
                                    start=True, stop=False,
                                    skip_group_check=True)
                                nc.tensor.matmul(
                                    sT[:, 128:512],
                                    kTt[hp][j][r0:r0 + 64, 0:128],
                                    qTt[hp][j][r0:r0 + 64, 128:512],
                                    start=False, stop=True,
                                    skip_group_check=True)
                            else:
                                nc.tensor.matmul(
                                    sT[:, 0:512],
                                    kTt[hp][j][r0:r0 + 64, 0:128],
                                    qTt[hp][j][r0:r0 + 64, :],
                                    start=True, stop=True,
                                    skip_group_check=True)
                            # cover [512:640] too so exp never reads stale
                            # bytes (those weights get memset to 0 after)
                            nc.tensor.matmul(
                                sT[:, 512:1024],
                                kTt[hp][j][r0:r0 + 64, 128:256],
                                qTt[hp][j][r0:r0 + 64, :],
                                start=True, stop=True, skip_group_check=True)
                            nc.scalar.activation(et[:], sT[:], Exp, scale=EXP_SCALE)
                            et3 = et[:].rearrange("p (a n) -> p a n", n=128)
                            nc.vector.tensor_mul(et3[:, 0:6:5, :], et3[:, 0:6:5, :],
                                                 trm[:])
                            stride, width = 512, 512
                        else:
                            # m2: chunks 4j+2 (cols 256:512 -> [0:256]),
                            #     4j+3 (cols 384:512 -> [384:512])
                            nc.tensor.matmul(
                                sT[:, 0:256],
                                kTt[hp][j][r0:r0 + 64, 256:384],
                                qTt[hp][j][r0:r0 + 64, 256:512],
                                start=True, stop=True, skip_group_check=True)
                            nc.tensor.matmul(
                                sT[:, 256:512],
                                kTt[hp][j][r0:r0 + 64, 384:512],
                                qTt[hp][j][r0:r0 + 64, 256:512],
                                start=False, stop=True, skip_group_check=True)
                            nc.scalar.activation(et[:, 0:512], sT[:, 0:512],
                                                 Exp, scale=EXP_SCALE)
                            et3 = et[:].rearrange("p (a n) -> p a n", n=128)
                            nc.vector.tensor_mul(et3[:, 0:4:3, :], et3[:, 0:4:3, :],
                                                 trm[:])
                            stride, width = 256, 256
                        etv = et[:, 0:2 * stride].rearrange(
                            "p (a n) -> p a n", a=2)
                        s_lo = 4 if m2 else 0
                        base = 256 if m2 else 0

                        def attn_emit(et=et, etv=etv, q=q, h=h, po=po,
                                      po_t=po_t, zc=zc,
                                      s_lo=s_lo, base=base, m0=m0, m2=m2, j=j):
                            # exactly ONE start=True per bank-use: start=True
                            # flags the whole 2KB bank pending-zero; every
                            # other group's first touch consumes its flag.
                            if j == 0:
                                # bf16 non-DR path (no fp8 noise on the short-
                                # support rows). (tile, subtiles, flat offset fn)
                                if m0:
                                    work = [(0, range(0, 8), lambda s: 64 * s),
                                            (1, range(2, 8),
                                             lambda s: 512 + 64 * s)]
                                else:
                                    work = [(0, range(4, 8),
                                             lambda s: 64 * s - 256),
                                            (1, range(6, 8), lambda s: 64 * s)]
                                stop_at = {0: (0, 0), 1: (0, 0), 2: (0, 1),
                                           3: (0, 1), 4: (1, 0), 5: (1, 0),
                                           6: (1, 1), 7: (1, 1)}
                                for tt, srange, off in work:
                                    for s in srange:
                                        fo = off(s)
                                        stop = stop_at[s] == (q, tt)
                                        st = (q == 0 and tt == 0 and s == 0)
                                        nc.tensor.matmul(
                                            po[:, 64 * s:64 * s + 64],
                                            et[:, fo:fo + 64],
                                            vb[q][:, tt, h, 0:64],
                                            start=st, stop=stop,
                                            skip_group_check=True)
                                        nc.tensor.matmul(
                                            po_t[64:128, s:s + 1],
                                            et[:, fo:fo + 64],
                                            vb[q][:, tt, h, 64:65],
                                            start=st, stop=stop,
                                            skip_group_check=True)
                                return
                            for s in range(s_lo, 8):
                                cc = 64 * s - base
                                stop = (s < 4 and m0) or (s >= 4 and m2)
                                # tile1's dead block is never read: subtiles
                                # under the diagonal use a single-tile matmul
                                single = (m0 and s < 2) or (m2 and s < 6)
                                if single:
                                    lhs_o = etv[:, 0, cc:cc + 64]
                                    rhs_o = vp[q][:, 0, h, 0:64]
                                    pm = None
                                else:
                                    lhs_o = etv[:, :, cc:cc + 64]
                                    rhs_o = vp[q][:, :, h, 0:64]
                                    pm = DR
                                nc.tensor.matmul(
                                    po[:, 64 * s:64 * s + 64],
                                    lhs_o, rhs_o,
                                    start=(q == 0 and s == 0), stop=stop,
                                    perf_mode=pm, skip_group_check=True)
                                # Z columns live at partitions 64:128 of the
                                # same po bank (single-tile, non-DR: base-64)
                                nc.tensor.matmul(
                                    po_t[64:128, s:s + 1],
                                    etv[:, 0, cc:cc + 64],
                                    vp[q][:, 0, h, 64:65],
                                    start=(q == 0 and s == 0), stop=(stop and single),
                                    skip_group_check=True)
                                if not single:
                                    nc.tensor.matmul(
                                        po_t[64:128, s:s + 1],
                                        etv[:, 1, cc:cc + 64],
                                        vp[q][:, 1, h, 64:65],
                                        start=False, stop=stop,
                                        skip_group_check=True)
                        # one-pair software pipeline: previous pair's attnV/Z
                        # runs while this pair's exp/masks are in flight
                        while pend:
                            pend.pop(0)()
                        pend.append(attn_emit)
                        drain_bg(3 if h == HL - 1 else (2 if len(bg) > 40 else 1))

                    force_bg(lambda t: (t[0] == "vq" and t[1] <= 2 * j + 1) or
                             (j == 0 and t[0] == "vb"))

                    def norm_emit(po=po, po_t=po_t, h=h, hp=hp,
                                  ot_tile=ot_tile):
                        rz = rzp.tile([64, 8], F32, tag="rz", name="rz")
                        nc.vector.reciprocal(rz[:], po_t[64:128, 0:8])
                        o_sb = osb.tile([64, 512], BF16, tag="os", name="os")
                        nc.vector.tensor_tensor(
                            o_sb[:].rearrange("p (e s) -> p e s", e=8),
                            po[:].rearrange("p (e s) -> p e s", e=8),
                            rz[:].unsqueeze(2).broadcast_to([64, 8, 64]),
                            mybir.AluOpType.mult)
                        o_sb_pair[h % 2] = o_sb
                        if h % 2 == 1:
                            def transpose_emit(hp=hp, ot_tile=ot_tile,
                                               pair=dict(o_sb_pair)):
                                pt_t = ps_po.tile([128, 512], F32, tag="po",
                                                  name="pt")
                                pt = pt_t[:].bitcast(BF16)[:, 0:512]
                                for hh in range(2):
                                    for s in range(8):
                                        nc.tensor.matmul(
                                            pt[64 * hh:64 * hh + 64,
                                               64 * s:64 * s + 64],
                                            pair[hh][:, 64 * s:64 * s + 64],
                                            idt[:], is_transpose=True,
                                            start=(s == 0),
                                            stop=(hh == 1 and s == 7),
                                            skip_group_check=True)
                                nc.vector.tensor_copy(ot_tile[:, hp, :], pt[:])
                            bg.insert(min(1, len(bg)),
                                      (("tr", None), transpose_emit))
                    pend.append(norm_emit)

                # flush the pipeline at the j boundary so the last head pair's
                # normalize + transpose are queued before proj tasks
                while pend:
                    pend.pop(0)()
                for t in range(4):
                    for nh in range(2):
                        pending_proj.append((("proj", si),
                                             proj_tile(j, t, nh, ot_tile)))

            while pend:
                pend.pop(0)()
            drain_bg(len(bg))
            for _, task in pending_proj:
                task()

    nc.compile()
    return nc


def _host_prep(x, Wq, Wk, Wv, Wp):
    """Per-core input maps."""
    tri = (np.arange(128)[None, :] >= np.arange(128)[:, None]).astype(np.float32)
    tri2 = np.concatenate([tri, tri], axis=1).astype(f8np)
    ident = np.eye(64, dtype=np.float32).astype(bfnp)
    WpT = np.ascontiguousarray(Wp.T) * (1.0 / SV)  # [E(hd), E]

    def wslice(W, h0, scale, dt=f8np):  # [H,E,D] -> [E, 8*64]
        w = W[h0:h0 + HL].transpose(1, 0, 2).reshape(E, HL * D) * scale
        return np.ascontiguousarray(w).astype(dt)

    in_maps = []
    for c in range(8):
        b, hh = c // 2, c % 2
        h0 = hh * HL
        in_maps.append({
            "xT8": np.ascontiguousarray(x[b].T).astype(f8np),
            "wq8": wslice(Wq, h0, SW),
            "wk8": wslice(Wk, h0, SW),
            "wv8": wslice(Wv, h0, SV),
            "wpT": np.ascontiguousarray(
                WpT[h0 * D:(h0 + HL) * D, :]).astype(bfnp),
            "tri2": tri2,
            "tri2b": tri2.astype(np.float32).astype(bfnp),
            "ident": ident,
            "xbT": np.ascontiguousarray(x[b].T[:, 0:512]).astype(bfnp),
            "wqb": wslice(Wq, h0, SW, bfnp),
            "wkb": wslice(Wk, h0, SW, bfnp),
            "wvb": wslice(Wv, h0, SV, bfnp),
        })
    return in_maps


def kernel(x, Wq, Wk, Wv, Wp, bp):
    x = np.asarray(x, dtype=np.float32)
    Wq = np.asarray(Wq, dtype=np.float32)
    Wk = np.asarray(Wk, dtype=np.float32)
    Wv = np.asarray(Wv, dtype=np.float32)
    Wp = np.asarray(Wp, dtype=np.float32)
    bp = np.asarray(bp, dtype=np.float32)

    if "nc" not in _CACHE:
        _CACHE["nc"] = _build()
    nc = _CACHE["nc"]

    in_maps = _host_prep(x, Wq, Wk, Wv, Wp)
    res = run_bass_kernel_spmd(nc, in_maps, list(range(8)))
    parts = [np.asarray(res.results[c]["out"], dtype=np.float32) for c in range(8)]
    out = np.stack([parts[2 * b] + parts[2 * b + 1] for b in range(B)], axis=0)
    return (out + bp[None, None, :]).astype(np.float32)


# revision 110
# speedup vs baseline: 1.3365x; 1.0122x over previous
"""Multi-head causal attention (B=4,T=2048,E=1024,H=16,D=64) on 8 TRN2 cores.

Sharding: core c -> batch b=c//2, heads h0=(c%2)*8 .. h0+8. Each core computes
its 8 heads' attention and a partial output projection (row-split Wp); host
sums the two partials per batch (+bias).

Per-core kernel (tq tiles processed in order 1,2,3,0):
  - Q/K/V projections in fp8e4 DoubleRow (2 k-tiles of 128 per instr, 0.5
    cyc/col), M=64 outputs at PSUM base 0. Host pre-scales Wq,Wk by 64 and
    Wv by 16 (powers of 2, folded back via exp scale 2^-15 / Wp scale).
  - scores: bf16 kT.T @ qT per 128-row tk chunk; chunk PAIRS share one
    [128,1024] PSUM tile (2 banks) so exp runs as one ACT op per pair.
    Diagonal chunks compute their dead region too so exp never reads
    stale PSUM (race-free; CoreSim conflict-checker clean).
  - exp -> fp8e4 "et" tiles [128, 2, N]; causal triangle masked by DVE
    multiplies; fully-dead subtiles skip the second DR k-tile instead of
    being zeroed.
  - attnV: o-layout fp8 DoubleRow o[tq64, d64] += et.T @ v over chunk
    pairs; softmax Z accumulates as single-tile matmul columns in the
    SAME po bank at partitions 64:127 (base-64 is legal for non-DR).
    Normalize is per-partition: DVE reciprocal [64,8] + broadcast mult.
  - o -> oT via PE transposes (identity matmul) packed 2 heads/bank,
    then bf16 output projection, DVE copy, DMA out.
  - j=0 (first 512 tokens) runs attention in bf16 (v, q/k for tokens
    0:128 recomputed from bf16 x/W): short-support softmax rows cannot
    average away fp8 quantization noise. Processed last so its extra
    DMAs/projections never gate the critical path.
  - Emission is software-pipelined: attnV/Z trail scores by one pair and
    background work (projections, transposes, output tiles) drains from
    a tagged queue with just-in-time forced ordering.
"""
import sys
import numpy as np

sys.path.insert(0, "/opt/trn_rl_repo")

import ml_dtypes
import concourse.bass as bass
import concourse.bacc as bacc
import concourse.mybir as mybir
from concourse import tile
from concourse.bass_utils import run_bass_kernel_spmd

B, T, E, H, D = 4, 2048, 1024, 16, 64
HL = H // 2          # 8 local heads per core
NJ = T // 512        # 4 tq tiles
NPE = E // 256       # 4 E-chunk-pairs
NPAIR = T // 256     # 8 tk chunk pairs
BF16 = mybir.dt.bfloat16
F32 = mybir.dt.float32
FP8 = mybir.dt.float8e4
DR = mybir.MatmulPerfMode.DoubleRow
Exp = mybir.ActivationFunctionType.Exp
f8np = ml_dtypes.float8_e4m3
bfnp = ml_dtypes.bfloat16

SW = 64.0   # Wq/Wk host prescale (exp scale folds 1/SW^2)
SV = 16.0   # Wv host prescale (Wp folds 1/SV)
EXP_SCALE = 0.125 / (SW * SW)  # 2^-15 exactly

_CACHE = {}


def _build():
    nc = bacc.Bacc("TRN2", target_bir_lowering=False)
    xT8 = nc.declare_dram_parameter("xT8", [E, T], FP8, isOutput=False)
    wq8 = nc.declare_dram_parameter("wq8", [E, HL * D], FP8, isOutput=False)
    wk8 = nc.declare_dram_parameter("wk8", [E, HL * D], FP8, isOutput=False)
    wv8 = nc.declare_dram_parameter("wv8", [E, HL * D], FP8, isOutput=False)
    wpT = nc.declare_dram_parameter("wpT", [HL * D, E], BF16, isOutput=False)
    tri2 = nc.declare_dram_parameter("tri2", [128, 256], FP8, isOutput=False)
    tri2b = nc.declare_dram_parameter("tri2b", [128, 256], BF16, isOutput=False)
    xbT = nc.declare_dram_parameter("xbT", [E, 512], BF16, isOutput=False)
    wqb = nc.declare_dram_parameter("wqb", [E, HL * D], BF16, isOutput=False)
    wkb = nc.declare_dram_parameter("wkb", [E, HL * D], BF16, isOutput=False)
    wvb = nc.declare_dram_parameter("wvb", [E, HL * D], BF16, isOutput=False)
    ident = nc.declare_dram_parameter("ident", [64, 64], BF16, isOutput=False)
    out = nc.declare_dram_parameter("out", [T, E], F32, isOutput=True)

    with tile.TileContext(nc) as tc:
        with (
            tc.tile_pool(name="pp", bufs=1) as pp,
            tc.tile_pool(name="etp", bufs=14) as etp,
            tc.tile_pool(name="osb", bufs=10) as osb,
            tc.tile_pool(name="rzp", bufs=8) as rzp,
            tc.tile_pool(name="otp", bufs=3) as otp,
            tc.tile_pool(name="obp", bufs=6) as obp,
            tc.tile_pool(name="sT", bufs=2, space=bass.MemorySpace.PSUM) as ps_sT,
            tc.tile_pool(name="po", bufs=2, space=bass.MemorySpace.PSUM) as ps_po,
            tc.tile_pool(name="mm", bufs=2, space=bass.MemorySpace.PSUM) as ps_mm,
        ):
            # ---- persistent SBUF tiles
            x8a = pp.tile([128, NPE, 2, T], FP8, tag="x8", name="x8a")
            wq8a = pp.tile([128, NPE, 2, 512], FP8, tag="wq", name="wq8a")
            wk8a = pp.tile([128, NPE, 2, 512], FP8, tag="wk", name="wk8a")
            wv8a = pp.tile([128, NPE, 2, 512], FP8, tag="wv", name="wv8a")
            x8t = [x8a[:, p] for p in range(NPE)]
            wq8t = [wq8a[:, p] for p in range(NPE)]
            wk8t = [wk8a[:, p] for p in range(NPE)]
            wv8t = [wv8a[:, p] for p in range(NPE)]
            wpt = [pp.tile([128, E], BF16, tag=f"wp{p}", name=f"wp{p}")
                   for p in range(4)]
            trit = pp.tile([128, 2, 128], FP8, tag="tri", name="trit")
            tritb = pp.tile([128, 2, 128], BF16, tag="trib", name="tritb")
            idt = pp.tile([64, 64], BF16, tag="id", name="idt")
            qTt = [[pp.tile([128, 512], BF16, tag=f"q{p}_{j}", name=f"q{p}_{j}")
                    for j in range(NJ)] for p in range(4)]
            kTt = [[pp.tile([128, 512], BF16, tag=f"k{p}_{j}", name=f"k{p}_{j}")
                    for j in range(NJ)] for p in range(4)]
            vp = [pp.tile([128, 2, HL, 65], FP8, tag=f"v{q}", name=f"v{q}")
                  for q in range(NPAIR)]
            # bf16 v for chunks 0..3: j=0 attention runs in bf16 (short-support
            # softmax rows can't average away fp8 quantization noise)
            vb = [pp.tile([128, 2, HL, 65], BF16, tag=f"vb{q}", name=f"vb{q}")
                  for q in range(2)]
            # bf16 x/W and q/k for the first 128 tokens (block-0 scores)
            xbt = [pp.tile([128, 2, 512], BF16, tag=f"xb{p}", name=f"xb{p}")
                   for p in range(NPE)]
            wqbt = [pp.tile([128, 2, 512], BF16, tag=f"wqb{p}", name=f"wqb{p}")
                    for p in range(NPE)]
            wkbt = [pp.tile([128, 2, 512], BF16, tag=f"wkb{p}", name=f"wkb{p}")
                    for p in range(NPE)]
            wvbt = [pp.tile([128, 2, 512], BF16, tag=f"wvb{p}", name=f"wvb{p}")
                    for p in range(NPE)]
            qbt = [pp.tile([128, 128], BF16, tag=f"qb{p}", name=f"qb{p}")
                   for p in range(4)]
            kbt = [pp.tile([128, 128], BF16, tag=f"kb{p}", name=f"kb{p}")
                   for p in range(4)]

            # ---- DMA in. j tiles are processed in order [1,2,3,0], so the
            # critical path is x cols [0:1024] + wv (SP) and wq/wk (ACT).
            # The bf16 sidecar tiles (j=0 accuracy path) ride at the SP tail.
            def cdma(queue, dst, src, cols=None):
                # whole class in one DMA: src rows (pe t p) -> [p, pe, t, n]
                s = src if cols is None else src[:, cols[0]:cols[1]]
                queue.dma_start(dst, s.rearrange("(pe t p) n -> p pe t n",
                                                 pe=NPE, t=2))

            def tdma(queue, dst, src, cols=None):
                # both t-halves in one DMA: src rows (t p) -> dst [p, t, n]
                s = src if cols is None else src[:, cols[0]:cols[1]]
                queue.dma_start(dst, s.rearrange("(t p) n -> p t n", t=2))
            cdma(nc.sync, x8a[:, :, :, 0:512], xT8, (0, 512))
            cdma(nc.sync, wk8a[:], wk8)
            cdma(nc.sync, wq8a[:], wq8)
            cdma(nc.sync, wv8a[:], wv8)
            cdma(nc.sync, x8a[:, :, :, 512:1024], xT8, (512, 1024))
            nc.sync.dma_start(trit[:], tri2[:].rearrange("p (a n) -> p a n", a=2))
            nc.sync.dma_start(idt[:], ident[:])
            for jj in range(2, NJ):
                cdma(nc.sync, x8a[:, :, :, 512 * jj:512 * jj + 512], xT8,
                     (512 * jj, 512 * jj + 512))
            for p in range(4):
                nc.sync.dma_start(wpt[p][:], wpT[128 * p:128 * p + 128, :])
            nc.sync.dma_start(tritb[:],
                              tri2b[:].rearrange("p (a n) -> p a n", a=2))
            for p in range(NPE):
                tdma(nc.sync, xbt[p][:], xbT[256 * p:256 * p + 256])
                tdma(nc.sync, wvbt[p][:], wvb[256 * p:256 * p + 256])
                tdma(nc.sync, wqbt[p][:], wqb[256 * p:256 * p + 256])
                tdma(nc.sync, wkbt[p][:], wkb[256 * p:256 * p + 256])
            for q in range(NPAIR):
                nc.gpsimd.memset(vp[q][:, :, :, 64:65], 1.0)
            for q in range(2):
                nc.gpsimd.memset(vb[q][:, :, :, 64:65], 1.0)

            # ---- background task machinery (qkv groups, proj tiles)
            def qk_group(kind, h, j, pool=None):
                def emit():
                    wt = wq8t if kind == "q" else wk8t
                    dst = qTt if kind == "q" else kTt
                    if pool is None:
                        m = ps_mm.tile([128, 512], F32, tag="mm", name="mmq")
                    else:
                        m = pool.tile([128, 1024], F32, tag="sT",
                                      name="mmq")[:, 0:512]
                    for p in range(NPE):
                        nc.tensor.matmul(
                            m[0:64, :], wt[p][:, :, 64 * h:64 * h + 64],
                            x8t[p][:, :, 512 * j:512 * j + 512],
                            start=(p == 0), stop=(p == NPE - 1), perf_mode=DR)
                    nc.vector.tensor_copy(
                        dst[h // 2][j][64 * (h % 2):64 * (h % 2) + 64, :],
                        m[0:64, :])
                return emit

            def qkb_group(pe_hp):  # bf16 q/k for tokens 0..127, head pair
                def emit():
                    for wt, dst in ((wqbt, qbt), (wkbt, kbt)):
                        m = ps_mm.tile([128, 512], F32, tag="mm", name="mmb")
                        for p in range(NPE):
                            for t in range(2):
                                nc.tensor.matmul(
                                    m[0:128, 0:128],
                                    wt[p][:, t, 128 * pe_hp:128 * pe_hp + 128],
                                    xbt[p][:, t, 0:128],
                                    start=(p == 0 and t == 0),
                                    stop=(p == NPE - 1 and t == 1))
                        nc.vector.tensor_copy(dst[pe_hp][:], m[0:128, 0:128])
                return emit

            def v_group(c, g, pool=None):  # chunk c, t-64 sub g
                def emit():
                    if pool is None:
                        m = ps_mm.tile([128, 512], F32, tag="mm", name="mmv")
                    else:
                        m = pool.tile([128, 1024], F32, tag="sT",
                                      name="mmv")[:, 0:512]
                    t0 = 128 * c + 64 * g
                    for p in range(NPE):
                        nc.tensor.matmul(
                            m[0:64, :], x8t[p][:, :, t0:t0 + 64], wv8t[p][:],
                            start=(p == 0), stop=(p == NPE - 1), perf_mode=DR)
                    nc.vector.tensor_copy(
                        vp[c // 2][64 * g:64 * g + 64, c % 2, :, 0:64],
                        m[0:64, :].rearrange("p (h d) -> p h d", d=64))
                return emit

            def vb_group(c, g):  # true bf16 v for j=0 (bf16 x and Wv)
                def emit():
                    m = ps_mm.tile([128, 512], F32, tag="mm", name="mvb")
                    t0 = 128 * c + 64 * g
                    for p in range(NPE):
                        for t in range(2):
                            nc.tensor.matmul(
                                m[0:64, :],
                                xbt[p][:, t, t0:t0 + 64],
                                wvbt[p][:, t, :],
                                start=(p == 0 and t == 0),
                                stop=(p == NPE - 1 and t == 1))
                    nc.vector.tensor_copy(
                        vb[c // 2][64 * g:64 * g + 64, c % 2, :, 0:64],
                        m[0:64, :].rearrange("p (h d) -> p h d", d=64))
                return emit

            def proj_tile(j, t, nh, ot_tile):
                def emit():
                    m = ps_mm.tile([128, 512], F32, tag="mm", name="mmp")
                    for p in range(4):
                        nc.tensor.matmul(
                            m[:], ot_tile[:, p, 128 * t:128 * t + 128],
                            wpt[p][:, 512 * nh:512 * nh + 512],
                            start=(p == 0), stop=(p == 3))
                    ob = obp.tile([128, 512], F32, tag="ob", name="ob")
                    nc.vector.tensor_copy(ob[:], m[:])
                    nc.sync.dma_start(
                        out[512 * j + 128 * t:512 * j + 128 * t + 128,
                            512 * nh:512 * nh + 512], ob[:])
                return emit

            bg = []  # queue of (tag, emitter); tags order forced at stream starts

            def drain_bg(n=1):
                for _ in range(min(n, len(bg))):
                    bg.pop(0)[1]()

            def force_bg(pred):
                """Emit from the front until no queued task satisfies pred."""
                while any(pred(t) for t, _ in bg):
                    bg.pop(0)[1]()

            # j tiles processed [1, 2, 3, 0]: the bf16 j=0 accuracy path runs
            # last so its DMAs/projections never gate the critical path.
            JORDER = (1, 2, 3, 0)
            # prologue: v pairs 0..1 (chunks 0..3); q tile 1, k tiles 0..1
            # for h=0. k projections are scheduled by tk-tile index: stream
            # (j, h) consumes k tiles 0..j.
            # interleave so the first stream's scores are unblocked ASAP
            qk_group("k", 0, 0)()
            v_group(0, 0, ps_sT)()
            v_group(0, 1)()
            qk_group("k", 0, 1, ps_sT)()
            v_group(1, 0)()
            v_group(1, 1, ps_sT)()
            qk_group("q", 0, 1)()
            for c in (2, 3):
                for g in range(2):
                    bg.append((("vq", 1), v_group(c, g)))
            for q in range(2, 4):
                for g in range(2):
                    bg.append((("vq", q), v_group(2 * q, g)))
                    bg.append((("vq", q), v_group(2 * q + 1, g)))
            for h in range(1, HL):
                bg.append((("q", 1, h), qk_group("q", h, 1)))
                bg.append((("k", 0, h), qk_group("k", h, 0)))
                bg.append((("k", 1, h), qk_group("k", h, 1)))
            # bf16 sidecar tasks wait on tail-end DMAs; keep them out of the
            # main drain queue until their inputs have surely landed
            bg_late = []
            for hp in range(4):
                bg_late.append((("qkb", 2 * hp), qkb_group(hp)))
            for c in range(4):
                for g in range(2):
                    bg_late.append((("vb", c // 2), vb_group(c, g)))

            ot_tiles = {}
            pend = []       # one-pair-lagged PE emissions (attnV/Z, normalize)
            o_sb_pair = {}  # normalized o for the in-flight head pair

            pending_proj = []
            for si, j in enumerate(JORDER):
                if si == 1:  # release the bf16 sidecar work mid-flight
                    bg.extend(bg_late)
                    bg_late = []
                # enqueue the next sequence step's inputs
                if si + 1 < NJ:
                    nj = JORDER[si + 1]
                    if nj != 0:
                        for q in range(2 * nj, 2 * nj + 2):
                            for g in range(2):
                                bg.append((("vq", q), v_group(2 * q, g)))
                                bg.append((("vq", q), v_group(2 * q + 1, g)))
                    for h in range(HL):
                        bg.append((("q", nj, h), qk_group("q", h, nj)))
                        if nj != 0:  # k tile 0 was produced in the prologue
                            bg.append((("k", nj, h), qk_group("k", h, nj)))
                # weave last step's proj tiles into the fresh queue (avoids a
                # PE-only burst that starves ACT at the step boundary); all
                # transposes must go first (proj reads their output)
                if pending_proj:
                    force_bg(lambda t: t[0] == "tr")
                for i, task in enumerate(pending_proj):
                    bg.insert(min(5 * i + 2, len(bg)), task)
                pending_proj = []
                # proj from two sequence steps back must be out before its
                # ot slot is reused
                force_bg(lambda t: t[0] == "proj" and t[1] <= si - 2)
                ot_tile = otp.tile([128, 4, 512], BF16, tag="ot", name=f"ot{j}")
                ot_tiles[j] = ot_tile

                for h in range(HL):
                    hp, r0 = h // 2, 64 * (h % 2)
                    zc = 8 * (8 * j + h)
                    # everything this stream reads must already be emitted
                    force_bg(lambda t: (t[0] == "q" and t[1] == j and t[2] <= h) or
                             (t[0] == "k" and t[1] <= j and t[2] <= h) or
                             (j == 0 and t[0] == "qkb" and t[1] <= h))
                    po_t = ps_po.tile([128, 512], F32, tag="po", name="po")
                    po = po_t[0:64, :]
                    nq = 2 * j + 2
                    for q in range(nq):
                        m0, m2 = (q == 2 * j), (q == 2 * j + 1)
                        force_bg(lambda t: (t[0] == "vq" and t[1] < q) or
                                 (j == 0 and t[0] == "vb" and t[1] < q))
                        sT = ps_sT.tile([128, 1024], F32, tag="sT", name="sT")
                        if j == 0:
                            et = etp.tile([128, 1024], BF16, tag="etb",
                                          name="etb", bufs=4)
                            trm = tritb
                        else:
                            et = etp.tile([128, 1024], FP8, tag="et", name="et")
                            trm = trit
                        if not (m0 or m2):
                            # off-diag pair: both chunks full [0:512]
                            for t in range(2):
                                c = 2 * q + t
                                nc.tensor.matmul(
                                    sT[:, 512 * t:512 * t + 512],
                                    kTt[hp][c // 4][r0:r0 + 64,
                                                    128 * (c % 4):128 * (c % 4) + 128],
                                    qTt[hp][j][r0:r0 + 64, :],
                                    start=True, stop=True, skip_group_check=True)
                            nc.scalar.activation(et[:], sT[:], Exp, scale=EXP_SCALE)
                            stride, width = 512, 512
                        elif m0:
                            # chunks 4j (full), 4j+1 (cols 128:512)
                            c = 4 * j
                            if j == 0:
                                # block-0 scores from bf16-accurate q/k
                                nc.tensor.matmul(
                                    sT[:, 0:128],
                                    kbt[hp][r0:r0 + 64, :],
                                    qbt[hp][r0:r0 + 64, :],
                                    start=True, stop=False,
                                    skip_group_check=True)
                                nc.tensor.matmul(
                                    sT[:, 128:512],
                                    kTt[hp][j][r0:r0 + 64, 0:128],
                                    qTt[hp][j][r0:r0 + 64, 128:512],
                                    start=False, stop=True,
                                    skip_group_check=True)
                            else:
                                nc.tensor.matmul(
                                    sT[:, 0:512],
                                    kTt[hp][j][r0:r0 + 64, 0:128],
                                    qTt[hp][j][r0:r0 + 64, :],
                                    start=True, stop=True,
                                    skip_group_check=True)
                            # cover [512:640] too so exp never reads stale
                            # bytes (those weights get memset to 0 after)
                            nc.tensor.matmul(
                                sT[:, 512:1024],
                                kTt[hp][j][r0:r0 + 64, 128:256],
                                qTt[hp][j][r0:r0 + 64, :],
                                start=True, stop=True, skip_group_check=True)
                            nc.scalar.activation(et[:], sT[:], Exp, scale=EXP_SCALE)
                            et3 = et[:].rearrange("p (a n) -> p a n", n=128)
                            nc.vector.tensor_mul(et3[:, 0:6:5, :], et3[:, 0:6:5, :],
                                                 trm[:])
                            stride, width = 512, 512
                        else:
                            # m2: chunks 4j+2 (cols 256:512 -> [0:256]),
                            #     4j+3 (cols 384:512 -> [384:512])
                            nc.tensor.matmul(
                                sT[:, 0:256],
                                kTt[hp][j][r0:r0 + 64, 256:384],
                                qTt[hp][j][r0:r0 + 64, 256:512],
                                start=True, stop=True, skip_group_check=True)
                            nc.tensor.matmul(
                                sT[:, 256:512],
                                kTt[hp][j][r0:r0 + 64, 384:512],
                                qTt[hp][j][r0:r0 + 64, 256:512],
                                start=False, stop=True, skip_group_check=True)
                            nc.scalar.activation(et[:, 0:512], sT[:, 0:512],
                                                 Exp, scale=EXP_SCALE)
                            et3 = et[:].rearrange("p (a n) -> p a n", n=128)
                            nc.vector.tensor_mul(et3[:, 0:4:3, :], et3[:, 0:4:3, :],
                                                 trm[:])
                            stride, width = 256, 256
                        etv = et[:, 0:2 * stride].rearrange(
                            "p (a n) -> p a n", a=2)
                        s_lo = 4 if m2 else 0
                        base = 256 if m2 else 0

                        def attn_emit(et=et, etv=etv, q=q, h=h, po=po,
                                      po_t=po_t, zc=zc,
                                      s_lo=s_lo, base=base, m0=m0, m2=m2, j=j):
                            # exactly ONE start=True per bank-use: start=True
                            # flags the whole 2KB bank pending-zero; every
                            # other group's first touch consumes its flag.
                            if j == 0:
                                # bf16 non-DR path (no fp8 noise on the short-
                                # support rows). (tile, subtiles, flat offset fn)
                                if m0:
                                    work = [(0, range(0, 8), lambda s: 64 * s),
                                            (1, range(2, 8),
                                             lambda s: 512 + 64 * s)]
                                else:
                                    work = [(0, range(4, 8),
                                             lambda s: 64 * s - 256),
                                            (1, range(6, 8), lambda s: 64 * s)]
                                stop_at = {0: (0, 0), 1: (0, 0), 2: (0, 1),
                                           3: (0, 1), 4: (1, 0), 5: (1, 0),
                                           6: (1, 1), 7: (1, 1)}
                                for tt, srange, off in work:
                                    for s in srange:
                                        fo = off(s)
                                        stop = stop_at[s] == (q, tt)
                                        st = (q == 0 and tt == 0 and s == 0)
                                        nc.tensor.matmul(
                                            po[:, 64 * s:64 * s + 64],
                                            et[:, fo:fo + 64],
                                            vb[q][:, tt, h, 0:64],
                                            start=st, stop=stop,
                                            skip_group_check=True)
                                        nc.tensor.matmul(
                                            po_t[64:128, s:s + 1],
                                            et[:, fo:fo + 64],
                                            vb[q][:, tt, h, 64:65],
                                            start=st, stop=stop,
                                            skip_group_check=True)
                                return
                            for s in range(s_lo, 8):
                                cc = 64 * s - base
                                stop = (s < 4 and m0) or (s >= 4 and m2)
                                # tile1's dead block is never read: subtiles
                                # under the diagonal use a single-tile matmul
                                single = (m0 and s < 2) or (m2 and s < 6)
                                if single:
                                    lhs_o = etv[:, 0, cc:cc + 64]
                                    rhs_o = vp[q][:, 0, h, 0:64]
                                    pm = None
                                else:
                                    lhs_o = etv[:, :, cc:cc + 64]
                                    rhs_o = vp[q][:, :, h, 0:64]
                                    pm = DR
                                nc.tensor.matmul(
                                    po[:, 64 * s:64 * s + 64],
                                    lhs_o, rhs_o,
                                    start=(q == 0 and s == 0), stop=stop,
                                    perf_mode=pm, skip_group_check=True)
                                # Z columns live at partitions 64:128 of the
                                # same po bank (single-tile, non-DR: base-64)
                                nc.tensor.matmul(
                                    po_t[64:128, s:s + 1],
                                    etv[:, 0, cc:cc + 64],
                                    vp[q][:, 0, h, 64:65],
                                    start=(q == 0 and s == 0), stop=(stop and single),
                                    skip_group_check=True)
                                if not single:
                                    nc.tensor.matmul(
                                        po_t[64:128, s:s + 1],
                                        etv[:, 1, cc:cc + 64],
                                        vp[q][:, 1, h, 64:65],
                                        start=False, stop=stop,
                                        skip_group_check=True)
                        # one-pair software pipeline: previous pair's attnV/Z
                        # runs while this pair's exp/masks are in flight
                        while pend:
                            pend.pop(0)()
                        pend.append(attn_emit)
                        drain_bg(3 if h == HL - 1 else (2 if len(bg) > 40 else 1))

                    force_bg(lambda t: (t[0] == "vq" and t[1] <= 2 * j + 1) or
                             (j == 0 and t[0] == "vb"))

                    def norm_emit(po=po, po_t=po_t, h=h, hp=hp,
                                  ot_tile=ot_tile):
                        rz = rzp.tile([64, 8], F32, tag="rz", name="rz")
                        nc.vector.reciprocal(rz[:], po_t[64:128, 0:8])
                        o_sb = osb.tile([64, 512], BF16, tag="os", name="os")
                        nc.vector.tensor_tensor(
                            o_sb[:].rearrange("p (e s) -> p e s", e=8),
                            po[:].rearrange("p (e s) -> p e s", e=8),
                            rz[:].unsqueeze(2).broadcast_to([64, 8, 64]),
                            mybir.AluOpType.mult)
                        o_sb_pair[h % 2] = o_sb
                        if h % 2 == 1:
                            def transpose_emit(hp=hp, ot_tile=ot_tile,
                                               pair=dict(o_sb_pair)):
                                pt_t = ps_po.tile([128, 512], F32, tag="po",
                                                  name="pt")
                                pt = pt_t[:].bitcast(BF16)[:, 0:512]
                                for hh in range(2):
                                    for s in range(8):
                                        nc.tensor.matmul(
                                            pt[64 * hh:64 * hh + 64,
                                               64 * s:64 * s + 64],
                                            pair[hh][:, 64 * s:64 * s + 64],
                                            idt[:], is_transpose=True,
                                            start=(s == 0),
                                            stop=(hh == 1 and s == 7),
                                            skip_group_check=True)
                                nc.vector.tensor_copy(ot_tile[:, hp, :], pt[:])
                            bg.insert(min(1, len(bg)),
                                      (("tr", None), transpose_emit))
                    pend.append(norm_emit)

                # flush the pipeline at the j boundary so the last head pair's
                # normalize + transpose are queued before proj tasks
                while pend:
                    pend.pop(0)()
                for t in range(4):
                    for nh in range(2):
                        pending_proj.append((("proj", si),
                                             proj_tile(j, t, nh, ot_tile)))

            while pend:
                pend.pop(0)()
            drain_bg(len(bg))
            for _, task in pending_proj:
                task()

    nc.compile()
    return nc


def _host_prep(x, Wq, Wk, Wv, Wp):
    """Per-core input maps."""
    tri = (np.arange(128)[None, :] >= np.arange(128)[:, None]).astype(np.float32)
    tri2 = np.concatenate([tri, tri], axis=1).astype(f8np)
    ident = np.eye(64, dtype=np.float32).astype(bfnp)
    WpT = np.ascontiguousarray(Wp.T) * (1.0 / SV)  # [E(hd), E]

    def wslice(W, h0, scale, dt=f8np):  # [H,E,D] -> [E, 8*64]
        w = W[h0:h0 + HL].transpose(1, 0, 2).reshape(E, HL * D) * scale
        return np.ascontiguousarray(w).astype(dt)

    in_maps = []
    for c in range(8):
        b, hh = c // 2, c % 2
        h0 = hh * HL
        in_maps.append({
            "xT8": np.ascontiguousarray(x[b].T).astype(f8np),
            "wq8": wslice(Wq, h0, SW),
            "wk8": wslice(Wk, h0, SW),
            "wv8": wslice(Wv, h0, SV),
            "wpT": np.ascontiguousarray(
                WpT[h0 * D:(h0 + HL) * D, :]).astype(bfnp),
            "tri2": tri2,
            "tri2b": tri2.astype(np.float32).astype(bfnp),
            "ident": ident,
            "xbT": np.ascontiguousarray(x[b].T[:, 0:512]).astype(bfnp),
            "wqb": wslice(Wq, h0, SW, bfnp),
            "wkb": wslice(Wk, h0, SW, bfnp),
            "wvb": wslice(Wv, h0, SV, bfnp),
        })
    return in_maps


def kernel(x, Wq, Wk, Wv, Wp, bp):
    x = np.asarray(x, dtype=np.float32)
    Wq = np.asarray(Wq, dtype=np.float32)
    Wk = np.asarray(Wk, dtype=np.float32)
    Wv = np.asarray(Wv, dtype=np.float32)
    Wp = np.asarray(Wp, dtype=np.float32)
    bp = np.asarray(bp, dtype=np.float32)

    if "nc" not in _CACHE:
        _CACHE["nc"] = _build()
    nc = _CACHE["nc"]

    in_maps = _host_prep(x, Wq, Wk, Wv, Wp)
    res = run_bass_kernel_spmd(nc, in_maps, list(range(8)))
    parts = [np.asarray(res.results[c]["out"], dtype=np.float32) for c in range(8)]
    out = np.stack([parts[2 * b] + parts[2 * b + 1] for b in range(B)], axis=0)
    return (out + bp[None, None, :]).astype(np.float32)


# revision 111
# speedup vs baseline: 1.3368x; 1.0002x over previous
"""Multi-head causal attention (B=4,T=2048,E=1024,H=16,D=64) on 8 TRN2 cores.

Sharding: core c -> batch b=c//2, heads h0=(c%2)*8 .. h0+8. Each core computes
its 8 heads' attention and a partial output projection (row-split Wp); host
sums the two partials per batch (+bias).

Per-core kernel (tq tiles processed in order 1,2,3,0):
  - Q/K/V projections in fp8e4 DoubleRow (2 k-tiles of 128 per instr, 0.5
    cyc/col), M=64 outputs at PSUM base 0. Host pre-scales Wq,Wk by 64 and
    Wv by 16 (powers of 2, folded back via exp scale 2^-15 / Wp scale).
  - scores: bf16 kT.T @ qT per 128-row tk chunk; chunk PAIRS share one
    [128,1024] PSUM tile (2 banks) so exp runs as one ACT op per pair.
    Diagonal chunks compute their dead region too so exp never reads
    stale PSUM (race-free; CoreSim conflict-checker clean).
  - exp -> fp8e4 "et" tiles [128, 2, N]; causal triangle masked by DVE
    multiplies; fully-dead subtiles skip the second DR k-tile instead of
    being zeroed.
  - attnV: o-layout fp8 DoubleRow o[tq64, d64] += et.T @ v over chunk
    pairs; softmax Z accumulates as single-tile matmul columns in the
    SAME po bank at partitions 64:127 (base-64 is legal for non-DR).
    Normalize is per-partition: DVE reciprocal [64,8] + broadcast mult.
  - o -> oT via PE transposes (identity matmul) packed 2 heads/bank,
    then bf16 output projection, DVE copy, DMA out.
  - j=0 (first 512 tokens) runs attention in bf16 (v, q/k for tokens
    0:128 recomputed from bf16 x/W): short-support softmax rows cannot
    average away fp8 quantization noise. Processed last so its extra
    DMAs/projections never gate the critical path.
  - Emission is software-pipelined: attnV/Z trail scores by one pair and
    background work (projections, transposes, output tiles) drains from
    a tagged queue with just-in-time forced ordering.
"""
import sys
import numpy as np

sys.path.insert(0, "/opt/trn_rl_repo")

import ml_dtypes
import concourse.bass as bass
import concourse.bacc as bacc
import concourse.mybir as mybir
from concourse import tile
from concourse.bass_utils import run_bass_kernel_spmd

B, T, E, H, D = 4, 2048, 1024, 16, 64
HL = H // 2          # 8 local heads per core
NJ = T // 512        # 4 tq tiles
NPE = E // 256       # 4 E-chunk-pairs
NPAIR = T // 256     # 8 tk chunk pairs
BF16 = mybir.dt.bfloat16
F32 = mybir.dt.float32
FP8 = mybir.dt.float8e4
DR = mybir.MatmulPerfMode.DoubleRow
Exp = mybir.ActivationFunctionType.Exp
f8np = ml_dtypes.float8_e4m3
bfnp = ml_dtypes.bfloat16

SW = 64.0   # Wq/Wk host prescale (exp scale folds 1/SW^2)
SV = 16.0   # Wv host prescale (Wp folds 1/SV)
EXP_SCALE = 0.125 / (SW * SW)  # 2^-15 exactly

_CACHE = {}


def _build():
    nc = bacc.Bacc("TRN2", target_bir_lowering=False)
    xT8 = nc.declare_dram_parameter("xT8", [E, T], FP8, isOutput=False)
    wq8 = nc.declare_dram_parameter("wq8", [E, HL * D], FP8, isOutput=False)
    wk8 = nc.declare_dram_parameter("wk8", [E, HL * D], FP8, isOutput=False)
    wv8 = nc.declare_dram_parameter("wv8", [E, HL * D], FP8, isOutput=False)
    wpT = nc.declare_dram_parameter("wpT", [HL * D, E], BF16, isOutput=False)
    tri2 = nc.declare_dram_parameter("tri2", [128, 256], FP8, isOutput=False)
    tri2b = nc.declare_dram_parameter("tri2b", [128, 256], BF16, isOutput=False)
    xbT = nc.declare_dram_parameter("xbT", [E, 512], BF16, isOutput=False)
    wqb = nc.declare_dram_parameter("wqb", [E, HL * D], BF16, isOutput=False)
    wkb = nc.declare_dram_parameter("wkb", [E, HL * D], BF16, isOutput=False)
    wvb = nc.declare_dram_parameter("wvb", [E, HL * D], BF16, isOutput=False)
    ident = nc.declare_dram_parameter("ident", [64, 64], BF16, isOutput=False)
    out = nc.declare_dram_parameter("out", [T, E], F32, isOutput=True)

    with tile.TileContext(nc) as tc:
        with (
            tc.tile_pool(name="pp", bufs=1) as pp,
            tc.tile_pool(name="etp", bufs=14) as etp,
            tc.tile_pool(name="osb", bufs=10) as osb,
            tc.tile_pool(name="rzp", bufs=8) as rzp,
            tc.tile_pool(name="otp", bufs=3) as otp,
            tc.tile_pool(name="obp", bufs=6) as obp,
            tc.tile_pool(name="sT", bufs=2, space=bass.MemorySpace.PSUM) as ps_sT,
            tc.tile_pool(name="po", bufs=2, space=bass.MemorySpace.PSUM) as ps_po,
            tc.tile_pool(name="mm", bufs=2, space=bass.MemorySpace.PSUM) as ps_mm,
        ):
            # ---- persistent SBUF tiles
            x8a = pp.tile([128, NPE, 2, T], FP8, tag="x8", name="x8a")
            wq8a = pp.tile([128, NPE, 2, 512], FP8, tag="wq", name="wq8a")
            wk8a = pp.tile([128, NPE, 2, 512], FP8, tag="wk", name="wk8a")
            wv8a = pp.tile([128, NPE, 2, 512], FP8, tag="wv", name="wv8a")
            x8t = [x8a[:, p] for p in range(NPE)]
            wq8t = [wq8a[:, p] for p in range(NPE)]
            wk8t = [wk8a[:, p] for p in range(NPE)]
            wv8t = [wv8a[:, p] for p in range(NPE)]
            wpa = pp.tile([128, 4, E], BF16, tag="wp", name="wpa")
            wpt = [wpa[:, p] for p in range(4)]
            trit = pp.tile([128, 2, 128], FP8, tag="tri", name="trit")
            tritb = pp.tile([128, 2, 128], BF16, tag="trib", name="tritb")
            idt = pp.tile([64, 64], BF16, tag="id", name="idt")
            qTt = [[pp.tile([128, 512], BF16, tag=f"q{p}_{j}", name=f"q{p}_{j}")
                    for j in range(NJ)] for p in range(4)]
            kTt = [[pp.tile([128, 512], BF16, tag=f"k{p}_{j}", name=f"k{p}_{j}")
                    for j in range(NJ)] for p in range(4)]
            vp = [pp.tile([128, 2, HL, 65], FP8, tag=f"v{q}", name=f"v{q}")
                  for q in range(NPAIR)]
            # bf16 v for chunks 0..3: j=0 attention runs in bf16 (short-support
            # softmax rows can't average away fp8 quantization noise)
            vb = [pp.tile([128, 2, HL, 65], BF16, tag=f"vb{q}", name=f"vb{q}")
                  for q in range(2)]
            # bf16 x/W and q/k for the first 128 tokens (block-0 scores)
            xba = pp.tile([128, NPE, 2, 512], BF16, tag="xba", name="xba")
            wqba = pp.tile([128, NPE, 2, 512], BF16, tag="wqa", name="wqba")
            wkba = pp.tile([128, NPE, 2, 512], BF16, tag="wka", name="wkba")
            wvba = pp.tile([128, NPE, 2, 512], BF16, tag="wva", name="wvba")
            xbt = [xba[:, p] for p in range(NPE)]
            wqbt = [wqba[:, p] for p in range(NPE)]
            wkbt = [wkba[:, p] for p in range(NPE)]
            wvbt = [wvba[:, p] for p in range(NPE)]
            qbt = [pp.tile([128, 128], BF16, tag=f"qb{p}", name=f"qb{p}")
                   for p in range(4)]
            kbt = [pp.tile([128, 128], BF16, tag=f"kb{p}", name=f"kb{p}")
                   for p in range(4)]

            # ---- DMA in. j tiles are processed in order [1,2,3,0], so the
            # critical path is x cols [0:1024] + wv (SP) and wq/wk (ACT).
            # The bf16 sidecar tiles (j=0 accuracy path) ride at the SP tail.
            def cdma(queue, dst, src, cols=None):
                # whole class in one DMA: src rows (pe t p) -> [p, pe, t, n]
                s = src if cols is None else src[:, cols[0]:cols[1]]
                queue.dma_start(dst, s.rearrange("(pe t p) n -> p pe t n",
                                                 pe=NPE, t=2))

            def tdma(queue, dst, src, cols=None):
                # both t-halves in one DMA: src rows (t p) -> dst [p, t, n]
                s = src if cols is None else src[:, cols[0]:cols[1]]
                queue.dma_start(dst, s.rearrange("(t p) n -> p t n", t=2))
            cdma(nc.sync, x8a[:, :, :, 0:512], xT8, (0, 512))
            cdma(nc.sync, wk8a[:], wk8)
            cdma(nc.sync, wq8a[:], wq8)
            cdma(nc.sync, wv8a[:], wv8)
            cdma(nc.sync, x8a[:, :, :, 512:1024], xT8, (512, 1024))
            nc.sync.dma_start(trit[:], tri2[:].rearrange("p (a n) -> p a n", a=2))
            nc.sync.dma_start(idt[:], ident[:])
            for jj in range(2, NJ):
                cdma(nc.sync, x8a[:, :, :, 512 * jj:512 * jj + 512], xT8,
                     (512 * jj, 512 * jj + 512))
            nc.sync.dma_start(
                wpa[:], wpT[:].rearrange("(q p) n -> p q n", q=4))
            nc.sync.dma_start(tritb[:],
                              tri2b[:].rearrange("p (a n) -> p a n", a=2))
            cdma(nc.sync, xba[:], xbT)
            cdma(nc.sync, wvba[:], wvb)
            cdma(nc.sync, wqba[:], wqb)
            cdma(nc.sync, wkba[:], wkb)
            for q in range(NPAIR):
                nc.gpsimd.memset(vp[q][:, :, :, 64:65], 1.0)
            for q in range(2):
                nc.gpsimd.memset(vb[q][:, :, :, 64:65], 1.0)

            # ---- background task machinery (qkv groups, proj tiles)
            def qk_group(kind, h, j, pool=None):
                def emit():
                    wt = wq8t if kind == "q" else wk8t
                    dst = qTt if kind == "q" else kTt
                    if pool is None:
                        m = ps_mm.tile([128, 512], F32, tag="mm", name="mmq")
                    else:
                        m = pool.tile([128, 1024], F32, tag="sT",
                                      name="mmq")[:, 0:512]
                    for p in range(NPE):
                        nc.tensor.matmul(
                            m[0:64, :], wt[p][:, :, 64 * h:64 * h + 64],
                            x8t[p][:, :, 512 * j:512 * j + 512],
                            start=(p == 0), stop=(p == NPE - 1), perf_mode=DR)
                    nc.vector.tensor_copy(
                        dst[h // 2][j][64 * (h % 2):64 * (h % 2) + 64, :],
                        m[0:64, :])
                return emit

            def qkb_group(pe_hp):  # bf16 q/k for tokens 0..127, head pair
                def emit():
                    for wt, dst in ((wqbt, qbt), (wkbt, kbt)):
                        m = ps_mm.tile([128, 512], F32, tag="mm", name="mmb")
                        for p in range(NPE):
                            for t in range(2):
                                nc.tensor.matmul(
                                    m[0:128, 0:128],
                                    wt[p][:, t, 128 * pe_hp:128 * pe_hp + 128],
                                    xbt[p][:, t, 0:128],
                                    start=(p == 0 and t == 0),
                                    stop=(p == NPE - 1 and t == 1))
                        nc.vector.tensor_copy(dst[pe_hp][:], m[0:128, 0:128])
                return emit

            def v_group(c, g, pool=None):  # chunk c, t-64 sub g
                def emit():
                    if pool is None:
                        m = ps_mm.tile([128, 512], F32, tag="mm", name="mmv")
                    else:
                        m = pool.tile([128, 1024], F32, tag="sT",
                                      name="mmv")[:, 0:512]
                    t0 = 128 * c + 64 * g
                    for p in range(NPE):
                        nc.tensor.matmul(
                            m[0:64, :], x8t[p][:, :, t0:t0 + 64], wv8t[p][:],
                            start=(p == 0), stop=(p == NPE - 1), perf_mode=DR)
                    nc.vector.tensor_copy(
                        vp[c // 2][64 * g:64 * g + 64, c % 2, :, 0:64],
                        m[0:64, :].rearrange("p (h d) -> p h d", d=64))
                return emit

            def vb_group(c, g):  # true bf16 v for j=0 (bf16 x and Wv)
                def emit():
                    m = ps_mm.tile([128, 512], F32, tag="mm", name="mvb")
                    t0 = 128 * c + 64 * g
                    for p in range(NPE):
                        for t in range(2):
                            nc.tensor.matmul(
                                m[0:64, :],
                                xbt[p][:, t, t0:t0 + 64],
                                wvbt[p][:, t, :],
                                start=(p == 0 and t == 0),
                                stop=(p == NPE - 1 and t == 1))
                    nc.vector.tensor_copy(
                        vb[c // 2][64 * g:64 * g + 64, c % 2, :, 0:64],
                        m[0:64, :].rearrange("p (h d) -> p h d", d=64))
                return emit

            def proj_tile(j, t, nh, ot_tile):
                def emit():
                    m = ps_mm.tile([128, 512], F32, tag="mm", name="mmp")
                    for p in range(4):
                        nc.tensor.matmul(
                            m[:], ot_tile[:, p, 128 * t:128 * t + 128],
                            wpt[p][:, 512 * nh:512 * nh + 512],
                            start=(p == 0), stop=(p == 3))
                    ob = obp.tile([128, 512], F32, tag="ob", name="ob")
                    nc.vector.tensor_copy(ob[:], m[:])
                    nc.sync.dma_start(
                        out[512 * j + 128 * t:512 * j + 128 * t + 128,
                            512 * nh:512 * nh + 512], ob[:])
                return emit

            bg = []  # queue of (tag, emitter); tags order forced at stream starts

            def drain_bg(n=1):
                for _ in range(min(n, len(bg))):
                    bg.pop(0)[1]()

            def force_bg(pred):
                """Emit from the front until no queued task satisfies pred."""
                while any(pred(t) for t, _ in bg):
                    bg.pop(0)[1]()

            # j tiles processed [1, 2, 3, 0]: the bf16 j=0 accuracy path runs
            # last so its DMAs/projections never gate the critical path.
            JORDER = (1, 2, 3, 0)
            # prologue: v pairs 0..1 (chunks 0..3); q tile 1, k tiles 0..1
            # for h=0. k projections are scheduled by tk-tile index: stream
            # (j, h) consumes k tiles 0..j.
            # interleave so the first stream's scores are unblocked ASAP
            qk_group("k", 0, 0)()
            v_group(0, 0, ps_sT)()
            v_group(0, 1)()
            qk_group("k", 0, 1, ps_sT)()
            v_group(1, 0)()
            v_group(1, 1, ps_sT)()
            qk_group("q", 0, 1)()
            for c in (2, 3):
                for g in range(2):
                    bg.append((("vq", 1), v_group(c, g)))
            for q in range(2, 4):
                for g in range(2):
                    bg.append((("vq", q), v_group(2 * q, g)))
                    bg.append((("vq", q), v_group(2 * q + 1, g)))
            for h in range(1, HL):
                bg.append((("q", 1, h), qk_group("q", h, 1)))
                bg.append((("k", 0, h), qk_group("k", h, 0)))
                bg.append((("k", 1, h), qk_group("k", h, 1)))
            # bf16 sidecar tasks wait on tail-end DMAs; keep them out of the
            # main drain queue until their inputs have surely landed
            bg_late = []
            for hp in range(4):
                bg_late.append((("qkb", 2 * hp), qkb_group(hp)))
            for c in range(4):
                for g in range(2):
                    bg_late.append((("vb", c // 2), vb_group(c, g)))

            ot_tiles = {}
            pend = []       # one-pair-lagged PE emissions (attnV/Z, normalize)
            o_sb_pair = {}  # normalized o for the in-flight head pair

            pending_proj = []
            for si, j in enumerate(JORDER):
                if si == 1:  # release the bf16 sidecar work mid-flight
                    bg.extend(bg_late)
                    bg_late = []
                # enqueue the next sequence step's inputs
                if si + 1 < NJ:
                    nj = JORDER[si + 1]
                    if nj != 0:
                        for q in range(2 * nj, 2 * nj + 2):
                            for g in range(2):
                                bg.append((("vq", q), v_group(2 * q, g)))
                                bg.append((("vq", q), v_group(2 * q + 1, g)))
                    for h in range(HL):
                        bg.append((("q", nj, h), qk_group("q", h, nj)))
                        if nj != 0:  # k tile 0 was produced in the prologue
                            bg.append((("k", nj, h), qk_group("k", h, nj)))
                # weave last step's proj tiles into the fresh queue (avoids a
                # PE-only burst that starves ACT at the step boundary); all
                # transposes must go first (proj reads their output)
                if pending_proj:
                    force_bg(lambda t: t[0] == "tr")
                for i, task in enumerate(pending_proj):
                    bg.insert(min(5 * i + 2, len(bg)), task)
                pending_proj = []
                # proj from two sequence steps back must be out before its
                # ot slot is reused
                force_bg(lambda t: t[0] == "proj" and t[1] <= si - 2)
                ot_tile = otp.tile([128, 4, 512], BF16, tag="ot", name=f"ot{j}")
                ot_tiles[j] = ot_tile

                for h in range(HL):
                    hp, r0 = h // 2, 64 * (h % 2)
                    zc = 8 * (8 * j + h)
                    # everything this stream reads must already be emitted
                    force_bg(lambda t: (t[0] == "q" and t[1] == j and t[2] <= h) or
                             (t[0] == "k" and t[1] <= j and t[2] <= h) or
                             (j == 0 and t[0] == "qkb" and t[1] <= h))
                    po_t = ps_po.tile([128, 512], F32, tag="po", name="po")
                    po = po_t[0:64, :]
                    nq = 2 * j + 2
                    for q in range(nq):
                        m0, m2 = (q == 2 * j), (q == 2 * j + 1)
                        force_bg(lambda t: (t[0] == "vq" and t[1] < q) or
                                 (j == 0 and t[0] == "vb" and t[1] < q))
                        sT = ps_sT.tile([128, 1024], F32, tag="sT", name="sT")
                        if j == 0:
                            et = etp.tile([128, 1024], BF16, tag="etb",
                                          name="etb", bufs=4)
                            trm = tritb
                        else:
                            et = etp.tile([128, 1024], FP8, tag="et", name="et")
                            trm = trit
                        if not (m0 or m2):
                            # off-diag pair: both chunks full [0:512]
                            for t in range(2):
                                c = 2 * q + t
                                nc.tensor.matmul(
                                    sT[:, 512 * t:512 * t + 512],
                                    kTt[hp][c // 4][r0:r0 + 64,
                                                    128 * (c % 4):128 * (c % 4) + 128],
                                    qTt[hp][j][r0:r0 + 64, :],
                                    start=True, stop=True, skip_group_check=True)
                            nc.scalar.activation(et[:], sT[:], Exp, scale=EXP_SCALE)
                            stride, width = 512, 512
                        elif m0:
                            # chunks 4j (full), 4j+1 (cols 128:512)
                            c = 4 * j
                            if j == 0:
                                # block-0 scores from bf16-accurate q/k
                                nc.tensor.matmul(
                                    sT[:, 0:128],
                                    kbt[hp][r0:r0 + 64, :],
                                    qbt[hp][r0:r0 + 64, :],
                                    start=True, stop=False,
                                    skip_group_check=True)
                                nc.tensor.matmul(
                                    sT[:, 128:512],
                                    kTt[hp][j][r0:r0 + 64, 0:128],
                                    qTt[hp][j][r0:r0 + 64, 128:512],
                                    start=False, stop=True,
                                    skip_group_check=True)
                            else:
                                nc.tensor.matmul(
                                    sT[:, 0:512],
                                    kTt[hp][j][r0:r0 + 64, 0:128],
                                    qTt[hp][j][r0:r0 + 64, :],
                                    start=True, stop=True,
                                    skip_group_check=True)
                            # cover [512:640] too so exp never reads stale
                            # bytes (those weights get memset to 0 after)
                            nc.tensor.matmul(
                                sT[:, 512:1024],
                                kTt[hp][j][r0:r0 + 64, 128:256],
                                qTt[hp][j][r0:r0 + 64, :],
                                start=True, stop=True, skip_group_check=True)
                            nc.scalar.activation(et[:], sT[:], Exp, scale=EXP_SCALE)
                            et3 = et[:].rearrange("p (a n) -> p a n", n=128)
                            nc.vector.tensor_mul(et3[:, 0:6:5, :], et3[:, 0:6:5, :],
                                                 trm[:])
                            stride, width = 512, 512
                        else:
                            # m2: chunks 4j+2 (cols 256:512 -> [0:256]),
                            #     4j+3 (cols 384:512 -> [384:512])
                            nc.tensor.matmul(
                                sT[:, 0:256],
                                kTt[hp][j][r0:r0 + 64, 256:384],
                                qTt[hp][j][r0:r0 + 64, 256:512],
                                start=True, stop=True, skip_group_check=True)
                            nc.tensor.matmul(
                                sT[:, 256:512],
                                kTt[hp][j][r0:r0 + 64, 384:512],
                                qTt[hp][j][r0:r0 + 64, 256:512],
                                start=False, stop=True, skip_group_check=True)
                            nc.scalar.activation(et[:, 0:512], sT[:, 0:512],
                                                 Exp, scale=EXP_SCALE)
                            et3 = et[:].rearrange("p (a n) -> p a n", n=128)
                            nc.vector.tensor_mul(et3[:, 0:4:3, :], et3[:, 0:4:3, :],
                                                 trm[:])
                            stride, width = 256, 256
                        etv = et[:, 0:2 * stride].rearrange(
                            "p (a n) -> p a n", a=2)
                        s_lo = 4 if m2 else 0
                        base = 256 if m2 else 0

                        def attn_emit(et=et, etv=etv, q=q, h=h, po=po,
                                      po_t=po_t, zc=zc,
                                      s_lo=s_lo, base=base, m0=m0, m2=m2, j=j):
                            # exactly ONE start=True per bank-use: start=True
                            # flags the whole 2KB bank pending-zero; every
                            # other group's first touch consumes its flag.
                            if j == 0:
                                # bf16 non-DR path (no fp8 noise on the short-
                                # support rows). (tile, subtiles, flat offset fn)
                                if m0:
                                    work = [(0, range(0, 8), lambda s: 64 * s),
                                            (1, range(2, 8),
                                             lambda s: 512 + 64 * s)]
                                else:
                                    work = [(0, range(4, 8),
                                             lambda s: 64 * s - 256),
                                            (1, range(6, 8), lambda s: 64 * s)]
                                stop_at = {0: (0, 0), 1: (0, 0), 2: (0, 1),
                                           3: (0, 1), 4: (1, 0), 5: (1, 0),
                                           6: (1, 1), 7: (1, 1)}
                                for tt, srange, off in work:
                                    for s in srange:
                                        fo = off(s)
                                        stop = stop_at[s] == (q, tt)
                                        st = (q == 0 and tt == 0 and s == 0)
                                        nc.tensor.matmul(
                                            po[:, 64 * s:64 * s + 64],
                                            et[:, fo:fo + 64],
                                            vb[q][:, tt, h, 0:64],
                                            start=st, stop=stop,
                                            skip_group_check=True)
                                        nc.tensor.matmul(
                                            po_t[64:128, s:s + 1],
                                            et[:, fo:fo + 64],
                                            vb[q][:, tt, h, 64:65],
                                            start=st, stop=stop,
                                            skip_group_check=True)
                                return
                            for s in range(s_lo, 8):
                                cc = 64 * s - base
                                stop = (s < 4 and m0) or (s >= 4 and m2)
                                # tile1's dead block is never read: subtiles
                                # under the diagonal use a single-tile matmul
                                single = (m0 and s < 2) or (m2 and s < 6)
                                if single:
                                    lhs_o = etv[:, 0, cc:cc + 64]
                                    rhs_o = vp[q][:, 0, h, 0:64]
                                    pm = None
                                else:
                                    lhs_o = etv[:, :, cc:cc + 64]
                                    rhs_o = vp[q][:, :, h, 0:64]
                                    pm = DR
                                nc.tensor.matmul(
                                    po[:, 64 * s:64 * s + 64],
                                    lhs_o, rhs_o,
                                    start=(q == 0 and s == 0), stop=stop,
                                    perf_mode=pm, skip_group_check=True)
                                # Z columns live at partitions 64:128 of the
                                # same po bank (single-tile, non-DR: base-64)
                                nc.tensor.matmul(
                                    po_t[64:128, s:s + 1],
                                    etv[:, 0, cc:cc + 64],
                                    vp[q][:, 0, h, 64:65],
                                    start=(q == 0 and s == 0), stop=(stop and single),
                                    skip_group_check=True)
                                if not single:
                                    nc.tensor.matmul(
                                        po_t[64:128, s:s + 1],
                                        etv[:, 1, cc:cc + 64],
                                        vp[q][:, 1, h, 64:65],
                                        start=False, stop=stop,
                                        skip_group_check=True)
                        # one-pair software pipeline: previous pair's attnV/Z
                        # runs while this pair's exp/masks are in flight
                        while pend:
                            pend.pop(0)()
                        pend.append(attn_emit)
                        drain_bg(3 if h == HL - 1 else (2 if len(bg) > 40 else 1))

                    force_bg(lambda t: (t[0] == "vq" and t[1] <= 2 * j + 1) or
                             (j == 0 and t[0] == "vb"))

                    def norm_emit(po=po, po_t=po_t, h=h, hp=hp,
                                  ot_tile=ot_tile):
                        rz = rzp.tile([64, 8], F32, tag="rz", name="rz")
                        nc.vector.reciprocal(rz[:], po_t[64:128, 0:8])
                        o_sb = osb.tile([64, 512], BF16, tag="os", name="os")
                        nc.vector.tensor_tensor(
                            o_sb[:].rearrange("p (e s) -> p e s", e=8),
                            po[:].rearrange("p (e s) -> p e s", e=8),
                            rz[:].unsqueeze(2).broadcast_to([64, 8, 64]),
                            mybir.AluOpType.mult)
                        o_sb_pair[h % 2] = o_sb
                        if h % 2 == 1:
                            def transpose_emit(hp=hp, ot_tile=ot_tile,
                                               pair=dict(o_sb_pair)):
                                pt_t = ps_po.tile([128, 512], F32, tag="po",
                                                  name="pt")
                                pt = pt_t[:].bitcast(BF16)[:, 0:512]
                                for hh in range(2):
                                    for s in range(8):
                                        nc.tensor.matmul(
                                            pt[64 * hh:64 * hh + 64,
                                               64 * s:64 * s + 64],
                                            pair[hh][:, 64 * s:64 * s + 64],
                                            idt[:], is_transpose=True,
                                            start=(s == 0),
                                            stop=(hh == 1 and s == 7),
                                            skip_group_check=True)
                                nc.vector.tensor_copy(ot_tile[:, hp, :], pt[:])
                            bg.insert(min(1, len(bg)),
                                      (("tr", None), transpose_emit))
                    pend.append(norm_emit)

                # flush the pipeline at the j boundary so the last head pair's
                # normalize + transpose are queued before proj tasks
                while pend:
                    pend.pop(0)()
                for t in range(4):
                    for nh in range(2):
                        pending_proj.append((("proj", si),
                                             proj_tile(j, t, nh, ot_tile)))

            while pend:
                pend.pop(0)()
            drain_bg(len(bg))
            for _, task in pending_proj:
                task()

    nc.compile()
    return nc


def _host_prep(x, Wq, Wk, Wv, Wp):
    """Per-core input maps."""
    tri = (np.arange(128)[None, :] >= np.arange(128)[:, None]).astype(np.float32)
    tri2 = np.concatenate([tri, tri], axis=1).astype(f8np)
    ident = np.eye(64, dtype=np.float32).astype(bfnp)
    WpT = np.ascontiguousarray(Wp.T) * (1.0 / SV)  # [E(hd), E]

    def wslice(W, h0, scale, dt=f8np):  # [H,E,D] -> [E, 8*64]
        w = W[h0:h0 + HL].transpose(1, 0, 2).reshape(E, HL * D) * scale
        return np.ascontiguousarray(w).astype(dt)

    in_maps = []
    for c in range(8):
        b, hh = c // 2, c % 2
        h0 = hh * HL
        in_maps.append({
            "xT8": np.ascontiguousarray(x[b].T).astype(f8np),
            "wq8": wslice(Wq, h0, SW),
            "wk8": wslice(Wk, h0, SW),
            "wv8": wslice(Wv, h0, SV),
            "wpT": np.ascontiguousarray(
                WpT[h0 * D:(h0 + HL) * D, :]).astype(bfnp),
            "tri2": tri2,
            "tri2b": tri2.astype(np.float32).astype(bfnp),
            "ident": ident,
            "xbT": np.ascontiguousarray(x[b].T[:, 0:512]).astype(bfnp),
            "wqb": wslice(Wq, h0, SW, bfnp),
            "wkb": wslice(Wk, h0, SW, bfnp),
            "wvb": wslice(Wv, h0, SV, bfnp),
        })
    return in_maps


def kernel(x, Wq, Wk, Wv, Wp, bp):
    x = np.asarray(x, dtype=np.float32)
    Wq = np.asarray(Wq, dtype=np.float32)
    Wk = np.asarray(Wk, dtype=np.float32)
    Wv = np.asarray(Wv, dtype=np.float32)
    Wp = np.asarray(Wp, dtype=np.float32)
    bp = np.asarray(bp, dtype=np.float32)

    if "nc" not in _CACHE:
        _CACHE["nc"] = _build()
    nc = _CACHE["nc"]

    in_maps = _host_prep(x, Wq, Wk, Wv, Wp)
    res = run_bass_kernel_spmd(nc, in_maps, list(range(8)))
    parts = [np.asarray(res.results[c]["out"], dtype=np.float32) for c in range(8)]
    out = np.stack([parts[2 * b] + parts[2 * b + 1] for b in range(B)], axis=0)
    return (out + bp[None, None, :]).astype(np.float32)


# revision 112
# speedup vs baseline: 1.3416x; 1.0036x over previous
"""Multi-head causal attention (B=4,T=2048,E=1024,H=16,D=64) on 8 TRN2 cores.

Sharding: core c -> batch b=c//2, heads h0=(c%2)*8 .. h0+8. Each core computes
its 8 heads' attention and a partial output projection (row-split Wp); host
sums the two partials per batch (+bias).

Per-core kernel (tq tiles processed in order 1,2,3,0):
  - Q/K/V projections in fp8e4 DoubleRow (2 k-tiles of 128 per instr, 0.5
    cyc/col), M=64 outputs at PSUM base 0. Host pre-scales Wq,Wk by 64 and
    Wv by 16 (powers of 2, folded back via exp scale 2^-15 / Wp scale).
  - scores: bf16 kT.T @ qT per 128-row tk chunk; chunk PAIRS share one
    [128,1024] PSUM tile (2 banks) so exp runs as one ACT op per pair.
    Diagonal chunks compute their dead region too so exp never reads
    stale PSUM (race-free; CoreSim conflict-checker clean).
  - exp -> fp8e4 "et" tiles [128, 2, N]; causal triangle masked by DVE
    multiplies; fully-dead subtiles skip the second DR k-tile instead of
    being zeroed.
  - attnV: o-layout fp8 DoubleRow o[tq64, d64] += et.T @ v over chunk
    pairs; softmax Z accumulates as single-tile matmul columns in the
    SAME po bank at partitions 64:127 (base-64 is legal for non-DR).
    Normalize is per-partition: DVE reciprocal [64,8] + broadcast mult.
  - o -> oT via PE transposes (identity matmul) packed 2 heads/bank,
    then bf16 output projection, DVE copy, DMA out.
  - j=0 (first 512 tokens) runs attention in bf16 (v, q/k for tokens
    0:128 recomputed from bf16 x/W): short-support softmax rows cannot
    average away fp8 quantization noise. Processed last so its extra
    DMAs/projections never gate the critical path.
  - Emission is software-pipelined: attnV/Z trail scores by one pair and
    background work (projections, transposes, output tiles) drains from
    a tagged queue with just-in-time forced ordering.
"""
import sys
import numpy as np

sys.path.insert(0, "/opt/trn_rl_repo")

import ml_dtypes
import concourse.bass as bass
import concourse.bacc as bacc
import concourse.mybir as mybir
from concourse import tile
from concourse.bass_utils import run_bass_kernel_spmd

B, T, E, H, D = 4, 2048, 1024, 16, 64
HL = H // 2          # 8 local heads per core
NJ = T // 512        # 4 tq tiles
NPE = E // 256       # 4 E-chunk-pairs
NPAIR = T // 256     # 8 tk chunk pairs
BF16 = mybir.dt.bfloat16
F32 = mybir.dt.float32
FP8 = mybir.dt.float8e4
DR = mybir.MatmulPerfMode.DoubleRow
Exp = mybir.ActivationFunctionType.Exp
f8np = ml_dtypes.float8_e4m3
bfnp = ml_dtypes.bfloat16

SW = 64.0   # Wq/Wk host prescale (exp scale folds 1/SW^2)
SV = 16.0   # Wv host prescale (Wp folds 1/SV)
EXP_SCALE = 0.125 / (SW * SW)  # 2^-15 exactly

_CACHE = {}


def _build():
    nc = bacc.Bacc("TRN2", target_bir_lowering=False)
    xT8 = nc.declare_dram_parameter("xT8", [E, T], FP8, isOutput=False)
    wq8 = nc.declare_dram_parameter("wq8", [E, HL * D], FP8, isOutput=False)
    wk8 = nc.declare_dram_parameter("wk8", [E, HL * D], FP8, isOutput=False)
    wv8 = nc.declare_dram_parameter("wv8", [E, HL * D], FP8, isOutput=False)
    wpT = nc.declare_dram_parameter("wpT", [HL * D, E], BF16, isOutput=False)
    tri2 = nc.declare_dram_parameter("tri2", [128, 256], FP8, isOutput=False)
    tri2b = nc.declare_dram_parameter("tri2b", [128, 256], BF16, isOutput=False)
    xbT = nc.declare_dram_parameter("xbT", [E, 512], BF16, isOutput=False)
    wqb = nc.declare_dram_parameter("wqb", [E, HL * D], BF16, isOutput=False)
    wkb = nc.declare_dram_parameter("wkb", [E, HL * D], BF16, isOutput=False)
    wvb = nc.declare_dram_parameter("wvb", [E, HL * D], BF16, isOutput=False)
    ident = nc.declare_dram_parameter("ident", [64, 64], BF16, isOutput=False)
    out = nc.declare_dram_parameter("out", [T, E], F32, isOutput=True)

    with tile.TileContext(nc) as tc:
        with (
            tc.tile_pool(name="pp", bufs=1) as pp,
            tc.tile_pool(name="etp", bufs=14) as etp,
            tc.tile_pool(name="osb", bufs=10) as osb,
            tc.tile_pool(name="rzp", bufs=8) as rzp,
            tc.tile_pool(name="otp", bufs=3) as otp,
            tc.tile_pool(name="obp", bufs=6) as obp,
            tc.tile_pool(name="sT", bufs=2, space=bass.MemorySpace.PSUM) as ps_sT,
            tc.tile_pool(name="po", bufs=2, space=bass.MemorySpace.PSUM) as ps_po,
            tc.tile_pool(name="mm", bufs=2, space=bass.MemorySpace.PSUM) as ps_mm,
        ):
            # ---- persistent SBUF tiles
            x8a = pp.tile([128, NPE, 2, T], FP8, tag="x8", name="x8a")
            wq8a = pp.tile([128, NPE, 2, 512], FP8, tag="wq", name="wq8a")
            wk8a = pp.tile([128, NPE, 2, 512], FP8, tag="wk", name="wk8a")
            wv8a = pp.tile([128, NPE, 2, 512], FP8, tag="wv", name="wv8a")
            x8t = [x8a[:, p] for p in range(NPE)]
            wq8t = [wq8a[:, p] for p in range(NPE)]
            wk8t = [wk8a[:, p] for p in range(NPE)]
            wv8t = [wv8a[:, p] for p in range(NPE)]
            wpa = pp.tile([128, 4, E], BF16, tag="wp", name="wpa")
            wpt = [wpa[:, p] for p in range(4)]
            trit = pp.tile([128, 2, 128], FP8, tag="tri", name="trit")
            tritb = pp.tile([128, 2, 128], BF16, tag="trib", name="tritb")
            idt = pp.tile([64, 64], BF16, tag="id", name="idt")
            qTt = [[pp.tile([128, 512], BF16, tag=f"q{p}_{j}", name=f"q{p}_{j}")
                    for j in range(NJ)] for p in range(4)]
            kTt = [[pp.tile([128, 512], BF16, tag=f"k{p}_{j}", name=f"k{p}_{j}")
                    for j in range(NJ)] for p in range(4)]
            vp = [pp.tile([128, 2, HL, 65], FP8, tag=f"v{q}", name=f"v{q}")
                  for q in range(NPAIR)]
            # bf16 v for chunks 0..3: j=0 attention runs in bf16 (short-support
            # softmax rows can't average away fp8 quantization noise)
            vb = [pp.tile([128, 2, HL, 65], BF16, tag=f"vb{q}", name=f"vb{q}")
                  for q in range(2)]
            # bf16 x/W and q/k for the first 128 tokens (block-0 scores)
            xba = pp.tile([128, NPE, 2, 512], BF16, tag="xba", name="xba")
            wqba = pp.tile([128, NPE, 2, 512], BF16, tag="wqa", name="wqba")
            wkba = pp.tile([128, NPE, 2, 512], BF16, tag="wka", name="wkba")
            wvba = pp.tile([128, NPE, 2, 512], BF16, tag="wva", name="wvba")
            xbt = [xba[:, p] for p in range(NPE)]
            wqbt = [wqba[:, p] for p in range(NPE)]
            wkbt = [wkba[:, p] for p in range(NPE)]
            wvbt = [wvba[:, p] for p in range(NPE)]
            qbt = [pp.tile([128, 128], BF16, tag=f"qb{p}", name=f"qb{p}")
                   for p in range(4)]
            kbt = [pp.tile([128, 128], BF16, tag=f"kb{p}", name=f"kb{p}")
                   for p in range(4)]

            # ---- DMA in. j tiles are processed in order [1,2,3,0], so the
            # critical path is x cols [0:1024] + wv (SP) and wq/wk (ACT).
            # The bf16 sidecar tiles (j=0 accuracy path) ride at the SP tail.
            def cdma(queue, dst, src, cols=None):
                # whole class in one DMA: src rows (pe t p) -> [p, pe, t, n]
                s = src if cols is None else src[:, cols[0]:cols[1]]
                queue.dma_start(dst, s.rearrange("(pe t p) n -> p pe t n",
                                                 pe=NPE, t=2))

            def tdma(queue, dst, src, cols=None):
                # both t-halves in one DMA: src rows (t p) -> dst [p, t, n]
                s = src if cols is None else src[:, cols[0]:cols[1]]
                queue.dma_start(dst, s.rearrange("(t p) n -> p t n", t=2))
            cdma(nc.sync, x8a[:, :, :, 0:512], xT8, (0, 512))
            cdma(nc.sync, wk8a[:], wk8)
            cdma(nc.sync, wq8a[:], wq8)
            cdma(nc.sync, x8a[:, :, :, 512:1024], xT8, (512, 1024))
            cdma(nc.sync, wv8a[:], wv8)
            nc.sync.dma_start(trit[:], tri2[:].rearrange("p (a n) -> p a n", a=2))
            nc.sync.dma_start(idt[:], ident[:])
            for jj in range(2, NJ):
                cdma(nc.sync, x8a[:, :, :, 512 * jj:512 * jj + 512], xT8,
                     (512 * jj, 512 * jj + 512))
            nc.sync.dma_start(
                wpa[:], wpT[:].rearrange("(q p) n -> p q n", q=4))
            nc.sync.dma_start(tritb[:],
                              tri2b[:].rearrange("p (a n) -> p a n", a=2))
            cdma(nc.sync, xba[:], xbT)
            cdma(nc.sync, wvba[:], wvb)
            cdma(nc.sync, wqba[:], wqb)
            cdma(nc.sync, wkba[:], wkb)
            for q in range(NPAIR):
                nc.gpsimd.memset(vp[q][:, :, :, 64:65], 1.0)
            for q in range(2):
                nc.gpsimd.memset(vb[q][:, :, :, 64:65], 1.0)

            # ---- background task machinery (qkv groups, proj tiles)
            def qk_group(kind, h, j, pool=None):
                def emit():
                    wt = wq8t if kind == "q" else wk8t
                    dst = qTt if kind == "q" else kTt
                    if pool is None:
                        m = ps_mm.tile([128, 512], F32, tag="mm", name="mmq")
                    else:
                        m = pool.tile([128, 1024], F32, tag="sT",
                                      name="mmq")[:, 0:512]
                    for p in range(NPE):
                        nc.tensor.matmul(
                            m[0:64, :], wt[p][:, :, 64 * h:64 * h + 64],
                            x8t[p][:, :, 512 * j:512 * j + 512],
                            start=(p == 0), stop=(p == NPE - 1), perf_mode=DR)
                    nc.vector.tensor_copy(
                        dst[h // 2][j][64 * (h % 2):64 * (h % 2) + 64, :],
                        m[0:64, :])
                return emit

            def qkb_group(pe_hp):  # bf16 q/k for tokens 0..127, head pair
                def emit():
                    for wt, dst in ((wqbt, qbt), (wkbt, kbt)):
                        m = ps_mm.tile([128, 512], F32, tag="mm", name="mmb")
                        for p in range(NPE):
                            for t in range(2):
                                nc.tensor.matmul(
                                    m[0:128, 0:128],
                                    wt[p][:, t, 128 * pe_hp:128 * pe_hp + 128],
                                    xbt[p][:, t, 0:128],
                                    start=(p == 0 and t == 0),
                                    stop=(p == NPE - 1 and t == 1))
                        nc.vector.tensor_copy(dst[pe_hp][:], m[0:128, 0:128])
                return emit

            def v_group(c, g, pool=None):  # chunk c, t-64 sub g
                def emit():
                    if pool is None:
                        m = ps_mm.tile([128, 512], F32, tag="mm", name="mmv")
                    else:
                        m = pool.tile([128, 1024], F32, tag="sT",
                                      name="mmv")[:, 0:512]
                    t0 = 128 * c + 64 * g
                    for p in range(NPE):
                        nc.tensor.matmul(
                            m[0:64, :], x8t[p][:, :, t0:t0 + 64], wv8t[p][:],
                            start=(p == 0), stop=(p == NPE - 1), perf_mode=DR)
                    nc.vector.tensor_copy(
                        vp[c // 2][64 * g:64 * g + 64, c % 2, :, 0:64],
                        m[0:64, :].rearrange("p (h d) -> p h d", d=64))
                return emit

            def vb_group(c, g):  # true bf16 v for j=0 (bf16 x and Wv)
                def emit():
                    m = ps_mm.tile([128, 512], F32, tag="mm", name="mvb")
                    t0 = 128 * c + 64 * g
                    for p in range(NPE):
                        for t in range(2):
                            nc.tensor.matmul(
                                m[0:64, :],
                                xbt[p][:, t, t0:t0 + 64],
                                wvbt[p][:, t, :],
                                start=(p == 0 and t == 0),
                                stop=(p == NPE - 1 and t == 1))
                    nc.vector.tensor_copy(
                        vb[c // 2][64 * g:64 * g + 64, c % 2, :, 0:64],
                        m[0:64, :].rearrange("p (h d) -> p h d", d=64))
                return emit

            def proj_tile(j, t, nh, ot_tile):
                def emit():
                    m = ps_mm.tile([128, 512], F32, tag="mm", name="mmp")
                    for p in range(4):
                        nc.tensor.matmul(
                            m[:], ot_tile[:, p, 128 * t:128 * t + 128],
                            wpt[p][:, 512 * nh:512 * nh + 512],
                            start=(p == 0), stop=(p == 3))
                    ob = obp.tile([128, 512], F32, tag="ob", name="ob")
                    nc.vector.tensor_copy(ob[:], m[:])
                    nc.sync.dma_start(
                        out[512 * j + 128 * t:512 * j + 128 * t + 128,
                            512 * nh:512 * nh + 512], ob[:])
                return emit

            bg = []  # queue of (tag, emitter); tags order forced at stream starts

            def drain_bg(n=1):
                for _ in range(min(n, len(bg))):
                    bg.pop(0)[1]()

            def force_bg(pred):
                """Emit from the front until no queued task satisfies pred."""
                while any(pred(t) for t, _ in bg):
                    bg.pop(0)[1]()

            # j tiles processed [1, 2, 3, 0]: the bf16 j=0 accuracy path runs
            # last so its DMAs/projections never gate the critical path.
            JORDER = (1, 2, 3, 0)
            # prologue: v pairs 0..1 (chunks 0..3); q tile 1, k tiles 0..1
            # for h=0. k projections are scheduled by tk-tile index: stream
            # (j, h) consumes k tiles 0..j.
            # interleave so the first stream's scores are unblocked ASAP
            qk_group("k", 0, 0)()
            qk_group("k", 0, 1, ps_sT)()
            qk_group("q", 0, 1)()
            v_group(0, 0, ps_sT)()
            v_group(0, 1)()
            v_group(1, 0, ps_sT)()
            v_group(1, 1)()
            for c in (2, 3):
                for g in range(2):
                    bg.append((("vq", 1), v_group(c, g)))
            for q in range(2, 4):
                for g in range(2):
                    bg.append((("vq", q), v_group(2 * q, g)))
                    bg.append((("vq", q), v_group(2 * q + 1, g)))
            for h in range(1, HL):
                bg.append((("q", 1, h), qk_group("q", h, 1)))
                bg.append((("k", 0, h), qk_group("k", h, 0)))
                bg.append((("k", 1, h), qk_group("k", h, 1)))
            # bf16 sidecar tasks wait on tail-end DMAs; keep them out of the
            # main drain queue until their inputs have surely landed
            bg_late = []
            for hp in range(4):
                bg_late.append((("qkb", 2 * hp), qkb_group(hp)))
            for c in range(4):
                for g in range(2):
                    bg_late.append((("vb", c // 2), vb_group(c, g)))

            ot_tiles = {}
            pend = []       # one-pair-lagged PE emissions (attnV/Z, normalize)
            o_sb_pair = {}  # normalized o for the in-flight head pair

            pending_proj = []
            for si, j in enumerate(JORDER):
                if si == 1:  # release the bf16 sidecar work mid-flight
                    bg.extend(bg_late)
                    bg_late = []
                # enqueue the next sequence step's inputs
                if si + 1 < NJ:
                    nj = JORDER[si + 1]
                    if nj != 0:
                        for q in range(2 * nj, 2 * nj + 2):
                            for g in range(2):
                                bg.append((("vq", q), v_group(2 * q, g)))
                                bg.append((("vq", q), v_group(2 * q + 1, g)))
                    for h in range(HL):
                        bg.append((("q", nj, h), qk_group("q", h, nj)))
                        if nj != 0:  # k tile 0 was produced in the prologue
                            bg.append((("k", nj, h), qk_group("k", h, nj)))
                # weave last step's proj tiles into the fresh queue (avoids a
                # PE-only burst that starves ACT at the step boundary); all
                # transposes must go first (proj reads their output)
                if pending_proj:
                    force_bg(lambda t: t[0] == "tr")
                for i, task in enumerate(pending_proj):
                    bg.insert(min(5 * i + 2, len(bg)), task)
                pending_proj = []
                # proj from two sequence steps back must be out before its
                # ot slot is reused
                force_bg(lambda t: t[0] == "proj" and t[1] <= si - 2)
                ot_tile = otp.tile([128, 4, 512], BF16, tag="ot", name=f"ot{j}")
                ot_tiles[j] = ot_tile

                for h in range(HL):
                    hp, r0 = h // 2, 64 * (h % 2)
                    zc = 8 * (8 * j + h)
                    # everything this stream reads must already be emitted
                    force_bg(lambda t: (t[0] == "q" and t[1] == j and t[2] <= h) or
                             (t[0] == "k" and t[1] <= j and t[2] <= h) or
                             (j == 0 and t[0] == "qkb" and t[1] <= h))
                    po_t = ps_po.tile([128, 512], F32, tag="po", name="po")
                    po = po_t[0:64, :]
                    nq = 2 * j + 2
                    for q in range(nq):
                        m0, m2 = (q == 2 * j), (q == 2 * j + 1)
                        force_bg(lambda t: (t[0] == "vq" and t[1] < q) or
                                 (j == 0 and t[0] == "vb" and t[1] < q))
                        sT = ps_sT.tile([128, 1024], F32, tag="sT", name="sT")
                        if j == 0:
                            et = etp.tile([128, 1024], BF16, tag="etb",
                                          name="etb", bufs=4)
                            trm = tritb
                        else:
                            et = etp.tile([128, 1024], FP8, tag="et", name="et")
                            trm = trit
                        if not (m0 or m2):
                            # off-diag pair: both chunks full [0:512]
                            for t in range(2):
                                c = 2 * q + t
                                nc.tensor.matmul(
                                    sT[:, 512 * t:512 * t + 512],
                                    kTt[hp][c // 4][r0:r0 + 64,
                                                    128 * (c % 4):128 * (c % 4) + 128],
                                    qTt[hp][j][r0:r0 + 64, :],
                                    start=True, stop=True, skip_group_check=True)
                            nc.scalar.activation(et[:], sT[:], Exp, scale=EXP_SCALE)
                            stride, width = 512, 512
                        elif m0:
                            # chunks 4j (full), 4j+1 (cols 128:512)
                            c = 4 * j
                            if j == 0:
                                # block-0 scores from bf16-accurate q/k
                                nc.tensor.matmul(
                                    sT[:, 0:128],
                                    kbt[hp][r0:r0 + 64, :],
                                    qbt[hp][r0:r0 + 64, :],
                                    start=True, stop=False,
                                    skip_group_check=True)
                                nc.tensor.matmul(
                                    sT[:, 128:512],
                                    kTt[hp][j][r0:r0 + 64, 0:128],
                                    qTt[hp][j][r0:r0 + 64, 128:512],
                                    start=False, stop=True,
                                    skip_group_check=True)
                            else:
                                nc.tensor.matmul(
                                    sT[:, 0:512],
                                    kTt[hp][j][r0:r0 + 64, 0:128],
                                    qTt[hp][j][r0:r0 + 64, :],
                                    start=True, stop=True,
                                    skip_group_check=True)
                            # cover [512:640] too so exp never reads stale
                            # bytes (those weights get memset to 0 after)
                            nc.tensor.matmul(
                                sT[:, 512:1024],
                                kTt[hp][j][r0:r0 + 64, 128:256],
                                qTt[hp][j][r0:r0 + 64, :],
                                start=True, stop=True, skip_group_check=True)
                            nc.scalar.activation(et[:], sT[:], Exp, scale=EXP_SCALE)
                            et3 = et[:].rearrange("p (a n) -> p a n", n=128)
                            nc.vector.tensor_mul(et3[:, 0:6:5, :], et3[:, 0:6:5, :],
                                                 trm[:])
                            stride, width = 512, 512
                        else:
                            # m2: chunks 4j+2 (cols 256:512 -> [0:256]),
                            #     4j+3 (cols 384:512 -> [384:512])
                            nc.tensor.matmul(
                                sT[:, 0:256],
                                kTt[hp][j][r0:r0 + 64, 256:384],
                                qTt[hp][j][r0:r0 + 64, 256:512],
                                start=True, stop=True, skip_group_check=True)
                            nc.tensor.matmul(
                                sT[:, 256:512],
                                kTt[hp][j][r0:r0 + 64, 384:512],
                                qTt[hp][j][r0:r0 + 64, 256:512],
                                start=False, stop=True, skip_group_check=True)
                            nc.scalar.activation(et[:, 0:512], sT[:, 0:512],
                                                 Exp, scale=EXP_SCALE)
                            et3 = et[:].rearrange("p (a n) -> p a n", n=128)
                            nc.vector.tensor_mul(et3[:, 0:4:3, :], et3[:, 0:4:3, :],
                                                 trm[:])
                            stride, width = 256, 256
                        etv = et[:, 0:2 * stride].rearrange(
                            "p (a n) -> p a n", a=2)
                        s_lo = 4 if m2 else 0
                        base = 256 if m2 else 0

                        def attn_emit(et=et, etv=etv, q=q, h=h, po=po,
                                      po_t=po_t, zc=zc,
                                      s_lo=s_lo, base=base, m0=m0, m2=m2, j=j):
                            # exactly ONE start=True per bank-use: start=True
                            # flags the whole 2KB bank pending-zero; every
                            # other group's first touch consumes its flag.
                            if j == 0:
                                # bf16 non-DR path (no fp8 noise on the short-
                                # support rows). (tile, subtiles, flat offset fn)
                                if m0:
                                    work = [(0, range(0, 8), lambda s: 64 * s),
                                            (1, range(2, 8),
                                             lambda s: 512 + 64 * s)]
                                else:
                                    work = [(0, range(4, 8),
                                             lambda s: 64 * s - 256),
                                            (1, range(6, 8), lambda s: 64 * s)]
                                stop_at = {0: (0, 0), 1: (0, 0), 2: (0, 1),
                                           3: (0, 1), 4: (1, 0), 5: (1, 0),
                                           6: (1, 1), 7: (1, 1)}
                                for tt, srange, off in work:
                                    for s in srange:
                                        fo = off(s)
                                        stop = stop_at[s] == (q, tt)
                                        st = (q == 0 and tt == 0 and s == 0)
                                        nc.tensor.matmul(
                                            po[:, 64 * s:64 * s + 64],
                                            et[:, fo:fo + 64],
                                            vb[q][:, tt, h, 0:64],
                                            start=st, stop=stop,
                                            skip_group_check=True)
                                        nc.tensor.matmul(
                                            po_t[64:128, s:s + 1],
                                            et[:, fo:fo + 64],
                                            vb[q][:, tt, h, 64:65],
                                            start=st, stop=stop,
                                            skip_group_check=True)
                                return
                            for s in range(s_lo, 8):
                                cc = 64 * s - base
                                stop = (s < 4 and m0) or (s >= 4 and m2)
                                # tile1's dead block is never read: subtiles
                                # under the diagonal use a single-tile matmul
                                single = (m0 and s < 2) or (m2 and s < 6)
                                if single:
                                    lhs_o = etv[:, 0, cc:cc + 64]
                                    rhs_o = vp[q][:, 0, h, 0:64]
                                    pm = None
                                else:
                                    lhs_o = etv[:, :, cc:cc + 64]
                                    rhs_o = vp[q][:, :, h, 0:64]
                                    pm = DR
                                nc.tensor.matmul(
                                    po[:, 64 * s:64 * s + 64],
                                    lhs_o, rhs_o,
                                    start=(q == 0 and s == 0), stop=stop,
                                    perf_mode=pm, skip_group_check=True)
                                # Z columns live at partitions 64:128 of the
                                # same po bank (single-tile, non-DR: base-64)
                                nc.tensor.matmul(
                                    po_t[64:128, s:s + 1],
                                    etv[:, 0, cc:cc + 64],
                                    vp[q][:, 0, h, 64:65],
                                    start=(q == 0 and s == 0), stop=(stop and single),
                                    skip_group_check=True)
                                if not single:
                                    nc.tensor.matmul(
                                        po_t[64:128, s:s + 1],
                                        etv[:, 1, cc:cc + 64],
                                        vp[q][:, 1, h, 64:65],
                                        start=False, stop=stop,
                                        skip_group_check=True)
                        # one-pair software pipeline: previous pair's attnV/Z
                        # runs while this pair's exp/masks are in flight
                        while pend:
                            pend.pop(0)()
                        pend.append(attn_emit)
                        drain_bg(3 if h == HL - 1 else (2 if len(bg) > 40 else 1))

                    force_bg(lambda t: (t[0] == "vq" and t[1] <= 2 * j + 1) or
                             (j == 0 and t[0] == "vb"))

                    def norm_emit(po=po, po_t=po_t, h=h, hp=hp,
                                  ot_tile=ot_tile):
                        rz = rzp.tile([64, 8], F32, tag="rz", name="rz")
                        nc.vector.reciprocal(rz[:], po_t[64:128, 0:8])
                        o_sb = osb.tile([64, 512], BF16, tag="os", name="os")
                        nc.vector.tensor_tensor(
                            o_sb[:].rearrange("p (e s) -> p e s", e=8),
                            po[:].rearrange("p (e s) -> p e s", e=8),
                            rz[:].unsqueeze(2).broadcast_to([64, 8, 64]),
                            mybir.AluOpType.mult)
                        o_sb_pair[h % 2] = o_sb
                        if h % 2 == 1:
                            def transpose_emit(hp=hp, ot_tile=ot_tile,
                                               pair=dict(o_sb_pair)):
                                pt_t = ps_po.tile([128, 512], F32, tag="po",
                                                  name="pt")
                                pt = pt_t[:].bitcast(BF16)[:, 0:512]
                                for hh in range(2):
                                    for s in range(8):
                                        nc.tensor.matmul(
                                            pt[64 * hh:64 * hh + 64,
                                               64 * s:64 * s + 64],
                                            pair[hh][:, 64 * s:64 * s + 64],
                                            idt[:], is_transpose=True,
                                            start=(s == 0),
                                            stop=(hh == 1 and s == 7),
                                            skip_group_check=True)
                                nc.vector.tensor_copy(ot_tile[:, hp, :], pt[:])
                            bg.insert(min(1, len(bg)),
                                      (("tr", None), transpose_emit))
                    pend.append(norm_emit)

                # flush the pipeline at the j boundary so the last head pair's
                # normalize + transpose are queued before proj tasks
                while pend:
                    pend.pop(0)()
                for t in range(4):
                    for nh in range(2):
                        pending_proj.append((("proj", si),
                                             proj_tile(j, t, nh, ot_tile)))

            while pend:
                pend.pop(0)()
            drain_bg(len(bg))
            for _, task in pending_proj:
                task()

    nc.compile()
    return nc


def _host_prep(x, Wq, Wk, Wv, Wp):
    """Per-core input maps."""
    tri = (np.arange(128)[None, :] >= np.arange(128)[:, None]).astype(np.float32)
    tri2 = np.concatenate([tri, tri], axis=1).astype(f8np)
    ident = np.eye(64, dtype=np.float32).astype(bfnp)
    WpT = np.ascontiguousarray(Wp.T) * (1.0 / SV)  # [E(hd), E]

    def wslice(W, h0, scale, dt=f8np):  # [H,E,D] -> [E, 8*64]
        w = W[h0:h0 + HL].transpose(1, 0, 2).reshape(E, HL * D) * scale
        return np.ascontiguousarray(w).astype(dt)

    in_maps = []
    for c in range(8):
        b, hh = c // 2, c % 2
        h0 = hh * HL
        in_maps.append({
            "xT8": np.ascontiguousarray(x[b].T).astype(f8np),
            "wq8": wslice(Wq, h0, SW),
            "wk8": wslice(Wk, h0, SW),
            "wv8": wslice(Wv, h0, SV),
            "wpT": np.ascontiguousarray(
                WpT[h0 * D:(h0 + HL) * D, :]).astype(bfnp),
            "tri2": tri2,
            "tri2b": tri2.astype(np.float32).astype(bfnp),
            "ident": ident,
            "xbT": np.ascontiguousarray(x[b].T[:, 0:512]).astype(bfnp),
            "wqb": wslice(Wq, h0, SW, bfnp),
            "wkb": wslice(Wk, h0, SW, bfnp),
            "wvb": wslice(Wv, h0, SV, bfnp),
        })
    return in_maps


def kernel(x, Wq, Wk, Wv, Wp, bp):
    x = np.asarray(x, dtype=np.float32)
    Wq = np.asarray(Wq, dtype=np.float32)
    Wk = np.asarray(Wk, dtype=np.float32)
    Wv = np.asarray(Wv, dtype=np.float32)
    Wp = np.asarray(Wp, dtype=np.float32)
    bp = np.asarray(bp, dtype=np.float32)

    if "nc" not in _CACHE:
        _CACHE["nc"] = _build()
    nc = _CACHE["nc"]

    in_maps = _host_prep(x, Wq, Wk, Wv, Wp)
    res = run_bass_kernel_spmd(nc, in_maps, list(range(8)))
    parts = [np.asarray(res.results[c]["out"], dtype=np.float32) for c in range(8)]
    out = np.stack([parts[2 * b] + parts[2 * b + 1] for b in range(B)], axis=0)
    return (out + bp[None, None, :]).astype(np.float32)


# revision 115
# speedup vs baseline: 1.3424x; 1.0006x over previous
"""Multi-head causal attention (B=4,T=2048,E=1024,H=16,D=64) on 8 TRN2 cores.

Sharding: core c -> batch b=c//2, heads h0=(c%2)*8 .. h0+8. Each core computes
its 8 heads' attention and a partial output projection (row-split Wp); host
sums the two partials per batch (+bias).

Per-core kernel (tq tiles processed in order 1,2,3,0):
  - Q/K/V projections in fp8e4 DoubleRow (2 k-tiles of 128 per instr, 0.5
    cyc/col), M=64 outputs at PSUM base 0. Host pre-scales Wq,Wk by 64 and
    Wv by 16 (powers of 2, folded back via exp scale 2^-15 / Wp scale).
  - scores: bf16 kT.T @ qT per 128-row tk chunk; chunk PAIRS share one
    [128,1024] PSUM tile (2 banks) so exp runs as one ACT op per pair.
    Diagonal chunks compute their dead region too so exp never reads
    stale PSUM (race-free; CoreSim conflict-checker clean).
  - exp -> fp8e4 "et" tiles [128, 2, N]; causal triangle masked by DVE
    multiplies; fully-dead subtiles skip the second DR k-tile instead of
    being zeroed.
  - attnV: o-layout fp8 DoubleRow o[tq64, d64] += et.T @ v over chunk
    pairs; softmax Z accumulates as single-tile matmul columns in the
    SAME po bank at partitions 64:127 (base-64 is legal for non-DR).
    Normalize is per-partition: DVE reciprocal [64,8] + broadcast mult.
  - o -> oT via PE transposes (identity matmul) packed 2 heads/bank,
    then bf16 output projection, DVE copy, DMA out.
  - j=0 (first 512 tokens) runs attention in bf16 (v, q/k for tokens
    0:128 recomputed from bf16 x/W): short-support softmax rows cannot
    average away fp8 quantization noise. Processed last so its extra
    DMAs/projections never gate the critical path.
  - Emission is software-pipelined: attnV/Z trail scores by one pair and
    background work (projections, transposes, output tiles) drains from
    a tagged queue with just-in-time forced ordering.
"""
import sys
import numpy as np

sys.path.insert(0, "/opt/trn_rl_repo")

import ml_dtypes
import concourse.bass as bass
import concourse.bacc as bacc
import concourse.mybir as mybir
from concourse import tile
from concourse.bass_utils import run_bass_kernel_spmd

B, T, E, H, D = 4, 2048, 1024, 16, 64
HL = H // 2          # 8 local heads per core
NJ = T // 512        # 4 tq tiles
NPE = E // 256       # 4 E-chunk-pairs
NPAIR = T // 256     # 8 tk chunk pairs
BF16 = mybir.dt.bfloat16
F32 = mybir.dt.float32
FP8 = mybir.dt.float8e4
DR = mybir.MatmulPerfMode.DoubleRow
Exp = mybir.ActivationFunctionType.Exp
f8np = ml_dtypes.float8_e4m3
bfnp = ml_dtypes.bfloat16

SW = 64.0   # Wq/Wk host prescale (exp scale folds 1/SW^2)
SV = 16.0   # Wv host prescale (Wp folds 1/SV)
EXP_SCALE = 0.125 / (SW * SW)  # 2^-15 exactly

_CACHE = {}


def _build():
    nc = bacc.Bacc("TRN2", target_bir_lowering=False)
    xT8 = nc.declare_dram_parameter("xT8", [E, T], FP8, isOutput=False)
    wq8 = nc.declare_dram_parameter("wq8", [E, HL * D], FP8, isOutput=False)
    wk8 = nc.declare_dram_parameter("wk8", [E, HL * D], FP8, isOutput=False)
    wv8 = nc.declare_dram_parameter("wv8", [E, HL * D], FP8, isOutput=False)
    wpT = nc.declare_dram_parameter("wpT", [HL * D, E], BF16, isOutput=False)
    tri2 = nc.declare_dram_parameter("tri2", [128, 256], FP8, isOutput=False)
    tri2b = nc.declare_dram_parameter("tri2b", [128, 256], BF16, isOutput=False)
    xbT = nc.declare_dram_parameter("xbT", [E, 512], BF16, isOutput=False)
    wqb = nc.declare_dram_parameter("wqb", [E, HL * D], BF16, isOutput=False)
    wkb = nc.declare_dram_parameter("wkb", [E, HL * D], BF16, isOutput=False)
    wvb = nc.declare_dram_parameter("wvb", [E, HL * D], BF16, isOutput=False)
    ident = nc.declare_dram_parameter("ident", [64, 64], BF16, isOutput=False)
    out = nc.declare_dram_parameter("out", [T, E], F32, isOutput=True)

    with tile.TileContext(nc) as tc:
        with (
            tc.tile_pool(name="pp", bufs=1) as pp,
            tc.tile_pool(name="etp", bufs=14) as etp,
            tc.tile_pool(name="osb", bufs=10) as osb,
            tc.tile_pool(name="rzp", bufs=8) as rzp,
            tc.tile_pool(name="otp", bufs=3) as otp,
            tc.tile_pool(name="obp", bufs=6) as obp,
            tc.tile_pool(name="sT", bufs=2, space=bass.MemorySpace.PSUM) as ps_sT,
            tc.tile_pool(name="po", bufs=2, space=bass.MemorySpace.PSUM) as ps_po,
            tc.tile_pool(name="mm", bufs=2, space=bass.MemorySpace.PSUM) as ps_mm,
        ):
            # ---- persistent SBUF tiles
            x8a = pp.tile([128, NPE, 2, T], FP8, tag="x8", name="x8a")
            wq8a = pp.tile([128, NPE, 2, 512], FP8, tag="wq", name="wq8a")
            wk8a = pp.tile([128, NPE, 2, 512], FP8, tag="wk", name="wk8a")
            wv8a = pp.tile([128, NPE, 2, 512], FP8, tag="wv", name="wv8a")
            x8t = [x8a[:, p] for p in range(NPE)]
            wq8t = [wq8a[:, p] for p in range(NPE)]
            wk8t = [wk8a[:, p] for p in range(NPE)]
            wv8t = [wv8a[:, p] for p in range(NPE)]
            wpa = pp.tile([128, 4, E], BF16, tag="wp", name="wpa")
            wpt = [wpa[:, p] for p in range(4)]
            trit = pp.tile([128, 2, 128], FP8, tag="tri", name="trit")
            tritb = pp.tile([128, 2, 128], BF16, tag="trib", name="tritb")
            idt = pp.tile([64, 64], BF16, tag="id", name="idt")
            qTt = [[pp.tile([128, 512], BF16, tag=f"q{p}_{j}", name=f"q{p}_{j}")
                    for j in range(NJ)] for p in range(4)]
            kTt = [[pp.tile([128, 512], BF16, tag=f"k{p}_{j}", name=f"k{p}_{j}")
                    for j in range(NJ)] for p in range(4)]
            vp = [pp.tile([128, 2, HL, 65], FP8, tag=f"v{q}", name=f"v{q}")
                  for q in range(NPAIR)]
            # bf16 v for chunks 0..3: j=0 attention runs in bf16 (short-support
            # softmax rows can't average away fp8 quantization noise)
            vb = [pp.tile([128, 2, HL, 65], BF16, tag=f"vb{q}", name=f"vb{q}")
                  for q in range(2)]
            # bf16 x/W and q/k for the first 128 tokens (block-0 scores)
            xba = pp.tile([128, NPE, 2, 512], BF16, tag="xba", name="xba")
            wqba = pp.tile([128, NPE, 2, 512], BF16, tag="wqa", name="wqba")
            wkba = pp.tile([128, NPE, 2, 512], BF16, tag="wka", name="wkba")
            wvba = pp.tile([128, NPE, 2, 512], BF16, tag="wva", name="wvba")
            xbt = [xba[:, p] for p in range(NPE)]
            wqbt = [wqba[:, p] for p in range(NPE)]
            wkbt = [wkba[:, p] for p in range(NPE)]
            wvbt = [wvba[:, p] for p in range(NPE)]
            qbt = [pp.tile([128, 128], BF16, tag=f"qb{p}", name=f"qb{p}")
                   for p in range(4)]
            kbt = [pp.tile([128, 128], BF16, tag=f"kb{p}", name=f"kb{p}")
                   for p in range(4)]

            # ---- DMA in. j tiles are processed in order [1,2,3,0], so the
            # critical path is x cols [0:1024] + wv (SP) and wq/wk (ACT).
            # The bf16 sidecar tiles (j=0 accuracy path) ride at the SP tail.
            def cdma(queue, dst, src, cols=None):
                # whole class in one DMA: src rows (pe t p) -> [p, pe, t, n]
                s = src if cols is None else src[:, cols[0]:cols[1]]
                queue.dma_start(dst, s.rearrange("(pe t p) n -> p pe t n",
                                                 pe=NPE, t=2))

            def tdma(queue, dst, src, cols=None):
                # both t-halves in one DMA: src rows (t p) -> dst [p, t, n]
                s = src if cols is None else src[:, cols[0]:cols[1]]
                queue.dma_start(dst, s.rearrange("(t p) n -> p t n", t=2))
            cdma(nc.sync, x8a[:, :, :, 0:512], xT8, (0, 512))
            cdma(nc.sync, wk8a[:], wk8)
            cdma(nc.sync, wq8a[:], wq8)
            cdma(nc.sync, x8a[:, :, :, 512:1024], xT8, (512, 1024))
            cdma(nc.sync, wv8a[:], wv8)
            nc.sync.dma_start(trit[:], tri2[:].rearrange("p (a n) -> p a n", a=2))
            nc.sync.dma_start(idt[:], ident[:])
            for jj in range(2, NJ):
                cdma(nc.sync, x8a[:, :, :, 512 * jj:512 * jj + 512], xT8,
                     (512 * jj, 512 * jj + 512))
            nc.sync.dma_start(
                wpa[:], wpT[:].rearrange("(q p) n -> p q n", q=4))
            nc.sync.dma_start(tritb[:],
                              tri2b[:].rearrange("p (a n) -> p a n", a=2))
            cdma(nc.sync, xba[:], xbT)
            cdma(nc.sync, wvba[:], wvb)
            cdma(nc.sync, wqba[:], wqb)
            cdma(nc.sync, wkba[:], wkb)
            for q in range(NPAIR):
                nc.gpsimd.memset(vp[q][:, :, :, 64:65], 1.0)
            for q in range(2):
                nc.gpsimd.memset(vb[q][:, :, :, 64:65], 1.0)

            # ---- background task machinery (qkv groups, proj tiles)
            def qk_group(kind, h, j, pool=None):
                def emit():
                    wt = wq8t if kind == "q" else wk8t
                    dst = qTt if kind == "q" else kTt
                    if pool is None:
                        m = ps_mm.tile([128, 512], F32, tag="mm", name="mmq")
                    else:
                        m = pool.tile([128, 1024], F32, tag="sT",
                                      name="mmq")[:, 0:512]
                    for p in range(NPE):
                        nc.tensor.matmul(
                            m[0:64, :], wt[p][:, :, 64 * h:64 * h + 64],
                            x8t[p][:, :, 512 * j:512 * j + 512],
                            start=(p == 0), stop=(p == NPE - 1), perf_mode=DR)
                    nc.vector.tensor_copy(
                        dst[h // 2][j][64 * (h % 2):64 * (h % 2) + 64, :],
                        m[0:64, :])
                return emit

            def qkb_group(pe_hp):  # bf16 q/k for tokens 0..127, head pair
                def emit():
                    for wt, dst in ((wqbt, qbt), (wkbt, kbt)):
                        m = ps_mm.tile([128, 512], F32, tag="mm", name="mmb")
                        for p in range(NPE):
                            for t in range(2):
                                nc.tensor.matmul(
                                    m[0:128, 0:128],
                                    wt[p][:, t, 128 * pe_hp:128 * pe_hp + 128],
                                    xbt[p][:, t, 0:128],
                                    start=(p == 0 and t == 0),
                                    stop=(p == NPE - 1 and t == 1))
                        nc.vector.tensor_copy(dst[pe_hp][:], m[0:128, 0:128])
                return emit

            def v_group(c, g, pool=None):  # chunk c, t-64 sub g
                def emit():
                    if pool is None:
                        m = ps_mm.tile([128, 512], F32, tag="mm", name="mmv")
                    else:
                        m = pool.tile([128, 1024], F32, tag="sT",
                                      name="mmv")[:, 0:512]
                    t0 = 128 * c + 64 * g
                    for p in range(NPE):
                        nc.tensor.matmul(
                            m[0:64, :], x8t[p][:, :, t0:t0 + 64], wv8t[p][:],
                            start=(p == 0), stop=(p == NPE - 1), perf_mode=DR)
                    nc.vector.tensor_copy(
                        vp[c // 2][64 * g:64 * g + 64, c % 2, :, 0:64],
                        m[0:64, :].rearrange("p (h d) -> p h d", d=64))
                return emit

            def vb_group(c, g):  # true bf16 v for j=0 (bf16 x and Wv)
                def emit():
                    m = ps_mm.tile([128, 512], F32, tag="mm", name="mvb")
                    t0 = 128 * c + 64 * g
                    for p in range(NPE):
                        for t in range(2):
                            nc.tensor.matmul(
                                m[0:64, :],
                                xbt[p][:, t, t0:t0 + 64],
                                wvbt[p][:, t, :],
                                start=(p == 0 and t == 0),
                                stop=(p == NPE - 1 and t == 1))
                    nc.vector.tensor_copy(
                        vb[c // 2][64 * g:64 * g + 64, c % 2, :, 0:64],
                        m[0:64, :].rearrange("p (h d) -> p h d", d=64))
                return emit

            def proj_tile(j, t, nh, ot_tile):
                def emit():
                    m = ps_mm.tile([128, 512], F32, tag="mm", name="mmp")
                    for p in range(4):
                        nc.tensor.matmul(
                            m[:], ot_tile[:, p, 128 * t:128 * t + 128],
                            wpt[p][:, 512 * nh:512 * nh + 512],
                            start=(p == 0), stop=(p == 3))
                    ob = obp.tile([128, 512], F32, tag="ob", name="ob")
                    nc.vector.tensor_copy(ob[:], m[:])
                    nc.sync.dma_start(
                        out[512 * j + 128 * t:512 * j + 128 * t + 128,
                            512 * nh:512 * nh + 512], ob[:])
                return emit

            bg = []  # queue of (tag, emitter); tags order forced at stream starts

            def drain_bg(n=1):
                for _ in range(min(n, len(bg))):
                    bg.pop(0)[1]()

            def force_bg(pred):
                """Emit from the front until no queued task satisfies pred."""
                while any(pred(t) for t, _ in bg):
                    bg.pop(0)[1]()

            # j tiles processed [1, 2, 3, 0]: the bf16 j=0 accuracy path runs
            # last so its DMAs/projections never gate the critical path.
            JORDER = (1, 2, 3, 0)
            # prologue: v pairs 0..1 (chunks 0..3); q tile 1, k tiles 0..1
            # for h=0. k projections are scheduled by tk-tile index: stream
            # (j, h) consumes k tiles 0..j.
            # interleave so the first stream's scores are unblocked ASAP
            qk_group("k", 0, 0)()
            qk_group("k", 0, 1, ps_sT)()
            qk_group("q", 0, 1)()
            v_group(0, 0, ps_sT)()
            v_group(0, 1)()
            v_group(1, 0, ps_sT)()
            v_group(1, 1)()
            for c in (2, 3):
                for g in range(2):
                    bg.append((("vq", 1), v_group(c, g)))
            for q in range(2, 4):
                for g in range(2):
                    bg.append((("vq", q), v_group(2 * q, g)))
                    bg.append((("vq", q), v_group(2 * q + 1, g)))
            for h in range(1, HL):
                bg.append((("q", 1, h), qk_group("q", h, 1)))
                bg.append((("k", 0, h), qk_group("k", h, 0)))
                bg.append((("k", 1, h), qk_group("k", h, 1)))
            # bf16 sidecar tasks wait on tail-end DMAs; keep them out of the
            # main drain queue until their inputs have surely landed
            bg_late = []
            for hp in range(4):
                bg_late.append((("qkb", 2 * hp), qkb_group(hp)))
            for c in range(4):
                for g in range(2):
                    bg_late.append((("vb", c // 2), vb_group(c, g)))

            ot_tiles = {}
            pend = []       # one-pair-lagged PE emissions (attnV/Z, normalize)
            o_sb_pair = {}  # normalized o for the in-flight head pair

            pending_proj = []
            for si, j in enumerate(JORDER):
                if si == 1:  # release the bf16 sidecar work mid-flight
                    bg.extend(bg_late)
                    bg_late = []
                # enqueue the next sequence step's inputs
                if si + 1 < NJ:
                    nj = JORDER[si + 1]
                    if nj != 0:
                        for q in range(2 * nj, 2 * nj + 2):
                            for g in range(2):
                                bg.append((("vq", q), v_group(2 * q, g)))
                                bg.append((("vq", q), v_group(2 * q + 1, g)))
                    for h in range(HL):
                        bg.append((("q", nj, h), qk_group("q", h, nj)))
                        if nj != 0:  # k tile 0 was produced in the prologue
                            bg.append((("k", nj, h), qk_group("k", h, nj)))
                # weave last step's proj tiles into the fresh queue (avoids a
                # PE-only burst that starves ACT at the step boundary); all
                # transposes must go first (proj reads their output)
                if pending_proj:
                    force_bg(lambda t: t[0] == "tr")
                for i, task in enumerate(pending_proj):
                    bg.insert(min(5 * i + 2, len(bg)), task)
                pending_proj = []
                # proj from two sequence steps back must be out before its
                # ot slot is reused
                force_bg(lambda t: t[0] == "proj" and t[1] <= si - 2)
                ot_tile = otp.tile([128, 4, 512], BF16, tag="ot", name=f"ot{j}")
                ot_tiles[j] = ot_tile

                for h in range(HL):
                    hp, r0 = h // 2, 64 * (h % 2)
                    zc = 8 * (8 * j + h)
                    # everything this stream reads must already be emitted
                    force_bg(lambda t: (t[0] == "q" and t[1] == j and t[2] <= h) or
                             (t[0] == "k" and t[1] <= j and t[2] <= h) or
                             (j == 0 and t[0] == "qkb" and t[1] <= h))
                    po_t = ps_po.tile([128, 512], F32, tag="po", name="po")
                    po = po_t[0:64, :]
                    nq = 2 * j + 2
                    for q in range(nq):
                        m0, m2 = (q == 2 * j), (q == 2 * j + 1)
                        force_bg(lambda t: (t[0] == "vq" and t[1] <= q) or
                                 (j == 0 and t[0] == "vb" and t[1] <= q))
                        sT = ps_sT.tile([128, 1024], F32, tag="sT", name="sT")
                        if j == 0:
                            et = etp.tile([128, 1024], BF16, tag="etb",
                                          name="etb", bufs=4)
                            trm = tritb
                        else:
                            et = etp.tile([128, 1024], FP8, tag="et", name="et")
                            trm = trit
                        if not (m0 or m2):
                            # off-diag pair: both chunks full [0:512]
                            for t in range(2):
                                c = 2 * q + t
                                nc.tensor.matmul(
                                    sT[:, 512 * t:512 * t + 512],
                                    kTt[hp][c // 4][r0:r0 + 64,
                                                    128 * (c % 4):128 * (c % 4) + 128],
                                    qTt[hp][j][r0:r0 + 64, :],
                                    start=True, stop=True, skip_group_check=True)
                            nc.scalar.activation(et[:], sT[:], Exp, scale=EXP_SCALE)
                            stride, width = 512, 512
                        elif m0:
                            # chunks 4j (full), 4j+1 (cols 128:512)
                            c = 4 * j
                            if j == 0:
                                # block-0 scores from bf16-accurate q/k
                                nc.tensor.matmul(
                                    sT[:, 0:128],
                                    kbt[hp][r0:r0 + 64, :],
                                    qbt[hp][r0:r0 + 64, :],
                                    start=True, stop=False,
                                    skip_group_check=True)
                                nc.tensor.matmul(
                                    sT[:, 128:512],
                                    kTt[hp][j][r0:r0 + 64, 0:128],
                                    qTt[hp][j][r0:r0 + 64, 128:512],
                                    start=False, stop=True,
                                    skip_group_check=True)
                            else:
                                nc.tensor.matmul(
                                    sT[:, 0:512],
                                    kTt[hp][j][r0:r0 + 64, 0:128],
                                    qTt[hp][j][r0:r0 + 64, :],
                                    start=True, stop=True,
                                    skip_group_check=True)
                            # cover [512:640] too so exp never reads stale
                            # bytes (those weights get memset to 0 after)
                            nc.tensor.matmul(
                                sT[:, 512:1024],
                                kTt[hp][j][r0:r0 + 64, 128:256],
                                qTt[hp][j][r0:r0 + 64, :],
                                start=True, stop=True, skip_group_check=True)
                            nc.scalar.activation(et[:], sT[:], Exp, scale=EXP_SCALE)
                            et3 = et[:].rearrange("p (a n) -> p a n", n=128)
                            nc.vector.tensor_mul(et3[:, 0:6:5, :], et3[:, 0:6:5, :],
                                                 trm[:])
                            stride, width = 512, 512
                        else:
                            # m2: chunks 4j+2 (cols 256:512 -> [0:256]),
                            #     4j+3 (cols 384:512 -> [384:512])
                            nc.tensor.matmul(
                                sT[:, 0:256],
                                kTt[hp][j][r0:r0 + 64, 256:384],
                                qTt[hp][j][r0:r0 + 64, 256:512],
                                start=True, stop=True, skip_group_check=True)
                            nc.tensor.matmul(
                                sT[:, 256:512],
                                kTt[hp][j][r0:r0 + 64, 384:512],
                                qTt[hp][j][r0:r0 + 64, 256:512],
                                start=False, stop=True, skip_group_check=True)
                            nc.scalar.activation(et[:, 0:512], sT[:, 0:512],
                                                 Exp, scale=EXP_SCALE)
                            et3 = et[:].rearrange("p (a n) -> p a n", n=128)
                            nc.vector.tensor_mul(et3[:, 0:4:3, :], et3[:, 0:4:3, :],
                                                 trm[:])
                            stride, width = 256, 256
                        etv = et[:, 0:2 * stride].rearrange(
                            "p (a n) -> p a n", a=2)
                        s_lo = 4 if m2 else 0
                        base = 256 if m2 else 0

                        def attn_emit(et=et, etv=etv, q=q, h=h, po=po,
                                      po_t=po_t, zc=zc,
                                      s_lo=s_lo, base=base, m0=m0, m2=m2, j=j):
                            # exactly ONE start=True per bank-use: start=True
                            # flags the whole 2KB bank pending-zero; every
                            # other group's first touch consumes its flag.
                            if j == 0:
                                # bf16 non-DR path (no fp8 noise on the short-
                                # support rows). (tile, subtiles, flat offset fn)
                                if m0:
                                    work = [(0, range(0, 8), lambda s: 64 * s),
                                            (1, range(2, 8),
                                             lambda s: 512 + 64 * s)]
                                else:
                                    work = [(0, range(4, 8),
                                             lambda s: 64 * s - 256),
                                            (1, range(6, 8), lambda s: 64 * s)]
                                stop_at = {0: (0, 0), 1: (0, 0), 2: (0, 1),
                                           3: (0, 1), 4: (1, 0), 5: (1, 0),
                                           6: (1, 1), 7: (1, 1)}
                                for tt, srange, off in work:
                                    for s in srange:
                                        fo = off(s)
                                        stop = stop_at[s] == (q, tt)
                                        st = (q == 0 and tt == 0 and s == 0)
                                        nc.tensor.matmul(
                                            po[:, 64 * s:64 * s + 64],
                                            et[:, fo:fo + 64],
                                            vb[q][:, tt, h, 0:64],
                                            start=st, stop=stop,
                                            skip_group_check=True)
                                        nc.tensor.matmul(
                                            po_t[64:128, s:s + 1],
                                            et[:, fo:fo + 64],
                                            vb[q][:, tt, h, 64:65],
                                            start=st, stop=stop,
                                            skip_group_check=True)
                                return
                            for s in range(s_lo, 8):
                                cc = 64 * s - base
                                stop = (s < 4 and m0) or (s >= 4 and m2)
                                # tile1's dead block is never read: subtiles
                                # under the diagonal use a single-tile matmul
                                single = (m0 and s < 2) or (m2 and s < 6)
                                if single:
                                    lhs_o = etv[:, 0, cc:cc + 64]
                                    rhs_o = vp[q][:, 0, h, 0:64]
                                    pm = None
                                else:
                                    lhs_o = etv[:, :, cc:cc + 64]
                                    rhs_o = vp[q][:, :, h, 0:64]
                                    pm = DR
                                nc.tensor.matmul(
                                    po[:, 64 * s:64 * s + 64],
                                    lhs_o, rhs_o,
                                    start=(q == 0 and s == 0), stop=stop,
                                    perf_mode=pm, skip_group_check=True)
                                # Z columns live at partitions 64:128 of the
                                # same po bank (single-tile, non-DR: base-64)
                                nc.tensor.matmul(
                                    po_t[64:128, s:s + 1],
                                    etv[:, 0, cc:cc + 64],
                                    vp[q][:, 0, h, 64:65],
                                    start=(q == 0 and s == 0), stop=(stop and single),
                                    skip_group_check=True)
                                if not single:
                                    nc.tensor.matmul(
                                        po_t[64:128, s:s + 1],
                                        etv[:, 1, cc:cc + 64],
                                        vp[q][:, 1, h, 64:65],
                                        start=False, stop=stop,
                                        skip_group_check=True)
                        # one-pair software pipeline: previous pair's attnV/Z
                        # runs while this pair's exp/masks are in flight
                        while pend:
                            pend.pop(0)()
                        pend.append(attn_emit)
                        drain_bg(3 if h == HL - 1 else (2 if len(bg) > 40 else 1))

                    force_bg(lambda t: (t[0] == "vq" and t[1] <= 2 * j + 1) or
                             (j == 0 and t[0] == "vb"))

                    def norm_emit(po=po, po_t=po_t, h=h, hp=hp,
                                  ot_tile=ot_tile):
                        rz = rzp.tile([64, 8], F32, tag="rz", name="rz")
                        nc.vector.reciprocal(rz[:], po_t[64:128, 0:8])
                        o_sb = osb.tile([64, 512], BF16, tag="os", name="os")
                        nc.vector.tensor_tensor(
                            o_sb[:].rearrange("p (e s) -> p e s", e=8),
                            po[:].rearrange("p (e s) -> p e s", e=8),
                            rz[:].unsqueeze(2).broadcast_to([64, 8, 64]),
                            mybir.AluOpType.mult)
                        o_sb_pair[h % 2] = o_sb
                        if h % 2 == 1:
                            def transpose_emit(hp=hp, ot_tile=ot_tile,
                                               pair=dict(o_sb_pair)):
                                pt_t = ps_po.tile([128, 512], F32, tag="po",
                                                  name="pt")
                                pt = pt_t[:].bitcast(BF16)[:, 0:512]
                                for hh in range(2):
                                    for s in range(8):
                                        nc.tensor.matmul(
                                            pt[64 * hh:64 * hh + 64,
                                               64 * s:64 * s + 64],
                                            pair[hh][:, 64 * s:64 * s + 64],
                                            idt[:], is_transpose=True,
                                            start=(s == 0),
                                            stop=(hh == 1 and s == 7),
                                            skip_group_check=True)
                                nc.vector.tensor_copy(ot_tile[:, hp, :], pt[:])
                            bg.insert(min(1, len(bg)),
                                      (("tr", None), transpose_emit))
                    pend.append(norm_emit)

                # flush the pipeline at the j boundary so the last head pair's
                # normalize + transpose are queued before proj tasks
                while pend:
                    pend.pop(0)()
                for t in range(4):
                    for nh in range(2):
                        pending_proj.append((("proj", si),
                                             proj_tile(j, t, nh, ot_tile)))

            while pend:
                pend.pop(0)()
            drain_bg(len(bg))
            for _, task in pending_proj:
                task()

    nc.compile()
    return nc


def _host_prep(x, Wq, Wk, Wv, Wp):
    """Per-core input maps."""
    tri = (np.arange(128)[None, :] >= np.arange(128)[:, None]).astype(np.float32)
    tri2 = np.concatenate([tri, tri], axis=1).astype(f8np)
    ident = np.eye(64, dtype=np.float32).astype(bfnp)
    WpT = np.ascontiguousarray(Wp.T) * (1.0 / SV)  # [E(hd), E]

    def wslice(W, h0, scale, dt=f8np):  # [H,E,D] -> [E, 8*64]
        w = W[h0:h0 + HL].transpose(1, 0, 2).reshape(E, HL * D) * scale
        return np.ascontiguousarray(w).astype(dt)

    in_maps = []
    for c in range(8):
        b, hh = c // 2, c % 2
        h0 = hh * HL
        in_maps.append({
            "xT8": np.ascontiguousarray(x[b].T).astype(f8np),
            "wq8": wslice(Wq, h0, SW),
            "wk8": wslice(Wk, h0, SW),
            "wv8": wslice(Wv, h0, SV),
            "wpT": np.ascontiguousarray(
                WpT[h0 * D:(h0 + HL) * D, :]).astype(bfnp),
            "tri2": tri2,
            "tri2b": tri2.astype(np.float32).astype(bfnp),
            "ident": ident,
            "xbT": np.ascontiguousarray(x[b].T[:, 0:512]).astype(bfnp),
            "wqb": wslice(Wq, h0, SW, bfnp),
            "wkb": wslice(Wk, h0, SW, bfnp),
            "wvb": wslice(Wv, h0, SV, bfnp),
        })
    return in_maps


def kernel(x, Wq, Wk, Wv, Wp, bp):
    x = np.asarray(x, dtype=np.float32)
    Wq = np.asarray(Wq, dtype=np.float32)
    Wk = np.asarray(Wk, dtype=np.float32)
    Wv = np.asarray(Wv, dtype=np.float32)
    Wp = np.asarray(Wp, dtype=np.float32)
    bp = np.asarray(bp, dtype=np.float32)

    if "nc" not in _CACHE:
        _CACHE["nc"] = _build()
    nc = _CACHE["nc"]

    in_maps = _host_prep(x, Wq, Wk, Wv, Wp)
    res = run_bass_kernel_spmd(nc, in_maps, list(range(8)))
    parts = [np.asarray(res.results[c]["out"], dtype=np.float32) for c in range(8)]
    out = np.stack([parts[2 * b] + parts[2 * b + 1] for b in range(B)], axis=0)
    return (out + bp[None, None, :]).astype(np.float32)


# revision 116
# speedup vs baseline: 1.3443x; 1.0014x over previous
"""Multi-head causal attention (B=4,T=2048,E=1024,H=16,D=64) on 8 TRN2 cores.

Sharding: core c -> batch b=c//2, heads h0=(c%2)*8 .. h0+8. Each core computes
its 8 heads' attention and a partial output projection (row-split Wp); host
sums the two partials per batch (+bias).

Per-core kernel (tq tiles processed in order 1,2,3,0):
  - Q/K/V projections in fp8e4 DoubleRow (2 k-tiles of 128 per instr, 0.5
    cyc/col), M=64 outputs at PSUM base 0. Host pre-scales Wq,Wk by 64 and
    Wv by 16 (powers of 2, folded back via exp scale 2^-15 / Wp scale).
  - scores: bf16 kT.T @ qT per 128-row tk chunk; chunk PAIRS share one
    [128,1024] PSUM tile (2 banks) so exp runs as one ACT op per pair.
    Diagonal chunks compute their dead region too so exp never reads
    stale PSUM (race-free; CoreSim conflict-checker clean).
  - exp -> fp8e4 "et" tiles [128, 2, N]; causal triangle masked by DVE
    multiplies; fully-dead subtiles skip the second DR k-tile instead of
    being zeroed.
  - attnV: o-layout fp8 DoubleRow o[tq64, d64] += et.T @ v over chunk
    pairs; softmax Z accumulates as single-tile matmul columns in the
    SAME po bank at partitions 64:127 (base-64 is legal for non-DR).
    Normalize is per-partition: DVE reciprocal [64,8] + broadcast mult.
  - o -> oT via PE transposes (identity matmul) packed 2 heads/bank,
    then bf16 output projection, DVE copy, DMA out.
  - j=0 (first 512 tokens) runs attention in bf16 (v, q/k for tokens
    0:128 recomputed from bf16 x/W): short-support softmax rows cannot
    average away fp8 quantization noise. Processed last so its extra
    DMAs/projections never gate the critical path.
  - Emission is software-pipelined: attnV/Z trail scores by one pair and
    background work (projections, transposes, output tiles) drains from
    a tagged queue with just-in-time forced ordering.
"""
import sys
import numpy as np

sys.path.insert(0, "/opt/trn_rl_repo")

import ml_dtypes
import concourse.bass as bass
import concourse.bacc as bacc
import concourse.mybir as mybir
from concourse import tile
from concourse.bass_utils import run_bass_kernel_spmd

B, T, E, H, D = 4, 2048, 1024, 16, 64
HL = H // 2          # 8 local heads per core
NJ = T // 512        # 4 tq tiles
NPE = E // 256       # 4 E-chunk-pairs
NPAIR = T // 256     # 8 tk chunk pairs
BF16 = mybir.dt.bfloat16
F32 = mybir.dt.float32
FP8 = mybir.dt.float8e4
DR = mybir.MatmulPerfMode.DoubleRow
Exp = mybir.ActivationFunctionType.Exp
f8np = ml_dtypes.float8_e4m3
bfnp = ml_dtypes.bfloat16

SW = 64.0   # Wq/Wk host prescale (exp scale folds 1/SW^2)
SV = 16.0   # Wv host prescale (Wp folds 1/SV)
EXP_SCALE = 0.125 / (SW * SW)  # 2^-15 exactly

_CACHE = {}


def _build():
    nc = bacc.Bacc("TRN2", target_bir_lowering=False)
    xT8 = nc.declare_dram_parameter("xT8", [E, T], FP8, isOutput=False)
    wq8 = nc.declare_dram_parameter("wq8", [E, HL * D], FP8, isOutput=False)
    wk8 = nc.declare_dram_parameter("wk8", [E, HL * D], FP8, isOutput=False)
    wv8 = nc.declare_dram_parameter("wv8", [E, HL * D], FP8, isOutput=False)
    wpT = nc.declare_dram_parameter("wpT", [HL * D, E], BF16, isOutput=False)
    tri2 = nc.declare_dram_parameter("tri2", [128, 256], FP8, isOutput=False)
    tri2b = nc.declare_dram_parameter("tri2b", [128, 256], BF16, isOutput=False)
    xbT = nc.declare_dram_parameter("xbT", [E, 512], BF16, isOutput=False)
    wqb = nc.declare_dram_parameter("wqb", [E, HL * D], BF16, isOutput=False)
    wkb = nc.declare_dram_parameter("wkb", [E, HL * D], BF16, isOutput=False)
    wvb = nc.declare_dram_parameter("wvb", [E, HL * D], BF16, isOutput=False)
    ident = nc.declare_dram_parameter("ident", [64, 64], BF16, isOutput=False)
    out = nc.declare_dram_parameter("out", [T, E], F32, isOutput=True)

    with tile.TileContext(nc) as tc:
        with (
            tc.tile_pool(name="pp", bufs=1) as pp,
            tc.tile_pool(name="etp", bufs=14) as etp,
            tc.tile_pool(name="osb", bufs=10) as osb,
            tc.tile_pool(name="rzp", bufs=8) as rzp,
            tc.tile_pool(name="otp", bufs=3) as otp,
            tc.tile_pool(name="obp", bufs=6) as obp,
            tc.tile_pool(name="sT", bufs=2, space=bass.MemorySpace.PSUM) as ps_sT,
            tc.tile_pool(name="po", bufs=2, space=bass.MemorySpace.PSUM) as ps_po,
            tc.tile_pool(name="mm", bufs=2, space=bass.MemorySpace.PSUM) as ps_mm,
        ):
            # ---- persistent SBUF tiles
            x8a = pp.tile([128, NPE, 2, T], FP8, tag="x8", name="x8a")
            wq8a = pp.tile([128, NPE, 2, 512], FP8, tag="wq", name="wq8a")
            wk8a = pp.tile([128, NPE, 2, 512], FP8, tag="wk", name="wk8a")
            wv8a = pp.tile([128, NPE, 2, 512], FP8, tag="wv", name="wv8a")
            x8t = [x8a[:, p] for p in range(NPE)]
            wq8t = [wq8a[:, p] for p in range(NPE)]
            wk8t = [wk8a[:, p] for p in range(NPE)]
            wv8t = [wv8a[:, p] for p in range(NPE)]
            wpa = pp.tile([128, 4, E], BF16, tag="wp", name="wpa")
            wpt = [wpa[:, p] for p in range(4)]
            trit = pp.tile([128, 2, 128], FP8, tag="tri", name="trit")
            tritb = pp.tile([128, 2, 128], BF16, tag="trib", name="tritb")
            idt = pp.tile([64, 64], BF16, tag="id", name="idt")
            qTt = [[pp.tile([128, 512], BF16, tag=f"q{p}_{j}", name=f"q{p}_{j}")
                    for j in range(NJ)] for p in range(4)]
            kTt = [[pp.tile([128, 512], BF16, tag=f"k{p}_{j}", name=f"k{p}_{j}")
                    for j in range(NJ)] for p in range(4)]
            vp = [pp.tile([128, 2, HL, 65], FP8, tag=f"v{q}", name=f"v{q}")
                  for q in range(NPAIR)]
            # bf16 v for chunks 0..3: j=0 attention runs in bf16 (short-support
            # softmax rows can't average away fp8 quantization noise)
            vb = [pp.tile([128, 2, HL, 65], BF16, tag=f"vb{q}", name=f"vb{q}")
                  for q in range(2)]
            # bf16 x/W and q/k for the first 128 tokens (block-0 scores)
            xba = pp.tile([128, NPE, 2, 512], BF16, tag="xba", name="xba")
            wqba = pp.tile([128, NPE, 2, 512], BF16, tag="wqa", name="wqba")
            wkba = pp.tile([128, NPE, 2, 512], BF16, tag="wka", name="wkba")
            wvba = pp.tile([128, NPE, 2, 512], BF16, tag="wva", name="wvba")
            xbt = [xba[:, p] for p in range(NPE)]
            wqbt = [wqba[:, p] for p in range(NPE)]
            wkbt = [wkba[:, p] for p in range(NPE)]
            wvbt = [wvba[:, p] for p in range(NPE)]
            qbt = [pp.tile([128, 128], BF16, tag=f"qb{p}", name=f"qb{p}")
                   for p in range(4)]
            kbt = [pp.tile([128, 128], BF16, tag=f"kb{p}", name=f"kb{p}")
                   for p in range(4)]

            # ---- DMA in. j tiles are processed in order [1,2,3,0], so the
            # critical path is x cols [0:1024] + wv (SP) and wq/wk (ACT).
            # The bf16 sidecar tiles (j=0 accuracy path) ride at the SP tail.
            def cdma(queue, dst, src, cols=None):
                # whole class in one DMA: src rows (pe t p) -> [p, pe, t, n]
                s = src if cols is None else src[:, cols[0]:cols[1]]
                queue.dma_start(dst, s.rearrange("(pe t p) n -> p pe t n",
                                                 pe=NPE, t=2))

            def tdma(queue, dst, src, cols=None):
                # both t-halves in one DMA: src rows (t p) -> dst [p, t, n]
                s = src if cols is None else src[:, cols[0]:cols[1]]
                queue.dma_start(dst, s.rearrange("(t p) n -> p t n", t=2))
            cdma(nc.sync, x8a[:, :, :, 0:512], xT8, (0, 512))
            cdma(nc.sync, wk8a[:], wk8)
            cdma(nc.sync, wq8a[:], wq8)
            cdma(nc.sync, x8a[:, :, :, 512:1024], xT8, (512, 1024))
            cdma(nc.sync, wv8a[:], wv8)
            nc.sync.dma_start(trit[:], tri2[:].rearrange("p (a n) -> p a n", a=2))
            nc.sync.dma_start(idt[:], ident[:])
            for jj in range(2, NJ):
                cdma(nc.sync, x8a[:, :, :, 512 * jj:512 * jj + 512], xT8,
                     (512 * jj, 512 * jj + 512))
            nc.sync.dma_start(
                wpa[:], wpT[:].rearrange("(q p) n -> p q n", q=4))
            nc.sync.dma_start(tritb[:],
                              tri2b[:].rearrange("p (a n) -> p a n", a=2))
            cdma(nc.sync, xba[:], xbT)
            cdma(nc.sync, wvba[:], wvb)
            cdma(nc.sync, wqba[:], wqb)
            cdma(nc.sync, wkba[:], wkb)
            for q in range(NPAIR):
                nc.gpsimd.memset(vp[q][:, :, :, 64:65], 1.0)
            for q in range(2):
                nc.gpsimd.memset(vb[q][:, :, :, 64:65], 1.0)

            # ---- background task machinery (qkv groups, proj tiles)
            def qk_group(kind, h, j, pool=None):
                def emit():
                    wt = wq8t if kind == "q" else wk8t
                    dst = qTt if kind == "q" else kTt
                    if pool is None:
                        m = ps_mm.tile([128, 512], F32, tag="mm", name="mmq")
                    else:
                        m = pool.tile([128, 1024], F32, tag="sT",
                                      name="mmq")[:, 0:512]
                    for p in range(NPE):
                        nc.tensor.matmul(
                            m[0:64, :], wt[p][:, :, 64 * h:64 * h + 64],
                            x8t[p][:, :, 512 * j:512 * j + 512],
                            start=(p == 0), stop=(p == NPE - 1), perf_mode=DR)
                    nc.vector.tensor_copy(
                        dst[h // 2][j][64 * (h % 2):64 * (h % 2) + 64, :],
                        m[0:64, :])
                return emit

            def qkb_group(pe_hp):  # bf16 q/k for tokens 0..127, head pair
                def emit():
                    for wt, dst in ((wqbt, qbt), (wkbt, kbt)):
                        m = ps_mm.tile([128, 512], F32, tag="mm", name="mmb")
                        for p in range(NPE):
                            for t in range(2):
                                nc.tensor.matmul(
                                    m[0:128, 0:128],
                                    wt[p][:, t, 128 * pe_hp:128 * pe_hp + 128],
                                    xbt[p][:, t, 0:128],
                                    start=(p == 0 and t == 0),
                                    stop=(p == NPE - 1 and t == 1))
                        nc.vector.tensor_copy(dst[pe_hp][:], m[0:128, 0:128])
                return emit

            def v_group(c, g, pool=None):  # chunk c, t-64 sub g
                def emit():
                    if pool is None:
                        m = ps_mm.tile([128, 512], F32, tag="mm", name="mmv")
                    else:
                        m = pool.tile([128, 1024], F32, tag="sT",
                                      name="mmv")[:, 0:512]
                    t0 = 128 * c + 64 * g
                    for p in range(NPE):
                        nc.tensor.matmul(
                            m[0:64, :], x8t[p][:, :, t0:t0 + 64], wv8t[p][:],
                            start=(p == 0), stop=(p == NPE - 1), perf_mode=DR)
                    nc.vector.tensor_copy(
                        vp[c // 2][64 * g:64 * g + 64, c % 2, :, 0:64],
                        m[0:64, :].rearrange("p (h d) -> p h d", d=64))
                return emit

            def vb_group(c, g):  # true bf16 v for j=0 (bf16 x and Wv)
                def emit():
                    m = ps_mm.tile([128, 512], F32, tag="mm", name="mvb")
                    t0 = 128 * c + 64 * g
                    for p in range(NPE):
                        for t in range(2):
                            nc.tensor.matmul(
                                m[0:64, :],
                                xbt[p][:, t, t0:t0 + 64],
                                wvbt[p][:, t, :],
                                start=(p == 0 and t == 0),
                                stop=(p == NPE - 1 and t == 1))
                    nc.vector.tensor_copy(
                        vb[c // 2][64 * g:64 * g + 64, c % 2, :, 0:64],
                        m[0:64, :].rearrange("p (h d) -> p h d", d=64))
                return emit

            def proj_tile(j, t, nh, ot_tile):
                def emit():
                    m = ps_mm.tile([128, 512], F32, tag="mm", name="mmp")
                    for p in range(4):
                        nc.tensor.matmul(
                            m[:], ot_tile[:, p, 128 * t:128 * t + 128],
                            wpt[p][:, 512 * nh:512 * nh + 512],
                            start=(p == 0), stop=(p == 3))
                    ob = obp.tile([128, 512], F32, tag="ob", name="ob")
                    nc.vector.tensor_copy(ob[:], m[:])
                    nc.sync.dma_start(
                        out[512 * j + 128 * t:512 * j + 128 * t + 128,
                            512 * nh:512 * nh + 512], ob[:])
                return emit

            bg = []  # queue of (tag, emitter); tags order forced at stream starts

            def drain_bg(n=1):
                for _ in range(min(n, len(bg))):
                    bg.pop(0)[1]()

            def force_bg(pred):
                """Emit from the front until no queued task satisfies pred."""
                while any(pred(t) for t, _ in bg):
                    bg.pop(0)[1]()

            # j tiles processed [1, 2, 3, 0]: the bf16 j=0 accuracy path runs
            # last so its DMAs/projections never gate the critical path.
            JORDER = (1, 2, 3, 0)
            # prologue: v pairs 0..1 (chunks 0..3); q tile 1, k tiles 0..1
            # for h=0. k projections are scheduled by tk-tile index: stream
            # (j, h) consumes k tiles 0..j.
            # interleave so the first stream's scores are unblocked ASAP
            qk_group("k", 0, 0)()
            qk_group("k", 0, 1, ps_sT)()
            qk_group("q", 0, 1)()
            v_group(0, 0, ps_sT)()
            v_group(0, 1)()
            v_group(1, 0, ps_sT)()
            v_group(1, 1)()
            for c in (2, 3):
                for g in range(2):
                    bg.append((("vq", 1), v_group(c, g)))
            for q in range(2, 4):
                for g in range(2):
                    bg.append((("vq", q), v_group(2 * q, g)))
                    bg.append((("vq", q), v_group(2 * q + 1, g)))
            for h in range(1, HL):
                bg.append((("q", 1, h), qk_group("q", h, 1)))
                bg.append((("k", 0, h), qk_group("k", h, 0)))
                bg.append((("k", 1, h), qk_group("k", h, 1)))
            # bf16 sidecar tasks wait on tail-end DMAs; keep them out of the
            # main drain queue until their inputs have surely landed
            bg_late = []
            for hp in range(4):
                bg_late.append((("qkb", 2 * hp), qkb_group(hp)))
            for c in range(4):
                for g in range(2):
                    bg_late.append((("vb", c // 2), vb_group(c, g)))

            ot_tiles = {}
            pend = []       # one-pair-lagged PE emissions (attnV/Z, normalize)
            o_sb_pair = {}  # normalized o for the in-flight head pair

            pending_proj = []
            for si, j in enumerate(JORDER):
                if si == 1:  # release the bf16 sidecar work mid-flight
                    bg.extend(bg_late)
                    bg_late = []
                # enqueue the next sequence step's inputs
                if si + 1 < NJ:
                    nj = JORDER[si + 1]
                    if nj != 0:
                        for q in range(2 * nj, 2 * nj + 2):
                            for g in range(2):
                                bg.append((("vq", q), v_group(2 * q, g)))
                                bg.append((("vq", q), v_group(2 * q + 1, g)))
                    for h in range(HL):
                        bg.append((("q", nj, h), qk_group("q", h, nj)))
                        if nj != 0:  # k tile 0 was produced in the prologue
                            bg.append((("k", nj, h), qk_group("k", h, nj)))
                # weave last step's proj tiles into the fresh queue (avoids a
                # PE-only burst that starves ACT at the step boundary); all
                # transposes must go first (proj reads their output)
                if pending_proj:
                    force_bg(lambda t: t[0] == "tr")
                for i, task in enumerate(pending_proj):
                    bg.insert(min(5 * i + 2, len(bg)), task)
                pending_proj = []
                # proj from two sequence steps back must be out before its
                # ot slot is reused
                force_bg(lambda t: t[0] == "proj" and t[1] <= si - 2)
                ot_tile = otp.tile([128, 4, 512], BF16, tag="ot", name=f"ot{j}")
                ot_tiles[j] = ot_tile

                for h in range(HL):
                    hp, r0 = h // 2, 64 * (h % 2)
                    zc = 8 * (8 * j + h)
                    # everything this stream reads must already be emitted
                    force_bg(lambda t: (t[0] == "q" and t[1] == j and t[2] <= h) or
                             (t[0] == "k" and t[1] <= j and t[2] <= h) or
                             (j == 0 and t[0] == "qkb" and t[1] <= h))
                    po_t = ps_po.tile([128, 512], F32, tag="po", name="po")
                    po = po_t[0:64, :]
                    nq = 2 * j + 2
                    for q in range(nq):
                        m0, m2 = (q == 2 * j), (q == 2 * j + 1)
                        force_bg(lambda t: (t[0] == "vq" and t[1] <= q) or
                                 (j == 0 and t[0] == "vb" and t[1] <= q))
                        sT = ps_sT.tile([128, 1024], F32, tag="sT", name="sT")
                        if j == 0:
                            et = etp.tile([128, 1024], BF16, tag="etb",
                                          name="etb", bufs=4)
                            trm = tritb
                        else:
                            et = etp.tile([128, 1024], FP8, tag="et", name="et")
                            trm = trit
                        if not (m0 or m2):
                            # off-diag pair: both chunks full [0:512]
                            for t in range(2):
                                c = 2 * q + t
                                nc.tensor.matmul(
                                    sT[:, 512 * t:512 * t + 512],
                                    kTt[hp][c // 4][r0:r0 + 64,
                                                    128 * (c % 4):128 * (c % 4) + 128],
                                    qTt[hp][j][r0:r0 + 64, :],
                                    start=True, stop=True, skip_group_check=True)
                            nc.scalar.activation(et[:], sT[:], Exp, scale=EXP_SCALE)
                            stride, width = 512, 512
                        elif m0:
                            # chunks 4j (full), 4j+1 (cols 128:512)
                            c = 4 * j
                            if j == 0:
                                # block-0 scores from bf16-accurate q/k
                                nc.tensor.matmul(
                                    sT[:, 0:128],
                                    kbt[hp][r0:r0 + 64, :],
                                    qbt[hp][r0:r0 + 64, :],
                                    start=True, stop=False,
                                    skip_group_check=True)
                                nc.tensor.matmul(
                                    sT[:, 128:512],
                                    kTt[hp][j][r0:r0 + 64, 0:128],
                                    qTt[hp][j][r0:r0 + 64, 128:512],
                                    start=False, stop=True,
                                    skip_group_check=True)
                            else:
                                nc.tensor.matmul(
                                    sT[:, 0:512],
                                    kTt[hp][j][r0:r0 + 64, 0:128],
                                    qTt[hp][j][r0:r0 + 64, :],
                                    start=True, stop=True,
                                    skip_group_check=True)
                            # cover [512:640] too so exp never reads stale
                            # bytes (those weights get memset to 0 after)
                            nc.tensor.matmul(
                                sT[:, 512:1024],
                                kTt[hp][j][r0:r0 + 64, 128:256],
                                qTt[hp][j][r0:r0 + 64, :],
                                start=True, stop=True, skip_group_check=True)
                            nc.scalar.activation(et[:], sT[:], Exp, scale=EXP_SCALE)
                            et3 = et[:].rearrange("p (a n) -> p a n", n=128)
                            nc.vector.tensor_mul(et3[:, 0:6:5, :], et3[:, 0:6:5, :],
                                                 trm[:])
                            stride, width = 512, 512
                        else:
                            # m2: chunks 4j+2 (cols 256:512 -> [0:256]),
                            #     4j+3 (cols 384:512 -> [384:512])
                            nc.tensor.matmul(
                                sT[:, 0:256],
                                kTt[hp][j][r0:r0 + 64, 256:384],
                                qTt[hp][j][r0:r0 + 64, 256:512],
                                start=True, stop=True, skip_group_check=True)
                            nc.tensor.matmul(
                                sT[:, 256:512],
                                kTt[hp][j][r0:r0 + 64, 384:512],
                                qTt[hp][j][r0:r0 + 64, 256:512],
                                start=False, stop=True, skip_group_check=True)
                            nc.scalar.activation(et[:, 0:512], sT[:, 0:512],
                                                 Exp, scale=EXP_SCALE)
                            et3 = et[:].rearrange("p (a n) -> p a n", n=128)
                            nc.vector.tensor_mul(et3[:, 0:4:3, :], et3[:, 0:4:3, :],
                                                 trm[:])
                            stride, width = 256, 256
                        etv = et[:, 0:2 * stride].rearrange(
                            "p (a n) -> p a n", a=2)
                        s_lo = 4 if m2 else 0
                        base = 256 if m2 else 0

                        def attn_emit(et=et, etv=etv, q=q, h=h, po=po,
                                      po_t=po_t, zc=zc,
                                      s_lo=s_lo, base=base, m0=m0, m2=m2, j=j):
                            # exactly ONE start=True per bank-use: start=True
                            # flags the whole 2KB bank pending-zero; every
                            # other group's first touch consumes its flag.
                            if j == 0:
                                # bf16 non-DR path (no fp8 noise on the short-
                                # support rows). (tile, subtiles, flat offset fn)
                                if m0:
                                    work = [(0, range(0, 8), lambda s: 64 * s),
                                            (1, range(2, 8),
                                             lambda s: 512 + 64 * s)]
                                else:
                                    work = [(0, range(4, 8),
                                             lambda s: 64 * s - 256),
                                            (1, range(6, 8), lambda s: 64 * s)]
                                stop_at = {0: (0, 0), 1: (0, 0), 2: (0, 1),
                                           3: (0, 1), 4: (1, 0), 5: (1, 0),
                                           6: (1, 1), 7: (1, 1)}
                                for tt, srange, off in work:
                                    for s in srange:
                                        fo = off(s)
                                        stop = stop_at[s] == (q, tt)
                                        st = (q == 0 and tt == 0 and s == 0)
                                        nc.tensor.matmul(
                                            po[:, 64 * s:64 * s + 64],
                                            et[:, fo:fo + 64],
                                            vb[q][:, tt, h, 0:64],
                                            start=st, stop=stop,
                                            skip_group_check=True)
                                        nc.tensor.matmul(
                                            po_t[64:128, s:s + 1],
                                            et[:, fo:fo + 64],
                                            vb[q][:, tt, h, 64:65],
                                            start=st, stop=stop,
                                            skip_group_check=True)
                                return
                            for s in range(s_lo, 8):
                                cc = 64 * s - base
                                stop = (s < 4 and m0) or (s >= 4 and m2)
                                # tile1's dead block is never read: subtiles
                                # under the diagonal use a single-tile matmul
                                single = (m0 and s < 2) or (m2 and s < 6)
                                if single:
                                    lhs_o = etv[:, 0, cc:cc + 64]
                                    rhs_o = vp[q][:, 0, h, 0:64]
                                    pm = None
                                else:
                                    lhs_o = etv[:, :, cc:cc + 64]
                                    rhs_o = vp[q][:, :, h, 0:64]
                                    pm = DR
                                nc.tensor.matmul(
                                    po[:, 64 * s:64 * s + 64],
                                    lhs_o, rhs_o,
                                    start=(q == 0 and s == 0), stop=stop,
                                    perf_mode=pm, skip_group_check=True)
                                # Z columns live at partitions 64:128 of the
                                # same po bank (single-tile, non-DR: base-64)
                                nc.tensor.matmul(
                                    po_t[64:128, s:s + 1],
                                    etv[:, 0, cc:cc + 64],
                                    vp[q][:, 0, h, 64:65],
                                    start=(q == 0 and s == 0), stop=(stop and single),
                                    skip_group_check=True)
                                if not single:
                                    nc.tensor.matmul(
                                        po_t[64:128, s:s + 1],
                                        etv[:, 1, cc:cc + 64],
                                        vp[q][:, 1, h, 64:65],
                                        start=False, stop=stop,
                                        skip_group_check=True)
                        # one-pair software pipeline: previous pair's attnV/Z
                        # runs while this pair's exp/masks are in flight
                        while pend:
                            pend.pop(0)()
                        pend.append(attn_emit)
                        drain_bg(2 if h == HL - 1 else (2 if len(bg) > 40 else 1))

                    force_bg(lambda t: (t[0] == "vq" and t[1] <= 2 * j + 1) or
                             (j == 0 and t[0] == "vb"))

                    def norm_emit(po=po, po_t=po_t, h=h, hp=hp,
                                  ot_tile=ot_tile):
                        rz = rzp.tile([64, 8], F32, tag="rz", name="rz")
                        nc.vector.reciprocal(rz[:], po_t[64:128, 0:8])
                        o_sb = osb.tile([64, 512], BF16, tag="os", name="os")
                        nc.vector.tensor_tensor(
                            o_sb[:].rearrange("p (e s) -> p e s", e=8),
                            po[:].rearrange("p (e s) -> p e s", e=8),
                            rz[:].unsqueeze(2).broadcast_to([64, 8, 64]),
                            mybir.AluOpType.mult)
                        o_sb_pair[h % 2] = o_sb
                        if h % 2 == 1:
                            def transpose_emit(hp=hp, ot_tile=ot_tile,
                                               pair=dict(o_sb_pair)):
                                pt_t = ps_po.tile([128, 512], F32, tag="po",
                                                  name="pt")
                                pt = pt_t[:].bitcast(BF16)[:, 0:512]
                                for hh in range(2):
                                    for s in range(8):
                                        nc.tensor.matmul(
                                            pt[64 * hh:64 * hh + 64,
                                               64 * s:64 * s + 64],
                                            pair[hh][:, 64 * s:64 * s + 64],
                                            idt[:], is_transpose=True,
                                            start=(s == 0),
                                            stop=(hh == 1 and s == 7),
                                            skip_group_check=True)
                                nc.vector.tensor_copy(ot_tile[:, hp, :], pt[:])
                            bg.insert(min(1, len(bg)),
                                      (("tr", None), transpose_emit))
                    pend.append(norm_emit)

                # flush the pipeline at the j boundary so the last head pair's
                # normalize + transpose are queued before proj tasks
                while pend:
                    pend.pop(0)()
                for t in range(4):
                    for nh in range(2):
                        pending_proj.append((("proj", si),
                                             proj_tile(j, t, nh, ot_tile)))

            while pend:
                pend.pop(0)()
            drain_bg(len(bg))
            for _, task in pending_proj:
                task()

    nc.compile()
    return nc


def _host_prep(x, Wq, Wk, Wv, Wp):
    """Per-core input maps."""
    tri = (np.arange(128)[None, :] >= np.arange(128)[:, None]).astype(np.float32)
    tri2 = np.concatenate([tri, tri], axis=1).astype(f8np)
    ident = np.eye(64, dtype=np.float32).astype(bfnp)
    WpT = np.ascontiguousarray(Wp.T) * (1.0 / SV)  # [E(hd), E]

    def wslice(W, h0, scale, dt=f8np):  # [H,E,D] -> [E, 8*64]
        w = W[h0:h0 + HL].transpose(1, 0, 2).reshape(E, HL * D) * scale
        return np.ascontiguousarray(w).astype(dt)

    in_maps = []
    for c in range(8):
        b, hh = c // 2, c % 2
        h0 = hh * HL
        in_maps.append({
            "xT8": np.ascontiguousarray(x[b].T).astype(f8np),
            "wq8": wslice(Wq, h0, SW),
            "wk8": wslice(Wk, h0, SW),
            "wv8": wslice(Wv, h0, SV),
            "wpT": np.ascontiguousarray(
                WpT[h0 * D:(h0 + HL) * D, :]).astype(bfnp),
            "tri2": tri2,
            "tri2b": tri2.astype(np.float32).astype(bfnp),
            "ident": ident,
            "xbT": np.ascontiguousarray(x[b].T[:, 0:512]).astype(bfnp),
            "wqb": wslice(Wq, h0, SW, bfnp),
            "wkb": wslice(Wk, h0, SW, bfnp),
            "wvb": wslice(Wv, h0, SV, bfnp),
        })
    return in_maps


def kernel(x, Wq, Wk, Wv, Wp, bp):
    x = np.asarray(x, dtype=np.float32)
    Wq = np.asarray(Wq, dtype=np.float32)
    Wk = np.asarray(Wk, dtype=np.float32)
    Wv = np.asarray(Wv, dtype=np.float32)
    Wp = np.asarray(Wp, dtype=np.float32)
    bp = np.asarray(bp, dtype=np.float32)

    if "nc" not in _CACHE:
        _CACHE["nc"] = _build()
    nc = _CACHE["nc"]

    in_maps = _host_prep(x, Wq, Wk, Wv, Wp)
    res = run_bass_kernel_spmd(nc, in_maps, list(range(8)))
    parts = [np.asarray(res.results[c]["out"], dtype=np.float32) for c in range(8)]
    out = np.stack([parts[2 * b] + parts[2 * b + 1] for b in range(B)], axis=0)
    return (out + bp[None, None, :]).astype(np.float32)


# revision 117
# speedup vs baseline: 1.3748x; 1.0227x over previous
"""Multi-head causal attention (B=4,T=2048,E=1024,H=16,D=64) on 8 TRN2 cores.

Sharding: core c -> batch b=c//2, heads h0=(c%2)*8 .. h0+8. Each core computes
its 8 heads' attention and a partial output projection (row-split Wp); host
sums the two partials per batch (+bias).

Per-core kernel (tq tiles processed in order 1,2,3,0):
  - Q/K/V projections in fp8e4 DoubleRow (2 k-tiles of 128 per instr, 0.5
    cyc/col), M=64 outputs at PSUM base 0. Host pre-scales Wq,Wk by 64 and
    Wv by 16 (powers of 2, folded back via exp scale 2^-15 / Wp scale).
  - scores: bf16 kT.T @ qT per 128-row tk chunk; chunk PAIRS share one
    [128,1024] PSUM tile (2 banks) so exp runs as one ACT op per pair.
    Diagonal chunks compute their dead region too so exp never reads
    stale PSUM (race-free; CoreSim conflict-checker clean).
  - exp -> fp8e4 "et" tiles [128, 2, N]; causal triangle masked by DVE
    multiplies; fully-dead subtiles skip the second DR k-tile instead of
    being zeroed.
  - attnV: o-layout fp8 DoubleRow o[tq64, d64] += et.T @ v over chunk
    pairs; softmax Z accumulates as single-tile matmul columns in the
    SAME po bank at partitions 64:127 (base-64 is legal for non-DR).
    Normalize is per-partition: DVE reciprocal [64,8] + broadcast mult.
  - o -> oT via PE transposes (identity matmul) packed 2 heads/bank,
    then bf16 output projection, DVE copy, DMA out.
  - j=0 (first 512 tokens) runs attention in bf16 (v, q/k for tokens
    0:128 recomputed from bf16 x/W): short-support softmax rows cannot
    average away fp8 quantization noise. Processed last so its extra
    DMAs/projections never gate the critical path.
  - Emission is software-pipelined: attnV/Z trail scores by one pair and
    background work (projections, transposes, output tiles) drains from
    a tagged queue with just-in-time forced ordering.
"""
import sys
import numpy as np

sys.path.insert(0, "/opt/trn_rl_repo")

import ml_dtypes
import concourse.bass as bass
import concourse.bacc as bacc
import concourse.mybir as mybir
from concourse import tile
from concourse.bass_utils import run_bass_kernel_spmd

B, T, E, H, D = 4, 2048, 1024, 16, 64
HL = H // 2          # 8 local heads per core
NJ = T // 512        # 4 tq tiles
NPE = E // 256       # 4 E-chunk-pairs
NPAIR = T // 256     # 8 tk chunk pairs
BF16 = mybir.dt.bfloat16
F32 = mybir.dt.float32
FP8 = mybir.dt.float8e4
DR = mybir.MatmulPerfMode.DoubleRow
Exp = mybir.ActivationFunctionType.Exp
f8np = ml_dtypes.float8_e4m3
bfnp = ml_dtypes.bfloat16

SW = 64.0   # Wq/Wk host prescale (exp scale folds 1/SW^2)
SV = 16.0   # Wv host prescale (Wp folds 1/SV)
EXP_SCALE = 0.125 / (SW * SW)  # 2^-15 exactly

_CACHE = {}


def _build():
    nc = bacc.Bacc("TRN2", target_bir_lowering=False)
    xT8 = nc.declare_dram_parameter("xT8", [E, T], FP8, isOutput=False)
    wq8 = nc.declare_dram_parameter("wq8", [E, HL * D], FP8, isOutput=False)
    wk8 = nc.declare_dram_parameter("wk8", [E, HL * D], FP8, isOutput=False)
    wv8 = nc.declare_dram_parameter("wv8", [E, HL * D], FP8, isOutput=False)
    wpT = nc.declare_dram_parameter("wpT", [HL * D, E], BF16, isOutput=False)
    tri2 = nc.declare_dram_parameter("tri2", [128, 256], FP8, isOutput=False)
    tri2b = nc.declare_dram_parameter("tri2b", [128, 256], BF16, isOutput=False)
    xbT = nc.declare_dram_parameter("xbT", [E, 512], BF16, isOutput=False)
    wqb = nc.declare_dram_parameter("wqb", [E, HL * D], BF16, isOutput=False)
    wkb = nc.declare_dram_parameter("wkb", [E, HL * D], BF16, isOutput=False)
    wvb = nc.declare_dram_parameter("wvb", [E, HL * D], BF16, isOutput=False)
    ident = nc.declare_dram_parameter("ident", [64, 64], BF16, isOutput=False)
    out = nc.declare_dram_parameter("out", [T, E], F32, isOutput=True)

    with tile.TileContext(nc) as tc:
        with (
            tc.tile_pool(name="pp", bufs=1) as pp,
            tc.tile_pool(name="etp", bufs=14) as etp,
            tc.tile_pool(name="osb", bufs=10) as osb,
            tc.tile_pool(name="rzp", bufs=8) as rzp,
            tc.tile_pool(name="otp", bufs=3) as otp,
            tc.tile_pool(name="obp", bufs=6) as obp,
            tc.tile_pool(name="sT", bufs=2, space=bass.MemorySpace.PSUM) as ps_sT,
            tc.tile_pool(name="po", bufs=2, space=bass.MemorySpace.PSUM) as ps_po,
            tc.tile_pool(name="mm", bufs=2, space=bass.MemorySpace.PSUM) as ps_mm,
        ):
            # ---- persistent SBUF tiles
            x8a = pp.tile([128, NPE, 2, T], FP8, tag="x8", name="x8a")
            wq8a = pp.tile([128, NPE, 2, 512], FP8, tag="wq", name="wq8a")
            wk8a = pp.tile([128, NPE, 2, 512], FP8, tag="wk", name="wk8a")
            wv8a = pp.tile([128, NPE, 2, 512], FP8, tag="wv", name="wv8a")
            x8t = [x8a[:, p] for p in range(NPE)]
            wq8t = [wq8a[:, p] for p in range(NPE)]
            wk8t = [wk8a[:, p] for p in range(NPE)]
            wv8t = [wv8a[:, p] for p in range(NPE)]
            wpa = pp.tile([128, 4, E], BF16, tag="wp", name="wpa")
            wpt = [wpa[:, p] for p in range(4)]
            trit = pp.tile([128, 2, 128], FP8, tag="tri", name="trit")
            tritb = pp.tile([128, 2, 128], BF16, tag="trib", name="tritb")
            idt = pp.tile([64, 64], BF16, tag="id", name="idt")
            qTt = [[pp.tile([128, 512], BF16, tag=f"q{p}_{j}", name=f"q{p}_{j}")
                    for j in range(NJ)] for p in range(4)]
            kTt = [[pp.tile([128, 512], BF16, tag=f"k{p}_{j}", name=f"k{p}_{j}")
                    for j in range(NJ)] for p in range(4)]
            vp = [pp.tile([128, 2, HL, 65], FP8, tag=f"v{q}", name=f"v{q}")
                  for q in range(NPAIR)]
            # bf16 v for chunks 0..3: j=0 attention runs in bf16 (short-support
            # softmax rows can't average away fp8 quantization noise)
            vb = [pp.tile([128, 2, HL, 65], BF16, tag=f"vb{q}", name=f"vb{q}")
                  for q in range(2)]
            # bf16 x/W and q/k for the first 128 tokens (block-0 scores)
            xba = pp.tile([128, NPE, 2, 512], BF16, tag="xba", name="xba")
            wqba = pp.tile([128, NPE, 2, 512], BF16, tag="wqa", name="wqba")
            wkba = pp.tile([128, NPE, 2, 512], BF16, tag="wka", name="wkba")
            wvba = pp.tile([128, NPE, 2, 512], BF16, tag="wva", name="wvba")
            xbt = [xba[:, p] for p in range(NPE)]
            wqbt = [wqba[:, p] for p in range(NPE)]
            wkbt = [wkba[:, p] for p in range(NPE)]
            wvbt = [wvba[:, p] for p in range(NPE)]
            qbt = [pp.tile([128, 128], BF16, tag=f"qb{p}", name=f"qb{p}")
                   for p in range(4)]
            kbt = [pp.tile([128, 128], BF16, tag=f"kb{p}", name=f"kb{p}")
                   for p in range(4)]

            # ---- DMA in. j tiles are processed in order [1,2,3,0], so the
            # critical path is x cols [0:1024] + wv (SP) and wq/wk (ACT).
            # The bf16 sidecar tiles (j=0 accuracy path) ride at the SP tail.
            def cdma(queue, dst, src, cols=None):
                # whole class in one DMA: src rows (pe t p) -> [p, pe, t, n]
                s = src if cols is None else src[:, cols[0]:cols[1]]
                queue.dma_start(dst, s.rearrange("(pe t p) n -> p pe t n",
                                                 pe=NPE, t=2))

            def tdma(queue, dst, src, cols=None):
                # both t-halves in one DMA: src rows (t p) -> dst [p, t, n]
                s = src if cols is None else src[:, cols[0]:cols[1]]
                queue.dma_start(dst, s.rearrange("(t p) n -> p t n", t=2))
            cdma(nc.sync, x8a[:, :, :, 0:512], xT8, (0, 512))
            cdma(nc.sync, wk8a[:], wk8)
            cdma(nc.sync, wq8a[:], wq8)
            cdma(nc.sync, x8a[:, :, :, 512:1024], xT8, (512, 1024))
            cdma(nc.sync, wv8a[:], wv8)
            nc.sync.dma_start(trit[:], tri2[:].rearrange("p (a n) -> p a n", a=2))
            nc.sync.dma_start(idt[:], ident[:])
            for jj in range(2, NJ):
                cdma(nc.sync, x8a[:, :, :, 512 * jj:512 * jj + 512], xT8,
                     (512 * jj, 512 * jj + 512))
            nc.sync.dma_start(
                wpa[:], wpT[:].rearrange("(q p) n -> p q n", q=4))
            nc.sync.dma_start(tritb[:],
                              tri2b[:].rearrange("p (a n) -> p a n", a=2))
            cdma(nc.sync, xba[:], xbT)
            cdma(nc.sync, wvba[:], wvb)
            cdma(nc.sync, wqba[:], wqb)
            cdma(nc.sync, wkba[:], wkb)
            for q in range(NPAIR):
                nc.gpsimd.memset(vp[q][:, :, :, 64:65], 1.0)
            for q in range(2):
                nc.gpsimd.memset(vb[q][:, :, :, 64:65], 1.0)

            # ---- background task machinery (qkv groups, proj tiles)
            def qk_group(kind, h, j, pool=None):
                def emit():
                    wt = wq8t if kind == "q" else wk8t
                    dst = qTt if kind == "q" else kTt
                    if pool is None:
                        m = ps_mm.tile([128, 512], F32, tag="mm", name="mmq")
                    else:
                        m = pool.tile([128, 1024], F32, tag="sT",
                                      name="mmq")[:, 0:512]
                    for p in range(NPE):
                        nc.tensor.matmul(
                            m[0:64, :], wt[p][:, :, 64 * h:64 * h + 64],
                            x8t[p][:, :, 512 * j:512 * j + 512],
                            start=(p == 0), stop=(p == NPE - 1), perf_mode=DR)
                    nc.vector.tensor_copy(
                        dst[h // 2][j][64 * (h % 2):64 * (h % 2) + 64, :],
                        m[0:64, :])
                return emit

            def qkb_group(pe_hp):  # bf16 q/k for tokens 0..127, head pair
                def emit():
                    for wt, dst in ((wqbt, qbt), (wkbt, kbt)):
                        m = ps_mm.tile([128, 512], F32, tag="mm", name="mmb")
                        for p in range(NPE):
                            for t in range(2):
                                nc.tensor.matmul(
                                    m[0:128, 0:128],
                                    wt[p][:, t, 128 * pe_hp:128 * pe_hp + 128],
                                    xbt[p][:, t, 0:128],
                                    start=(p == 0 and t == 0),
                                    stop=(p == NPE - 1 and t == 1))
                        nc.vector.tensor_copy(dst[pe_hp][:], m[0:128, 0:128])
                return emit

            def v_group(c, g, pool=None):  # chunk c, t-64 sub g
                def emit():
                    if pool is None:
                        m = ps_mm.tile([128, 512], F32, tag="mm", name="mmv")
                    else:
                        m = pool.tile([128, 1024], F32, tag="sT",
                                      name="mmv")[:, 0:512]
                    t0 = 128 * c + 64 * g
                    for p in range(NPE):
                        nc.tensor.matmul(
                            m[0:64, :], x8t[p][:, :, t0:t0 + 64], wv8t[p][:],
                            start=(p == 0), stop=(p == NPE - 1), perf_mode=DR)
                    nc.vector.tensor_copy(
                        vp[c // 2][64 * g:64 * g + 64, c % 2, :, 0:64],
                        m[0:64, :].rearrange("p (h d) -> p h d", d=64))
                return emit

            def vb_group(c, g):  # true bf16 v for j=0 (bf16 x and Wv)
                def emit():
                    m = ps_mm.tile([128, 512], F32, tag="mm", name="mvb")
                    t0 = 128 * c + 64 * g
                    for p in range(NPE):
                        for t in range(2):
                            nc.tensor.matmul(
                                m[0:64, :],
                                xbt[p][:, t, t0:t0 + 64],
                                wvbt[p][:, t, :],
                                start=(p == 0 and t == 0),
                                stop=(p == NPE - 1 and t == 1))
                    nc.vector.tensor_copy(
                        vb[c // 2][64 * g:64 * g + 64, c % 2, :, 0:64],
                        m[0:64, :].rearrange("p (h d) -> p h d", d=64))
                return emit

            def proj_tile(j, t, nh, ot_tile):
                def emit():
                    m = ps_mm.tile([128, 512], F32, tag="mm", name="mmp")
                    for p in range(4):
                        nc.tensor.matmul(
                            m[:], ot_tile[:, p, 128 * t:128 * t + 128],
                            wpt[p][:, 512 * nh:512 * nh + 512],
                            start=(p == 0), stop=(p == 3))
                    ob = obp.tile([128, 512], F32, tag="ob", name="ob")
                    nc.vector.tensor_copy(ob[:], m[:])
                    nc.sync.dma_start(
                        out[512 * j + 128 * t:512 * j + 128 * t + 128,
                            512 * nh:512 * nh + 512], ob[:])
                return emit

            bg = []  # queue of (tag, emitter); tags order forced at stream starts

            def drain_bg(n=1):
                for _ in range(min(n, len(bg))):
                    bg.pop(0)[1]()

            def force_bg(pred):
                """Emit from the front until no queued task satisfies pred."""
                while any(pred(t) for t, _ in bg):
                    bg.pop(0)[1]()

            # j tiles processed [1, 2, 3, 0]: the bf16 j=0 accuracy path runs
            # last so its DMAs/projections never gate the critical path.
            JORDER = (1, 2, 3, 0)
            # prologue: v pairs 0..1 (chunks 0..3); q tile 1, k tiles 0..1
            # for h=0. k projections are scheduled by tk-tile index: stream
            # (j, h) consumes k tiles 0..j.
            # interleave so the first stream's scores are unblocked ASAP
            qk_group("k", 0, 0)()
            qk_group("k", 0, 1, ps_sT)()
            qk_group("q", 0, 1)()
            v_group(0, 0, ps_sT)()
            v_group(0, 1)()
            v_group(1, 0, ps_sT)()
            v_group(1, 1)()
            for c in (2, 3):
                for g in range(2):
                    bg.append((("vq", 1), v_group(c, g)))
            for q in range(2, 4):
                for g in range(2):
                    bg.append((("vq", q), v_group(2 * q, g)))
                    bg.append((("vq", q), v_group(2 * q + 1, g)))
            for h in range(1, HL):
                bg.append((("q", 1, h), qk_group("q", h, 1)))
                bg.append((("k", 0, h), qk_group("k", h, 0)))
                bg.append((("k", 1, h), qk_group("k", h, 1)))
            # bf16 sidecar tasks wait on tail-end DMAs; keep them out of the
            # main drain queue until their inputs have surely landed
            bg_late = []
            for hp in range(4):
                bg_late.append((("qkb", 2 * hp), qkb_group(hp)))
            for c in range(4):
                for g in range(2):
                    bg_late.append((("vb", c // 2), vb_group(c, g)))

            ot_tiles = {}
            pend = []       # one-pair-lagged PE emissions (attnV/Z, normalize)
            o_sb_pair = {}  # normalized o for the in-flight head pair

            pending_proj = []
            for si, j in enumerate(JORDER):
                if si == 1:  # release the bf16 sidecar work mid-flight
                    bg.extend(bg_late)
                    bg_late = []
                # enqueue the next sequence step's inputs
                if si + 1 < NJ:
                    nj = JORDER[si + 1]
                    if nj != 0:
                        for q in range(2 * nj, 2 * nj + 2):
                            for g in range(2):
                                bg.append((("vq", q), v_group(2 * q, g)))
                                bg.append((("vq", q), v_group(2 * q + 1, g)))
                    for h in range(HL):
                        bg.append((("q", nj, h), qk_group("q", h, nj)))
                        if nj != 0:  # k tile 0 was produced in the prologue
                            bg.append((("k", nj, h), qk_group("k", h, nj)))
                # weave last step's proj tiles into the fresh queue (avoids a
                # PE-only burst that starves ACT at the step boundary); all
                # transposes must go first (proj reads their output)
                if pending_proj:
                    force_bg(lambda t: t[0] == "tr")
                for i, task in enumerate(pending_proj):
                    bg.insert(min(5 * i + 2, len(bg)), task)
                pending_proj = []
                # proj from two sequence steps back must be out before its
                # ot slot is reused
                force_bg(lambda t: t[0] == "proj" and t[1] <= si - 2)
                ot_tile = otp.tile([128, 4, 512], BF16, tag="ot", name=f"ot{j}")
                ot_tiles[j] = ot_tile

                for h in range(HL):
                    hp, r0 = h // 2, 64 * (h % 2)
                    zc = 8 * (8 * j + h)
                    # everything this stream reads must already be emitted
                    force_bg(lambda t: (t[0] == "q" and t[1] == j and t[2] <= h) or
                             (t[0] == "k" and t[1] <= j and t[2] <= h) or
                             (j == 0 and t[0] == "qkb" and t[1] <= h))
                    po_t = ps_po.tile([128, 512], F32, tag="po", name="po")
                    po = po_t[0:64, :]
                    nq = 2 * j + 2
                    for q in range(nq):
                        m0, m2 = (q == 2 * j), (q == 2 * j + 1)
                        force_bg(lambda t: (t[0] == "vq" and t[1] <= q) or
                                 (j == 0 and t[0] == "vb" and t[1] <= q))
                        sT = ps_sT.tile([128, 1024], F32, tag="sT", name="sT")
                        if j == 0:
                            et = etp.tile([128, 1024], BF16, tag="etb",
                                          name="etb", bufs=4)
                            trm = tritb
                        else:
                            et = etp.tile([128, 1024], FP8, tag="et", name="et")
                            trm = trit
                        if not (m0 or m2):
                            # off-diag pair: both chunks full [0:512]
                            for t in range(2):
                                c = 2 * q + t
                                nc.tensor.matmul(
                                    sT[:, 512 * t:512 * t + 512],
                                    kTt[hp][c // 4][r0:r0 + 64,
                                                    128 * (c % 4):128 * (c % 4) + 128],
                                    qTt[hp][j][r0:r0 + 64, :],
                                    start=True, stop=True, skip_group_check=True)
                            nc.scalar.activation(et[:], sT[:], Exp, scale=EXP_SCALE)
                            stride, width = 512, 512
                        elif m0:
                            # chunks 4j (full), 4j+1 (cols 128:512)
                            c = 4 * j
                            if j == 0:
                                # block-0 scores from bf16-accurate q/k
                                nc.tensor.matmul(
                                    sT[:, 0:128],
                                    kbt[hp][r0:r0 + 64, :],
                                    qbt[hp][r0:r0 + 64, :],
                                    start=True, stop=False,
                                    skip_group_check=True)
                                nc.tensor.matmul(
                                    sT[:, 128:512],
                                    kTt[hp][j][r0:r0 + 64, 0:128],
                                    qTt[hp][j][r0:r0 + 64, 128:512],
                                    start=False, stop=True,
                                    skip_group_check=True)
                            else:
                                nc.tensor.matmul(
                                    sT[:, 0:512],
                                    kTt[hp][j][r0:r0 + 64, 0:128],
                                    qTt[hp][j][r0:r0 + 64, :],
                                    start=True, stop=True,
                                    skip_group_check=True)
                            # cover [512:640] too so exp never reads stale
                            # bytes (those weights get memset to 0 after)
                            nc.tensor.matmul(
                                sT[:, 512:1024],
                                kTt[hp][j][r0:r0 + 64, 128:256],
                                qTt[hp][j][r0:r0 + 64, :],
                                start=True, stop=True, skip_group_check=True)
                            nc.scalar.activation(et[:], sT[:], Exp, scale=EXP_SCALE)
                            et3 = et[:].rearrange("p (a n) -> p a n", n=128)
                            nc.vector.tensor_mul(et3[:, 0:6:5, :], et3[:, 0:6:5, :],
                                                 trm[:])
                            stride, width = 512, 512
                        else:
                            # m2: chunks 4j+2 (cols 256:512 -> [0:256]),
                            #     4j+3 (cols 384:512 -> [384:512])
                            nc.tensor.matmul(
                                sT[:, 0:256],
                                kTt[hp][j][r0:r0 + 64, 256:384],
                                qTt[hp][j][r0:r0 + 64, 256:512],
                                start=True, stop=True, skip_group_check=True)
                            nc.tensor.matmul(
                                sT[:, 256:512],
                                kTt[hp][j][r0:r0 + 64, 384:512],
                                qTt[hp][j][r0:r0 + 64, 256:512],
                                start=False, stop=True, skip_group_check=True)
                            nc.scalar.activation(et[:, 0:512], sT[:, 0:512],
                                                 Exp, scale=EXP_SCALE)
                            et3 = et[:].rearrange("p (a n) -> p a n", n=128)
                            nc.vector.tensor_mul(et3[:, 0:4:3, :], et3[:, 0:4:3, :],
                                                 trm[:])
                            stride, width = 256, 256
                        etv = et[:, 0:2 * stride].rearrange(
                            "p (a n) -> p a n", a=2)
                        s_lo = 4 if m2 else 0
                        base = 256 if m2 else 0

                        def attn_emit(et=et, etv=etv, q=q, h=h, po=po,
                                      po_t=po_t, zc=zc,
                                      s_lo=s_lo, base=base, m0=m0, m2=m2, j=j):
                            # exactly ONE start=True per bank-use: start=True
                            # flags the whole 2KB bank pending-zero; every
                            # other group's first touch consumes its flag.
                            if j == 0:
                                # bf16 non-DR path (no fp8 noise on the short-
                                # support rows). (tile, subtiles, flat offset fn)
                                if m0:
                                    work = [(0, range(0, 8), lambda s: 64 * s),
                                            (1, range(2, 8),
                                             lambda s: 512 + 64 * s)]
                                else:
                                    work = [(0, range(4, 8),
                                             lambda s: 64 * s - 256),
                                            (1, range(6, 8), lambda s: 64 * s)]
                                stop_at = {0: (0, 0), 1: (0, 0), 2: (0, 1),
                                           3: (0, 1), 4: (1, 0), 5: (1, 0),
                                           6: (1, 1), 7: (1, 1)}
                                for tt, srange, off in work:
                                    for s in srange:
                                        fo = off(s)
                                        stop = stop_at[s] == (q, tt)
                                        st = (q == 0 and tt == 0 and s == 0)
                                        nc.tensor.matmul(
                                            po[:, 64 * s:64 * s + 64],
                                            et[:, fo:fo + 64],
                                            vb[q][:, tt, h, 0:64],
                                            start=st, stop=stop,
                                            skip_group_check=True)
                                        nc.tensor.matmul(
                                            po_t[64:128, s:s + 1],
                                            et[:, fo:fo + 64],
                                            vb[q][:, tt, h, 64:65],
                                            start=st, stop=stop,
                                            skip_group_check=True)
                                return
                            for s in range(s_lo, 8):
                                cc = 64 * s - base
                                stop = (s < 4 and m0) or (s >= 4 and m2)
                                # tile1's dead block is never read: subtiles
                                # under the diagonal use a single-tile matmul
                                single = (m0 and s < 2) or (m2 and s < 6)
                                if single:
                                    lhs_o = etv[:, 0, cc:cc + 64]
                                    rhs_o = vp[q][:, 0, h, 0:64]
                                    pm = None
                                else:
                                    lhs_o = etv[:, :, cc:cc + 64]
                                    rhs_o = vp[q][:, :, h, 0:64]
                                    pm = DR
                                nc.tensor.matmul(
                                    po[:, 64 * s:64 * s + 64],
                                    lhs_o, rhs_o,
                                    start=(q == 0 and s == 0), stop=stop,
                                    perf_mode=pm, skip_group_check=True)
                                # Z columns live at partitions 64:128 of the
                                # same po bank (single-tile, non-DR: base-64)
                                nc.tensor.matmul(
                                    po_t[64:128, s:s + 1],
                                    etv[:, 0, cc:cc + 64],
                                    vp[q][:, 0, h, 64:65],
                                    start=(q == 0 and s == 0), stop=(stop and single),
                                    skip_group_check=True)
                                if not single:
                                    nc.tensor.matmul(
                                        po_t[64:128, s:s + 1],
                                        etv[:, 1, cc:cc + 64],
                                        vp[q][:, 1, h, 64:65],
                                        start=False, stop=stop,
                                        skip_group_check=True)
                        # one-pair software pipeline: previous pair's attnV/Z
                        # runs while this pair's exp/masks are in flight
                        while pend:
                            pend.pop(0)()
                        pend.append(attn_emit)
                        drain_bg(1)

                    force_bg(lambda t: (t[0] == "vq" and t[1] <= 2 * j + 1) or
                             (j == 0 and t[0] == "vb"))

                    def norm_emit(po=po, po_t=po_t, h=h, hp=hp,
                                  ot_tile=ot_tile):
                        rz = rzp.tile([64, 8], F32, tag="rz", name="rz")
                        nc.vector.reciprocal(rz[:], po_t[64:128, 0:8])
                        o_sb = osb.tile([64, 512], BF16, tag="os", name="os")
                        nc.vector.tensor_tensor(
                            o_sb[:].rearrange("p (e s) -> p e s", e=8),
                            po[:].rearrange("p (e s) -> p e s", e=8),
                            rz[:].unsqueeze(2).broadcast_to([64, 8, 64]),
                            mybir.AluOpType.mult)
                        o_sb_pair[h % 2] = o_sb
                        if h % 2 == 1:
                            def transpose_emit(hp=hp, ot_tile=ot_tile,
                                               pair=dict(o_sb_pair)):
                                pt_t = ps_po.tile([128, 512], F32, tag="po",
                                                  name="pt")
                                pt = pt_t[:].bitcast(BF16)[:, 0:512]
                                for hh in range(2):
                                    for s in range(8):
                                        nc.tensor.matmul(
                                            pt[64 * hh:64 * hh + 64,
                                               64 * s:64 * s + 64],
                                            pair[hh][:, 64 * s:64 * s + 64],
                                            idt[:], is_transpose=True,
                                            start=(s == 0),
                                            stop=(hh == 1 and s == 7),
                                            skip_group_check=True)
                                nc.vector.tensor_copy(ot_tile[:, hp, :], pt[:])
                            bg.insert(min(1, len(bg)),
                                      (("tr", None), transpose_emit))
                    pend.append(norm_emit)

                # flush the pipeline at the j boundary so the last head pair's
                # normalize + transpose are queued before proj tasks
                while pend:
                    pend.pop(0)()
                for t in range(4):
                    for nh in range(2):
                        pending_proj.append((("proj", si),
                                             proj_tile(j, t, nh, ot_tile)))

            while pend:
                pend.pop(0)()
            drain_bg(len(bg))
            for _, task in pending_proj:
                task()

    nc.compile()
    return nc


def _host_prep(x, Wq, Wk, Wv, Wp):
    """Per-core input maps."""
    tri = (np.arange(128)[None, :] >= np.arange(128)[:, None]).astype(np.float32)
    tri2 = np.concatenate([tri, tri], axis=1).astype(f8np)
    ident = np.eye(64, dtype=np.float32).astype(bfnp)
    WpT = np.ascontiguousarray(Wp.T) * (1.0 / SV)  # [E(hd), E]

    def wslice(W, h0, scale, dt=f8np):  # [H,E,D] -> [E, 8*64]
        w = W[h0:h0 + HL].transpose(1, 0, 2).reshape(E, HL * D) * scale
        return np.ascontiguousarray(w).astype(dt)

    in_maps = []
    for c in range(8):
        b, hh = c // 2, c % 2
        h0 = hh * HL
        in_maps.append({
            "xT8": np.ascontiguousarray(x[b].T).astype(f8np),
            "wq8": wslice(Wq, h0, SW),
            "wk8": wslice(Wk, h0, SW),
            "wv8": wslice(Wv, h0, SV),
            "wpT": np.ascontiguousarray(
                WpT[h0 * D:(h0 + HL) * D, :]).astype(bfnp),
            "tri2": tri2,
            "tri2b": tri2.astype(np.float32).astype(bfnp),
            "ident": ident,
            "xbT": np.ascontiguousarray(x[b].T[:, 0:512]).astype(bfnp),
            "wqb": wslice(Wq, h0, SW, bfnp),
            "wkb": wslice(Wk, h0, SW, bfnp),
            "wvb": wslice(Wv, h0, SV, bfnp),
        })
    return in_maps


def kernel(x, Wq, Wk, Wv, Wp, bp):
    x = np.asarray(x, dtype=np.float32)
    Wq = np.asarray(Wq, dtype=np.float32)
    Wk = np.asarray(Wk, dtype=np.float32)
    Wv = np.asarray(Wv, dtype=np.float32)
    Wp = np.asarray(Wp, dtype=np.float32)
    bp = np.asarray(bp, dtype=np.float32)

    if "nc" not in _CACHE:
        _CACHE["nc"] = _build()
    nc = _CACHE["nc"]

    in_maps = _host_prep(x, Wq, Wk, Wv, Wp)
    res = run_bass_kernel_spmd(nc, in_maps, list(range(8)))
    parts = [np.asarray(res.results[c]["out"], dtype=np.float32) for c in range(8)]
    out = np.stack([parts[2 * b] + parts[2 * b + 1] for b in range(B)], axis=0)
    return (out + bp[None, None, :]).astype(np.float32)
